# revision 1
# baseline (speedup 1.0000x reference)
"""Grouped-Query Attention (B=2, T=2048, C=2048, 16 Q heads / 4 KV heads,
D=128) on 8 Trainium2 NeuronCores.

Sharding: core (b, g) for b in {0,1}, g in {0..3} handles batch b and KV head
g (= query heads 4g..4g+3). Each core computes its 4 heads' attention plus the
partial output projection against its 512-row slice of Wo; the host sums the
4 partials per batch (the "all-reduce" of the o_proj, done in numpy).

On-core dataflow (all matmuls in float32r — full-rate fp32 on the PE):
  phase 1:  qT/kT/vT projections, transposed layout [d, t] via
            lhsT=W-tile [c,d-chunk], rhs=xT-tile [c, t-block].
  phase 1.5: RoPE on qT/kT (stream_shuffle pair-swap + cos/sin tables),
             vT -> v natural chunks via TensorE transpose.
  phase 2:  per (head, t-block): scores^T [s, t] = k-chunk^T q, exp on
            ScalarE (scale folded in), optional binary mask multiply,
            denominator via ones-matmul, P@V accumulation -> outT [d, t],
            rescale by 1/denominator (partition_broadcast on GpSimd).
  phase 3:  per t-block: o_proj partial [t, c] accumulated over 4 heads,
            DMA to DRAM.

Masking is specialized at build time from the actual mask input: each
(t-block, s-tile) is classified full / skip / partial (partial tiles get a
host-built 0/1 multiplicative mask applied after exp). mask=all-ones -> no
mask work at all; mask=causal tril -> upper tiles skipped, 4 unique diagonal
mask tiles.
"""
import sys

sys.path.insert(0, "/opt/trn_rl_repo")

import numpy as np

B, T, C = 2, 2048, 2048
NUM_HEADS, NUM_KV_HEADS, HEAD_DIM = 16, 4, 128
G = NUM_HEADS // NUM_KV_HEADS  # 4 query heads per core
SCALE = float(HEAD_DIM) ** -0.5
TB = 512  # t-block (matmul moving free dim)
NTB = T // TB  # 4
ST = 128  # s-tile
NST = T // ST  # 16
NCT = C // 128  # 16 contraction tiles

SWAP_MASK = [i ^ 1 for i in range(32)]

_nc_cache: dict = {}


def _classify_mask(mask2d: np.ndarray):
    """mask2d[t, s] bool. Returns (plan, mask_tiles) where
    plan[tb] = list of (s_tile_idx, mask_id or None) and mask_tiles is a
    float32 array [n, 128, TB] of deduplicated partial-tile masks in the
    transposed [s, t] tile layout."""
    plan = []
    uniq: dict = {}
    tiles = []
    for tb in range(NTB):
        sub_t = mask2d[tb * TB : (tb + 1) * TB]  # [TB, T]
        entries = []
        for s in range(NST):
            sub = sub_t[:, s * ST : (s + 1) * ST]  # [TB(t), ST(s)]
            if sub.all():
                entries.append((s, None))
            elif not sub.any():
                continue
            else:
                tile_m = np.ascontiguousarray(sub.T.astype(np.float32))  # [s, t]
                key = tile_m.tobytes()
                mid = uniq.get(key)
                if mid is None:
                    mid = len(tiles)
                    uniq[key] = mid
                    tiles.append(tile_m)
                entries.append((s, mid))
        plan.append(tuple(entries))
    mask_tiles = (
        np.stack(tiles) if tiles else np.zeros((0, ST, TB), dtype=np.float32)
    )
    return tuple(plan), mask_tiles


def _build(plan, n_masks, use_f32r=True):
    import concourse.bacc as bacc
    import concourse.mybir as mybir
    import concourse.tile as tile
    from concourse.masks import make_identity

    F32 = mybir.dt.float32
    MMDT = mybir.dt.float32r if use_f32r else mybir.dt.float32
    Exp = mybir.ActivationFunctionType.Exp

    nc = bacc.Bacc()

    def mdma(out, in_):
        nc.sync.dma_start(out=out, in_=in_.bitcast(MMDT) if use_f32r else in_)
    xT_d = nc.declare_dram_parameter("xT", [C, T], F32, isOutput=False)
    wq_d = nc.declare_dram_parameter("wq", [C, G * HEAD_DIM], F32, isOutput=False)
    wk_d = nc.declare_dram_parameter("wk", [C, HEAD_DIM], F32, isOutput=False)
    wv_d = nc.declare_dram_parameter("wv", [C, HEAD_DIM], F32, isOutput=False)
    wo_d = nc.declare_dram_parameter("wo", [G * HEAD_DIM, C], F32, isOutput=False)
    on_d = nc.declare_dram_parameter("ones", [128, 1], F32, isOutput=False)
    ct_d = nc.declare_dram_parameter("ctab", [HEAD_DIM, T], F32, isOutput=False)
    st_d = nc.declare_dram_parameter("stab", [HEAD_DIM, T], F32, isOutput=False)
    if n_masks:
        mk_d = nc.declare_dram_parameter(
            "masks", [n_masks * ST, TB], F32, isOutput=False
        )
    out_d = nc.declare_dram_parameter("out", [T, C], F32, isOutput=True)

    with tile.TileContext(nc) as tc:
        const = tc.alloc_tile_pool(name="const", bufs=1)
        qkv = tc.alloc_tile_pool(name="qkv", bufs=1)
        wop = tc.alloc_tile_pool(name="wop", bufs=1)
        outp = tc.alloc_tile_pool(name="outp", bufs=8)

        ones_sb = const.tile([128, 1], MMDT, name="ones_sb")
        mdma(ones_sb, on_d.ap())
        ident = const.tile([128, 128], F32, name="ident")
        make_identity(nc, ident)
        ctab = const.tile([HEAD_DIM, T], F32, name="ctab")
        stab = const.tile([HEAD_DIM, T], F32, name="stab")
        nc.sync.dma_start(out=ctab, in_=ct_d.ap())
        nc.sync.dma_start(out=stab, in_=st_d.ap())
        if n_masks:
            msk_sb = const.tile([ST, n_masks * TB], F32, name="msk_sb")
            for i in range(n_masks):
                nc.sync.dma_start(
                    out=msk_sb[:, i * TB : (i + 1) * TB],
                    in_=mk_d.ap()[i * ST : (i + 1) * ST, :],
                )

        qT = [qkv.tile([128, T], MMDT, name=f"qT{h}") for h in range(G)]
        kT = qkv.tile([128, T], MMDT, name="kT")
        vch = [qkv.tile([128, 128], MMDT, name=f"v{s}") for s in range(NST)]

        wo_sb = [wop.tile([128, C], MMDT, name=f"wo{h}") for h in range(G)]
        for h in range(G):
            mdma(wo_sb[h], wo_d.ap()[h * 128 : (h + 1) * 128, :])

        # ---- phase 1: projections (transposed outputs) ----
        wpool = tc.alloc_tile_pool(name="wpool", bufs=1)
        xs = tc.alloc_tile_pool(name="xs", bufs=4)
        p1ps = tc.alloc_tile_pool(name="p1ps", bufs=1, space="PSUM")

        wq_sb = [wpool.tile([128, G * HEAD_DIM], MMDT, name=f"wq{i}") for i in range(NCT)]
        wk_sb = [wpool.tile([128, HEAD_DIM], MMDT, name=f"wk{i}") for i in range(NCT)]
        wv_sb = [wpool.tile([128, HEAD_DIM], MMDT, name=f"wv{i}") for i in range(NCT)]
        vT = wpool.tile([128, T], F32, name="vT")
        for i in range(NCT):
            sl = slice(i * 128, (i + 1) * 128)
            mdma(wq_sb[i], wq_d.ap()[sl, :])
            mdma(wk_sb[i], wk_d.ap()[sl, :])
            mdma(wv_sb[i], wv_d.ap()[sl, :])

        for tb in range(NTB):
            tsl = slice(tb * TB, (tb + 1) * TB)
            q_ps = [
                p1ps.tile([128, TB], F32, name=f"qps{h}", tag=f"qps{h}")
                for h in range(G)
            ]
            k_ps = p1ps.tile([128, TB], F32, name="kps", tag="kps")
            v_ps = p1ps.tile([128, TB], F32, name="vps", tag="vps")
            for ci in range(NCT):
                xt = xs.tile([128, TB], MMDT, name="xt", tag="xt")
                mdma(xt, xT_d.ap()[ci * 128 : (ci + 1) * 128, tsl])
                first, last = ci == 0, ci == NCT - 1
                for h in range(G):
                    nc.tensor.matmul(
                        q_ps[h],
                        lhsT=wq_sb[ci][:, h * 128 : (h + 1) * 128],
                        rhs=xt,
                        start=first,
                        stop=last,
                    )
                nc.tensor.matmul(
                    k_ps, lhsT=wk_sb[ci], rhs=xt, start=first, stop=last
                )
                nc.tensor.matmul(
                    v_ps, lhsT=wv_sb[ci], rhs=xt, start=first, stop=last
                )
            for h in range(G):
                nc.vector.tensor_copy(qT[h][:, tsl], q_ps[h])
            nc.vector.tensor_copy(kT[:, tsl], k_ps)
            nc.vector.tensor_copy(vT[:, tsl], v_ps)

        # ---- phase 1.5: RoPE on qT/kT; transpose vT -> v natural chunks ----
        rpool = tc.alloc_tile_pool(name="rpool", bufs=3)
        p15ps = tc.alloc_tile_pool(name="p15ps", bufs=2, space="PSUM")
        for src in qT + [kT]:
            for tb in range(NTB):
                tsl = slice(tb * TB, (tb + 1) * TB)
                swp = rpool.tile([128, TB], F32, name="swp", tag="swp")
                tmp = rpool.tile([128, TB], F32, name="tmp", tag="tmp")
                nc.vector.stream_shuffle(swp, src[:, tsl], SWAP_MASK)
                nc.vector.tensor_mul(tmp, src[:, tsl], ctab[:, tsl])
                nc.vector.tensor_mul(swp, swp, stab[:, tsl])
                nc.vector.tensor_add(src[:, tsl], tmp, swp)
        for s in range(NST):
            vtp = p15ps.tile([128, 128], F32, name="vtp", tag="vtp")
            nc.tensor.transpose(vtp, vT[:, s * 128 : (s + 1) * 128], ident)
            nc.vector.tensor_copy(vch[s], vtp)

        rpool.release()
        xs.release()
        wpool.release()
        p15ps.release()
        p1ps.release()

        # ---- phases 2+3 ----
        p2sb = tc.alloc_tile_pool(name="p2sb", bufs=3)
        p2ps = tc.alloc_tile_pool(name="p2ps", bufs=1, space="PSUM")
        p3sb = tc.alloc_tile_pool(name="p3sb", bufs=3)

        for tb in range(NTB):
            tsl = slice(tb * TB, (tb + 1) * TB)
            entries = plan[tb]
            oT_sbs = []
            for h in range(G):
                if not entries:
                    oT_sb = outp.tile([128, TB], MMDT, name="oT", tag="oT")
                    nc.gpsimd.memset(oT_sb, 0.0)
                    oT_sbs.append(oT_sb)
                    continue
                oT_ps = p2ps.tile([128, TB], F32, name="oTps", tag="oTps", bufs=2)
                den = p2ps.tile([1, TB], F32, name="den", tag="den", bufs=2)
                n_e = len(entries)
                for idx, (s, mid) in enumerate(entries):
                    stp = p2ps.tile([128, TB], F32, name="stp", tag="stp", bufs=2)
                    nc.tensor.matmul(
                        stp,
                        lhsT=kT[:, s * 128 : (s + 1) * 128],
                        rhs=qT[h][:, tsl],
                        start=True,
                        stop=True,
                    )
                    ep = p2sb.tile([ST, TB], MMDT, name="ep", tag="ep")
                    nc.scalar.activation(ep, stp, Exp, scale=SCALE)
                    if mid is not None:
                        nc.vector.tensor_mul(
                            ep, ep, msk_sb[:, mid * TB : (mid + 1) * TB]
                        )
                    first, last = idx == 0, idx == n_e - 1
                    nc.tensor.matmul(
                        den, lhsT=ones_sb, rhs=ep, start=first, stop=last
                    )
                    nc.tensor.matmul(
                        oT_ps, lhsT=vch[s], rhs=ep, start=first, stop=last
                    )
                rcp = p2sb.tile([1, TB], F32, name="rcp", tag="rcp")
                nc.vector.reciprocal(rcp, den)
                rb = p2sb.tile([128, TB], F32, name="rb", tag="rb")
                nc.gpsimd.partition_broadcast(rb, rcp)
                oT_sb = outp.tile([128, TB], MMDT, name="oT", tag="oT")
                nc.vector.tensor_mul(oT_sb, oT_ps, rb)
                oT_sbs.append(oT_sb)

            # o_proj partial for this t-block
            for cb in range(C // 512):
                for tch in range(TB // 128):
                    ops = p2ps.tile([128, 512], F32, name="ops", tag="ops", bufs=2)
                    for h in range(G):
                        nc.tensor.matmul(
                            ops,
                            lhsT=oT_sbs[h][:, tch * 128 : (tch + 1) * 128],
                            rhs=wo_sb[h][:, cb * 512 : (cb + 1) * 512],
                            start=h == 0,
                            stop=h == G - 1,
                        )
                    osb = p3sb.tile([128, 512], F32, name="osb", tag="osb")
                    nc.vector.tensor_copy(osb, ops)
                    t0 = tb * TB + tch * 128
                    nc.sync.dma_start(
                        out=out_d.ap()[t0 : t0 + 128, cb * 512 : (cb + 1) * 512],
                        in_=osb,
                    )

        p3sb.release()
        p2sb.release()
        p2ps.release()
        outp.release()
        wop.release()
        qkv.release()
        const.release()

    nc.compile()
    return nc


def _prep_inputs(x, cos, sin, Wq, Wk, Wv, Wo, mask_tiles, n_masks):
    cos = np.asarray(cos, dtype=np.float32).reshape(T, HEAD_DIM // 2)
    sin = np.asarray(sin, dtype=np.float32).reshape(T, HEAD_DIM // 2)
    ctab = np.ascontiguousarray(np.repeat(cos, 2, axis=1).T)  # [128, T]
    s2 = np.repeat(sin, 2, axis=1)
    s2[:, 0::2] *= -1.0
    stab = np.ascontiguousarray(s2.T)

    in_maps = []
    for core in range(8):
        b, g = divmod(core, NUM_KV_HEADS)
        m = {
            "xT": np.ascontiguousarray(np.asarray(x[b], dtype=np.float32).T),
            "wq": np.ascontiguousarray(Wq[:, g * 512 : (g + 1) * 512]).astype(
                np.float32
            ),
            "wk": np.ascontiguousarray(Wk[:, g * 128 : (g + 1) * 128]).astype(
                np.float32
            ),
            "wv": np.ascontiguousarray(Wv[:, g * 128 : (g + 1) * 128]).astype(
                np.float32
            ),
            "wo": np.ascontiguousarray(Wo[g * 512 : (g + 1) * 512, :]).astype(
                np.float32
            ),
            "ctab": ctab,
            "stab": stab,
            "ones": np.ones((128, 1), dtype=np.float32),
        }
        if n_masks:
            m["masks"] = mask_tiles.reshape(n_masks * ST, TB)
        in_maps.append(m)
    return in_maps


def kernel(x, cos, sin, mask, Wq, Wk, Wv, Wo, _trace=False, _result_box=None):
    from concourse.bass_utils import run_bass_kernel_spmd

    mask2d = np.asarray(mask).reshape(T, T).astype(bool)
    plan, mask_tiles = _classify_mask(mask2d)
    n_masks = int(mask_tiles.shape[0])

    use_f32r = True
    key = (plan, n_masks, use_f32r)
    nc = _nc_cache.get(key)
    if nc is None:
        nc = _build(plan, n_masks, use_f32r=use_f32r)
        _nc_cache[key] = nc

    in_maps = _prep_inputs(x, cos, sin, Wq, Wk, Wv, Wo, mask_tiles, n_masks)
    res = run_bass_kernel_spmd(nc, in_maps, core_ids=list(range(8)), trace=_trace)
    if _result_box is not None:
        _result_box.append(res)

    out = np.zeros((B, T, C), dtype=np.float32)
    for core in range(8):
        b = core // NUM_KV_HEADS
        out[b] += res.results[core]["out"]
    return out



# revision 2
# speedup vs baseline: 1.3568x; 1.3568x over previous
"""Grouped-Query Attention (B=2, T=2048, C=2048, 16 Q heads / 4 KV heads,
D=128) on 8 Trainium2 NeuronCores.

Sharding: core (b, g) for b in {0,1}, g in {0..3} handles batch b and KV head
g (= query heads 4g..4g+3). Each core computes its 4 heads' attention plus the
partial output projection against its 512-row slice of Wo; the host sums the
4 partials per batch (the "all-reduce" of the o_proj, done in numpy).

All matmul operands are bf16 (host-cast); PSUM accumulation stays fp32, so
the only precision loss is input rounding (~4e-3 rel err vs the 2e-2 gate).

On-core dataflow, pipelined per 512-wide t-block:
  stage 1 (per tb): qT/kT/vT projections into PSUM via lhsT=W-chunk,
          rhs=resident xT tile; RoPE (DVE: copy + stream_shuffle pair-swap +
          cos/sin muls) writes bf16 qT/kT; vT chunks transposed (TensorE) to
          natural-layout v tiles. Next tb's projections overlap this tb's
          RoPE on the vector engine.
  stage 2 (per tb, after all projections): per (head, s-tile): scores^T
          [s, t-window] = k-chunk^T q, exp on ScalarE (scale folded in),
          diagonal tiles use a t-window (upper-triangle tiles never computed)
          plus one shared [128,128] triangular 0/1 multiply; denominator via
          ones-matmul, P@V accumulation -> oT [d, t], rescale by 1/den
          (partition_broadcast on GpSimd).
  stage 3 (per tb): o_proj partial [t, c] accumulated over 4 heads; PSUM ->
          SBUF copies alternate Scalar/Vector engines; DMA to DRAM fp32.

Masking is specialized at build time from the actual mask input: each
(t-block, s-tile) is classified full / skip / diag-window / general-partial.
mask=all-ones -> no mask work; causal tril -> windowed diagonal tiles with
the shared triangular multiply.
"""
import sys

sys.path.insert(0, "/opt/trn_rl_repo")

import numpy as np
import ml_dtypes

B, T, C = 2, 2048, 2048
NUM_HEADS, NUM_KV_HEADS, HEAD_DIM = 16, 4, 128
G = NUM_HEADS // NUM_KV_HEADS  # 4 query heads per core
SCALE = float(HEAD_DIM) ** -0.5
TB = 512  # t-block (matmul moving free dim)
NTB = T // TB  # 4
ST = 128  # s-tile
NST = T // ST  # 16
NCT = C // 128  # 16 contraction tiles

SWAP_MASK = [i ^ 1 for i in range(32)]
BF = ml_dtypes.bfloat16

_nc_cache: dict = {}

# plan entry kinds
FULL, DIAG, GEN = 0, 1, 2


def _classify_mask(mask2d: np.ndarray):
    """mask2d[t, s] bool. Returns (plan, mask_tiles).

    plan[tb] = tuple of (s_tile_idx, w0, kind, mask_id). w0 is the t-window
    start within the t-block (columns < w0 are entirely masked for this
    s-tile). kind: FULL (no mask work), DIAG (shared lower-triangular 0/1
    multiply on the first 128 window columns), GEN (per-tile 0/1 multiply
    over the whole window; mask_id indexes mask_tiles).

    mask_tiles: float32 [n, ST, TB] deduplicated general tiles in the
    transposed [s, t] layout (windowed part at [:, :TB-w0])."""
    # the shared diagonal pattern: allowed iff s_local <= t_local
    tri = (np.arange(ST)[:, None] <= np.arange(ST)[None, :])
    plan = []
    uniq: dict = {}
    tiles = []
    for tb in range(NTB):
        sub_t = mask2d[tb * TB : (tb + 1) * TB]  # [TB(t), T(s)]
        entries = []
        for s in range(NST):
            sub = sub_t[:, s * ST : (s + 1) * ST]  # [TB(t), ST(s)]
            if sub.all():
                entries.append((s, 0, FULL, None))
                continue
            if not sub.any():
                continue
            m = sub.T  # [s, t]
            # widest all-masked prefix of whole 128-columns
            w0 = 0
            while w0 + ST <= TB and not m[:, w0 : w0 + ST].any():
                w0 += ST
            win = m[:, w0:]
            if (
                win.shape[1] >= ST
                and (win[:, :ST] == tri).all()
                and win[:, ST:].all()
            ):
                entries.append((s, w0, DIAG, None))
                continue
            tile_m = np.zeros((ST, TB), dtype=np.float32)
            tile_m[:, : TB - w0] = win.astype(np.float32)
            key = (w0, tile_m.tobytes())
            mid = uniq.get(key)
            if mid is None:
                mid = len(tiles)
                uniq[key] = mid
                tiles.append(tile_m)
            entries.append((s, w0, GEN, mid))
        plan.append(tuple(entries))
    mask_tiles = (
        np.stack(tiles) if tiles else np.zeros((0, ST, TB), dtype=np.float32)
    )
    return tuple(plan), mask_tiles


def _build(plan, n_masks):
    import concourse.bacc as bacc
    import concourse.mybir as mybir
    import concourse.tile as tile

    F32 = mybir.dt.float32
    BF16 = mybir.dt.bfloat16
    Exp = mybir.ActivationFunctionType.Exp

    nc = bacc.Bacc()

    xT_d = nc.declare_dram_parameter("xT", [C, T], BF16, isOutput=False)
    wq_d = nc.declare_dram_parameter("wq", [C, G * HEAD_DIM], BF16, isOutput=False)
    wk_d = nc.declare_dram_parameter("wk", [C, HEAD_DIM], BF16, isOutput=False)
    wv_d = nc.declare_dram_parameter("wv", [C, HEAD_DIM], BF16, isOutput=False)
    wo_d = nc.declare_dram_parameter("wo", [G * HEAD_DIM, C], BF16, isOutput=False)
    on_d = nc.declare_dram_parameter("ones", [128, 1], BF16, isOutput=False)
    id_d = nc.declare_dram_parameter("ident", [128, 128], BF16, isOutput=False)
    tr_d = nc.declare_dram_parameter("tri", [ST, ST], BF16, isOutput=False)
    ct_d = nc.declare_dram_parameter("ctab", [HEAD_DIM, T], BF16, isOutput=False)
    st_d = nc.declare_dram_parameter("stab", [HEAD_DIM, T], BF16, isOutput=False)
    if n_masks:
        mk_d = nc.declare_dram_parameter(
            "masks", [n_masks * ST, TB], BF16, isOutput=False
        )
    out_d = nc.declare_dram_parameter("out", [T, C], F32, isOutput=True)

    with tile.TileContext(nc) as tc:
        const = tc.alloc_tile_pool(name="const", bufs=1)
        wop = tc.alloc_tile_pool(name="wop", bufs=1)
        qkv = tc.alloc_tile_pool(name="qkv", bufs=1)
        xp = tc.alloc_tile_pool(name="xp", bufs=1)

        # --- weights first (first matmuls need them) ---
        wq_sb = [wop.tile([128, G * HEAD_DIM], BF16, name=f"wq{i}") for i in range(NCT)]
        wk_sb = [wop.tile([128, HEAD_DIM], BF16, name=f"wk{i}") for i in range(NCT)]
        wv_sb = [wop.tile([128, HEAD_DIM], BF16, name=f"wv{i}") for i in range(NCT)]
        for i in range(NCT):
            sl = slice(i * 128, (i + 1) * 128)
            nc.sync.dma_start(out=wq_sb[i], in_=wq_d.ap()[sl, :])
            nc.sync.dma_start(out=wk_sb[i], in_=wk_d.ap()[sl, :])
            nc.sync.dma_start(out=wv_sb[i], in_=wv_d.ap()[sl, :])

        # resident xT: one [128, T] tile per contraction chunk
        xts = [xp.tile([128, T], BF16, name=f"xt{i}") for i in range(NCT)]
        for i in range(NCT):
            nc.sync.dma_start(out=xts[i], in_=xT_d.ap()[i * 128 : (i + 1) * 128, :])

        ctab = const.tile([HEAD_DIM, T], BF16, name="ctab")
        stab = const.tile([HEAD_DIM, T], BF16, name="stab")
        nc.sync.dma_start(out=ctab, in_=ct_d.ap())
        nc.sync.dma_start(out=stab, in_=st_d.ap())
        ones_sb = const.tile([128, 1], BF16, name="ones_sb")
        ident = const.tile([128, 128], BF16, name="ident")
        tri_sb = const.tile([ST, ST], BF16, name="tri_sb")
        nc.sync.dma_start(out=ones_sb, in_=on_d.ap())
        nc.sync.dma_start(out=ident, in_=id_d.ap())
        nc.sync.dma_start(out=tri_sb, in_=tr_d.ap())
        if n_masks:
            msk_sb = const.tile([ST, n_masks * TB], BF16, name="msk_sb")
            for i in range(n_masks):
                nc.sync.dma_start(
                    out=msk_sb[:, i * TB : (i + 1) * TB],
                    in_=mk_d.ap()[i * ST : (i + 1) * ST, :],
                )
        wo_sb = [wop.tile([128, C], BF16, name=f"wo{h}") for h in range(G)]
        for h in range(G):
            nc.sync.dma_start(out=wo_sb[h], in_=wo_d.ap()[h * 128 : (h + 1) * 128, :])

        qT = [qkv.tile([128, T], BF16, name=f"qT{h}") for h in range(G)]
        kT = qkv.tile([128, T], BF16, name="kT")
        vT = qkv.tile([128, T], BF16, name="vT")
        vch = [qkv.tile([128, 128], BF16, name=f"v{s}") for s in range(NST)]

        # ---- stage 1: projections + RoPE + v transpose, pipelined per tb ----
        p1ps = tc.alloc_tile_pool(name="p1ps", bufs=1, space="PSUM")
        p15ps = tc.alloc_tile_pool(name="p15ps", bufs=2, space="PSUM")
        rpool = tc.alloc_tile_pool(name="rpool", bufs=3)

        for tb in range(NTB):
            tsl = slice(tb * TB, (tb + 1) * TB)
            q_ps = [
                p1ps.tile([128, TB], F32, name=f"qps{h}", tag=f"qps{h}")
                for h in range(G)
            ]
            k_ps = p1ps.tile([128, TB], F32, name="kps", tag="kps")
            v_ps = p1ps.tile([128, TB], F32, name="vps", tag="vps")
            for ci in range(NCT):
                first, last = ci == 0, ci == NCT - 1
                rhs = xts[ci][:, tsl]
                for h in range(G):
                    nc.tensor.matmul(
                        q_ps[h],
                        lhsT=wq_sb[ci][:, h * 128 : (h + 1) * 128],
                        rhs=rhs,
                        start=first,
                        stop=last,
                    )
                nc.tensor.matmul(k_ps, lhsT=wk_sb[ci], rhs=rhs, start=first, stop=last)
                nc.tensor.matmul(v_ps, lhsT=wv_sb[ci], rhs=rhs, start=first, stop=last)

            # RoPE on k first (attention waits on k), then qs; v last
            for dst, src_ps in [(kT, k_ps)] + [(qT[h], q_ps[h]) for h in range(G)]:
                nc.vector.tensor_copy(dst[:, tsl], src_ps)
                swp = rpool.tile([128, TB], BF16, name="swp", tag="swp")
                tmp = rpool.tile([128, TB], BF16, name="tmp", tag="tmp")
                nc.vector.stream_shuffle(swp, dst[:, tsl], SWAP_MASK)
                nc.vector.tensor_mul(tmp, dst[:, tsl], ctab[:, tsl])
                nc.vector.tensor_mul(swp, swp, stab[:, tsl])
                nc.vector.tensor_add(dst[:, tsl], tmp, swp)
            nc.vector.tensor_copy(vT[:, tsl], v_ps)
            for r in range(4):
                s = 4 * tb + r
                vtp = p15ps.tile([128, 128], BF16, name="vtp", tag="vtp")
                nc.tensor.transpose(vtp, vT[:, s * 128 : (s + 1) * 128], ident)
                nc.vector.tensor_copy(vch[s], vtp)

        rpool.release()
        p15ps.release()
        p1ps.release()
        xp.release()

        # ---- stages 2+3 ----
        p2sb = tc.alloc_tile_pool(name="p2sb", bufs=3)
        p2ps = tc.alloc_tile_pool(name="p2ps", bufs=1, space="PSUM")
        p3sb = tc.alloc_tile_pool(name="p3sb", bufs=3)
        outp = tc.alloc_tile_pool(name="outp", bufs=8)
        ncopy = 0

        for tb in range(NTB):
            tsl = slice(tb * TB, (tb + 1) * TB)
            entries = plan[tb]
            oT_sbs = []
            for h in range(G):
                if not entries:
                    oT_sb = outp.tile([128, TB], BF16, name="oT", tag="oT")
                    nc.gpsimd.memset(oT_sb, 0.0)
                    oT_sbs.append(oT_sb)
                    continue
                oT_ps = p2ps.tile([128, TB], F32, name="oTps", tag="oTps", bufs=2)
                den = p2ps.tile([1, TB], F32, name="den", tag="den", bufs=2)
                n_e = len(entries)
                for idx, (s, w0, kind, mid) in enumerate(entries):
                    w = TB - w0
                    stp = p2ps.tile([128, TB], F32, name="stp", tag="stp", bufs=2)
                    nc.tensor.matmul(
                        stp[:, w0:],
                        lhsT=kT[:, s * 128 : (s + 1) * 128],
                        rhs=qT[h][:, tb * TB + w0 : (tb + 1) * TB],
                        start=True,
                        stop=True,
                    )
                    ep = p2sb.tile([ST, TB], BF16, name="ep", tag="ep")
                    nc.scalar.activation(ep[:, w0:], stp[:, w0:], Exp, scale=SCALE)
                    if kind == DIAG:
                        nc.vector.tensor_mul(
                            ep[:, w0 : w0 + ST], ep[:, w0 : w0 + ST], tri_sb
                        )
                    elif kind == GEN:
                        nc.vector.tensor_mul(
                            ep[:, w0:], ep[:, w0:], msk_sb[:, mid * TB : mid * TB + w]
                        )
                    first, last = idx == 0, idx == n_e - 1
                    nc.tensor.matmul(
                        den[:, w0:],
                        lhsT=ones_sb,
                        rhs=ep[:, w0:],
                        start=first,
                        stop=last,
                        skip_group_check=True,
                    )
                    nc.tensor.matmul(
                        oT_ps[:, w0:],
                        lhsT=vch[s],
                        rhs=ep[:, w0:],
                        start=first,
                        stop=last,
                        skip_group_check=True,
                    )
                rcp = p2sb.tile([1, TB], F32, name="rcp", tag="rcp")
                nc.vector.reciprocal(rcp, den)
                rb = p2sb.tile([128, TB], F32, name="rb", tag="rb")
                nc.gpsimd.partition_broadcast(rb, rcp)
                oT_sb = outp.tile([128, TB], BF16, name="oT", tag="oT")
                nc.vector.tensor_mul(oT_sb, oT_ps, rb)
                oT_sbs.append(oT_sb)

            # o_proj partial for this t-block
            for tch in range(TB // 128):
                for cb in range(C // 512):
                    ops = p2ps.tile([128, 512], F32, name="ops", tag="ops", bufs=2)
                    for h in range(G):
                        nc.tensor.matmul(
                            ops,
                            lhsT=oT_sbs[h][:, tch * 128 : (tch + 1) * 128],
                            rhs=wo_sb[h][:, cb * 512 : (cb + 1) * 512],
                            start=h == 0,
                            stop=h == G - 1,
                        )
                    osb = p3sb.tile([128, 512], F32, name="osb", tag="osb")
                    if ncopy % 2 == 0:
                        nc.scalar.copy(osb, ops)
                    else:
                        nc.vector.tensor_copy(osb, ops)
                    ncopy += 1
                    t0 = tb * TB + tch * 128
                    nc.sync.dma_start(
                        out=out_d.ap()[t0 : t0 + 128, cb * 512 : (cb + 1) * 512],
                        in_=osb,
                    )

        outp.release()
        p3sb.release()
        p2ps.release()
        p2sb.release()
        qkv.release()
        wop.release()
        const.release()

    nc.compile()
    return nc


def _prep_inputs(x, cos, sin, Wq, Wk, Wv, Wo, mask_tiles, n_masks):
    cos = np.asarray(cos, dtype=np.float32).reshape(T, HEAD_DIM // 2)
    sin = np.asarray(sin, dtype=np.float32).reshape(T, HEAD_DIM // 2)
    ctab = np.ascontiguousarray(np.repeat(cos, 2, axis=1).T).astype(BF)  # [128, T]
    s2 = np.repeat(sin, 2, axis=1)
    s2[:, 0::2] *= -1.0
    stab = np.ascontiguousarray(s2.T).astype(BF)
    tri = (np.arange(ST)[:, None] <= np.arange(ST)[None, :]).astype(BF)

    xTb = [
        np.ascontiguousarray(np.asarray(x[b], dtype=np.float32).T).astype(BF)
        for b in range(B)
    ]
    in_maps = []
    for core in range(8):
        b, g = divmod(core, NUM_KV_HEADS)
        m = {
            "xT": xTb[b],
            "wq": np.ascontiguousarray(Wq[:, g * 512 : (g + 1) * 512]).astype(BF),
            "wk": np.ascontiguousarray(Wk[:, g * 128 : (g + 1) * 128]).astype(BF),
            "wv": np.ascontiguousarray(Wv[:, g * 128 : (g + 1) * 128]).astype(BF),
            "wo": np.ascontiguousarray(Wo[g * 512 : (g + 1) * 512, :]).astype(BF),
            "ctab": ctab,
            "stab": stab,
            "ones": np.ones((128, 1), dtype=BF),
            "ident": np.eye(128, dtype=BF),
            "tri": tri,
        }
        if n_masks:
            m["masks"] = mask_tiles.reshape(n_masks * ST, TB).astype(BF)
        in_maps.append(m)
    return in_maps


def kernel(x, cos, sin, mask, Wq, Wk, Wv, Wo, _trace=False, _result_box=None):
    from concourse.bass_utils import run_bass_kernel_spmd

    mask2d = np.asarray(mask).reshape(T, T).astype(bool)
    plan, mask_tiles = _classify_mask(mask2d)
    n_masks = int(mask_tiles.shape[0])

    key = (plan, n_masks)
    nc = _nc_cache.get(key)
    if nc is None:
        nc = _build(plan, n_masks)
        _nc_cache[key] = nc

    in_maps = _prep_inputs(x, cos, sin, Wq, Wk, Wv, Wo, mask_tiles, n_masks)
    res = run_bass_kernel_spmd(nc, in_maps, core_ids=list(range(8)), trace=_trace)
    if _result_box is not None:
        _result_box.append(res)

    out = np.zeros((B, T, C), dtype=np.float32)
    for core in range(8):
        b = core // NUM_KV_HEADS
        out[b] += res.results[core]["out"]
    return out


# revision 4
# speedup vs baseline: 1.6100x; 1.1866x over previous
"""Grouped-Query Attention (B=2, T=2048, C=2048, 16 Q heads / 4 KV heads,
D=128) on 8 Trainium2 NeuronCores.

Sharding: core (b, g) for b in {0,1}, g in {0..3} handles batch b and KV head
g (= query heads 4g..4g+3). Each core computes its 4 heads' attention plus the
partial output projection against its 512-row slice of Wo; the host sums the
4 partials per batch (the "all-reduce" of the o_proj, done in numpy).

All matmul operands are bf16 (host-cast); PSUM accumulation stays fp32, so
the only precision loss is input rounding (~4e-3 rel err vs the 2e-2 gate).

Layout/scheduling notes (from trace analysis):
- One PSUM pool with 8 [128,512]-f32 bank tags reused across stages (no
  mid-kernel pool releases -> no cross-stage drain bubbles; the PE pstate
  ramp resets on idle gaps, so a dense PE queue is worth ~1.5x clock).
- Startup DMAs interleaved per contraction chunk (wq/wk/wv/x) so the first
  projection matmul unblocks after ~4 transfers instead of all weights.
- Softmax denominator is computed REPLICATED across all 128 partitions
  (lhsT = all-ones [128,128]) so the reciprocal runs as a full-width DVE op
  (~0.65us) instead of a 1-partition op (3.3us) + GpSimd partition
  broadcast; the per-head tail stall on the PE disappears.
- Stage-2 software pipeline: score matmuls run 2 s-tiles ahead of the
  dependent den/PV matmuls so the PE never waits on ScalarE's exp.
- Diagonal (causal-boundary) s-tiles only compute the t-window right of the
  diagonal plus one shared [128,128] triangular 0/1 multiply.
"""
import sys

sys.path.insert(0, "/opt/trn_rl_repo")

import numpy as np
import ml_dtypes

B, T, C = 2, 2048, 2048
NUM_HEADS, NUM_KV_HEADS, HEAD_DIM = 16, 4, 128
G = NUM_HEADS // NUM_KV_HEADS  # 4 query heads per core
SCALE = float(HEAD_DIM) ** -0.5
TB = 512  # t-block (matmul moving free dim)
NTB = T // TB  # 4
ST = 128  # s-tile
NST = T // ST  # 16
NCT = C // 128  # 16 contraction tiles
LA = 2  # stage-2 score-matmul lookahead (s-tiles in flight past exp)

SWAP_MASK = [i ^ 1 for i in range(32)]
BF = ml_dtypes.bfloat16

_nc_cache: dict = {}

# plan entry kinds
FULL, DIAG, GEN = 0, 1, 2


def _classify_mask(mask2d: np.ndarray):
    """mask2d[t, s] bool. Returns (plan, mask_tiles).

    plan[tb] = tuple of (s_tile_idx, w0, kind, mask_id). w0 is the t-window
    start within the t-block (columns < w0 are entirely masked for this
    s-tile). kind: FULL (no mask work), DIAG (shared lower-triangular 0/1
    multiply on the first 128 window columns), GEN (per-tile 0/1 multiply
    over the whole window; mask_id indexes mask_tiles)."""
    tri = (np.arange(ST)[:, None] <= np.arange(ST)[None, :])
    plan = []
    uniq: dict = {}
    tiles = []
    for tb in range(NTB):
        sub_t = mask2d[tb * TB : (tb + 1) * TB]  # [TB(t), T(s)]
        entries = []
        for s in range(NST):
            sub = sub_t[:, s * ST : (s + 1) * ST]  # [TB(t), ST(s)]
            if sub.all():
                entries.append((s, 0, FULL, None))
                continue
            if not sub.any():
                continue
            m = sub.T  # [s, t]
            w0 = 0
            while w0 + ST <= TB and not m[:, w0 : w0 + ST].any():
                w0 += ST
            win = m[:, w0:]
            if (
                win.shape[1] >= ST
                and (win[:, :ST] == tri).all()
                and win[:, ST:].all()
            ):
                entries.append((s, w0, DIAG, None))
                continue
            tile_m = np.zeros((ST, TB), dtype=np.float32)
            tile_m[:, : TB - w0] = win.astype(np.float32)
            key = (w0, tile_m.tobytes())
            mid = uniq.get(key)
            if mid is None:
                mid = len(tiles)
                uniq[key] = mid
                tiles.append(tile_m)
            entries.append((s, w0, GEN, mid))
        plan.append(tuple(entries))
    mask_tiles = (
        np.stack(tiles) if tiles else np.zeros((0, ST, TB), dtype=np.float32)
    )
    return tuple(plan), mask_tiles


def _build(plan, n_masks):
    import concourse.bacc as bacc
    import concourse.mybir as mybir
    import concourse.tile as tile

    F32 = mybir.dt.float32
    BF16 = mybir.dt.bfloat16
    Exp = mybir.ActivationFunctionType.Exp

    nc = bacc.Bacc()

    xT_d = nc.declare_dram_parameter("xT", [C, T], BF16, isOutput=False)
    wq_d = nc.declare_dram_parameter("wq", [C, G * HEAD_DIM], BF16, isOutput=False)
    wk_d = nc.declare_dram_parameter("wk", [C, HEAD_DIM], BF16, isOutput=False)
    wv_d = nc.declare_dram_parameter("wv", [C, HEAD_DIM], BF16, isOutput=False)
    wo_d = nc.declare_dram_parameter("wo", [G * HEAD_DIM, C], BF16, isOutput=False)
    on_d = nc.declare_dram_parameter("ones", [128, 128], BF16, isOutput=False)
    id_d = nc.declare_dram_parameter("ident", [128, 128], BF16, isOutput=False)
    tr_d = nc.declare_dram_parameter("tri", [ST, ST], BF16, isOutput=False)
    ct_d = nc.declare_dram_parameter("ctab", [HEAD_DIM, T], BF16, isOutput=False)
    st_d = nc.declare_dram_parameter("stab", [HEAD_DIM, T], BF16, isOutput=False)
    if n_masks:
        mk_d = nc.declare_dram_parameter(
            "masks", [n_masks * ST, TB], BF16, isOutput=False
        )
    out_d = nc.declare_dram_parameter("out", [T, C], F32, isOutput=True)

    with tile.TileContext(nc) as tc:
        const = tc.alloc_tile_pool(name="const", bufs=1)
        wop = tc.alloc_tile_pool(name="wop", bufs=1)
        qkv = tc.alloc_tile_pool(name="qkv", bufs=1)
        xp = tc.alloc_tile_pool(name="xp", bufs=1)

        # --- interleaved startup DMAs: per-chunk weights + x so the first
        # projection matmuls unblock after a handful of transfers ---
        wq_sb = [wop.tile([128, G * HEAD_DIM], BF16, name=f"wq{i}") for i in range(NCT)]
        wk_sb = [wop.tile([128, HEAD_DIM], BF16, name=f"wk{i}") for i in range(NCT)]
        wv_sb = [wop.tile([128, HEAD_DIM], BF16, name=f"wv{i}") for i in range(NCT)]
        xts = [xp.tile([128, T], BF16, name=f"xt{i}") for i in range(NCT)]
        for i in range(NCT):
            sl = slice(i * 128, (i + 1) * 128)
            nc.sync.dma_start(out=wq_sb[i], in_=wq_d.ap()[sl, :])
            nc.sync.dma_start(out=wk_sb[i], in_=wk_d.ap()[sl, :])
            nc.sync.dma_start(out=wv_sb[i], in_=wv_d.ap()[sl, :])
            nc.sync.dma_start(out=xts[i], in_=xT_d.ap()[sl, :])

        ctab = const.tile([HEAD_DIM, T], BF16, name="ctab")
        stab = const.tile([HEAD_DIM, T], BF16, name="stab")
        nc.sync.dma_start(out=ctab, in_=ct_d.ap())
        nc.sync.dma_start(out=stab, in_=st_d.ap())
        ones_sb = const.tile([128, 128], BF16, name="ones_sb")
        ident = const.tile([128, 128], BF16, name="ident")
        tri_sb = const.tile([ST, ST], BF16, name="tri_sb")
        nc.sync.dma_start(out=ones_sb, in_=on_d.ap())
        nc.sync.dma_start(out=ident, in_=id_d.ap())
        nc.sync.dma_start(out=tri_sb, in_=tr_d.ap())
        if n_masks:
            msk_sb = const.tile([ST, n_masks * TB], BF16, name="msk_sb")
            for i in range(n_masks):
                nc.sync.dma_start(
                    out=msk_sb[:, i * TB : (i + 1) * TB],
                    in_=mk_d.ap()[i * ST : (i + 1) * ST, :],
                )
        wo_sb = [wop.tile([128, C], BF16, name=f"wo{h}") for h in range(G)]
        for h in range(G):
            nc.sync.dma_start(out=wo_sb[h], in_=wo_d.ap()[h * 128 : (h + 1) * 128, :])

        qT = [qkv.tile([128, T], BF16, name=f"qT{h}") for h in range(G)]
        kT = qkv.tile([128, T], BF16, name="kT")
        vT = qkv.tile([128, T], BF16, name="vT")
        vch = [qkv.tile([128, 128], BF16, name=f"v{s}") for s in range(NST)]

        # single PSUM pool: 8 x [128, 512] f32 bank tags, reused across stages
        ps = tc.alloc_tile_pool(name="ps", bufs=1, space="PSUM")

        def bank(tag):
            return ps.tile([128, TB], F32, name=tag, tag=tag)

        rpool = tc.alloc_tile_pool(name="rpool", bufs=3)

        # ---- stage 1: projections + RoPE + v transpose, pipelined per tb ----
        for tb in range(NTB):
            tsl = slice(tb * TB, (tb + 1) * TB)
            q_ps = [bank(f"bk{h}") for h in range(G)]
            k_ps = bank("bk4")
            v_ps = bank("bk5")
            for ci in range(NCT):
                first, last = ci == 0, ci == NCT - 1
                rhs = xts[ci][:, tsl]
                for h in range(G):
                    nc.tensor.matmul(
                        q_ps[h],
                        lhsT=wq_sb[ci][:, h * 128 : (h + 1) * 128],
                        rhs=rhs,
                        start=first,
                        stop=last,
                    )
                nc.tensor.matmul(k_ps, lhsT=wk_sb[ci], rhs=rhs, start=first, stop=last)
                nc.tensor.matmul(v_ps, lhsT=wv_sb[ci], rhs=rhs, start=first, stop=last)

            # RoPE on k first (attention waits on k), then qs; v last
            for dst, src_ps in [(kT, k_ps)] + [(qT[h], q_ps[h]) for h in range(G)]:
                nc.vector.tensor_copy(dst[:, tsl], src_ps)
                swp = rpool.tile([128, TB], BF16, name="swp", tag="swp")
                tmp = rpool.tile([128, TB], BF16, name="tmp", tag="tmp")
                nc.vector.stream_shuffle(swp, dst[:, tsl], SWAP_MASK)
                nc.vector.tensor_mul(tmp, dst[:, tsl], ctab[:, tsl])
                nc.vector.tensor_mul(swp, swp, stab[:, tsl])
                nc.vector.tensor_add(dst[:, tsl], tmp, swp)
            nc.vector.tensor_copy(vT[:, tsl], v_ps)
            for r in range(4):
                s = 4 * tb + r
                vtp = bank(f"bk{6 + (r % 2)}").bitcast(BF16)[:, :128]
                nc.tensor.transpose(vtp, vT[:, s * 128 : (s + 1) * 128], ident)
                nc.vector.tensor_copy(vch[s], vtp)

        # ---- stages 2+3 ----
        p2sb = tc.alloc_tile_pool(name="p2sb", bufs=4)
        p3sb = tc.alloc_tile_pool(name="p3sb", bufs=3)
        outp = tc.alloc_tile_pool(name="outp", bufs=8)
        ncopy = 0

        for tb in range(NTB):
            entries = plan[tb]
            oT_sbs = []
            for h in range(G):
                if not entries:
                    oT_sb = outp.tile([128, TB], BF16, name="oT", tag="oT")
                    nc.gpsimd.memset(oT_sb, 0.0)
                    oT_sbs.append(oT_sb)
                    continue
                oT_ps = bank(f"bk{3 + (h % 2)}")
                den = bank("bk5")
                n_e = len(entries)
                stps = [None] * n_e
                eps = [None] * n_e

                def emit_score(idx):
                    s, w0, kind, mid = entries[idx]
                    stp = bank(f"bk{idx % 3}")
                    nc.tensor.matmul(
                        stp[:, w0:],
                        lhsT=kT[:, s * 128 : (s + 1) * 128],
                        rhs=qT[h][:, tb * TB + w0 : (tb + 1) * TB],
                        start=True,
                        stop=True,
                    )
                    ep = p2sb.tile([ST, TB], BF16, name="ep", tag="ep")
                    nc.scalar.activation(ep[:, w0:], stp[:, w0:], Exp, scale=SCALE)
                    if kind == DIAG:
                        nc.vector.tensor_mul(
                            ep[:, w0 : w0 + ST], ep[:, w0 : w0 + ST], tri_sb
                        )
                    elif kind == GEN:
                        nc.vector.tensor_mul(
                            ep[:, w0:],
                            ep[:, w0:],
                            msk_sb[:, mid * TB : mid * TB + TB - w0],
                        )
                    eps[idx] = ep

                def emit_acc(idx):
                    s, w0, kind, mid = entries[idx]
                    ep = eps[idx]
                    first, last = idx == 0, idx == n_e - 1
                    nc.tensor.matmul(
                        den[:, w0:],
                        lhsT=ones_sb,
                        rhs=ep[:, w0:],
                        start=first,
                        stop=last,
                        skip_group_check=True,
                    )
                    nc.tensor.matmul(
                        oT_ps[:, w0:],
                        lhsT=vch[s],
                        rhs=ep[:, w0:],
                        start=first,
                        stop=last,
                        skip_group_check=True,
                    )
                    eps[idx] = None

                for idx in range(n_e + LA):
                    if idx < n_e:
                        emit_score(idx)
                    if idx - LA >= 0:
                        emit_acc(idx - LA)

                rcp = p2sb.tile([128, TB], F32, name="rcp", tag="rcp")
                nc.vector.reciprocal(rcp, den)
                oT_sb = outp.tile([128, TB], BF16, name="oT", tag="oT")
                nc.vector.tensor_mul(oT_sb, oT_ps, rcp)
                oT_sbs.append(oT_sb)

            # o_proj partial for this t-block
            for tch in range(TB // 128):
                for cb in range(C // 512):
                    ops = bank(f"bk{6 + (cb % 2)}")
                    for h in range(G):
                        nc.tensor.matmul(
                            ops,
                            lhsT=oT_sbs[h][:, tch * 128 : (tch + 1) * 128],
                            rhs=wo_sb[h][:, cb * 512 : (cb + 1) * 512],
                            start=h == 0,
                            stop=h == G - 1,
                        )
                    osb = p3sb.tile([128, 512], F32, name="osb", tag="osb")
                    if ncopy % 2 == 0:
                        nc.scalar.copy(osb, ops)
                    else:
                        nc.vector.tensor_copy(osb, ops)
                    ncopy += 1
                    t0 = tb * TB + tch * 128
                    nc.sync.dma_start(
                        out=out_d.ap()[t0 : t0 + 128, cb * 512 : (cb + 1) * 512],
                        in_=osb,
                    )

        outp.release()
        p3sb.release()
        p2sb.release()
        rpool.release()
        ps.release()
        xp.release()
        qkv.release()
        wop.release()
        const.release()

    nc.compile()
    return nc


def _prep_inputs(x, cos, sin, Wq, Wk, Wv, Wo, mask_tiles, n_masks):
    cos = np.asarray(cos, dtype=np.float32).reshape(T, HEAD_DIM // 2)
    sin = np.asarray(sin, dtype=np.float32).reshape(T, HEAD_DIM // 2)
    ctab = np.ascontiguousarray(np.repeat(cos, 2, axis=1).T).astype(BF)  # [128, T]
    s2 = np.repeat(sin, 2, axis=1)
    s2[:, 0::2] *= -1.0
    stab = np.ascontiguousarray(s2.T).astype(BF)
    tri = (np.arange(ST)[:, None] <= np.arange(ST)[None, :]).astype(BF)

    xTb = [
        np.ascontiguousarray(np.asarray(x[b], dtype=np.float32).T).astype(BF)
        for b in range(B)
    ]
    in_maps = []
    for core in range(8):
        b, g = divmod(core, NUM_KV_HEADS)
        m = {
            "xT": xTb[b],
            "wq": np.ascontiguousarray(Wq[:, g * 512 : (g + 1) * 512]).astype(BF),
            "wk": np.ascontiguousarray(Wk[:, g * 128 : (g + 1) * 128]).astype(BF),
            "wv": np.ascontiguousarray(Wv[:, g * 128 : (g + 1) * 128]).astype(BF),
            "wo": np.ascontiguousarray(Wo[g * 512 : (g + 1) * 512, :]).astype(BF),
            "ctab": ctab,
            "stab": stab,
            "ones": np.ones((128, 128), dtype=BF),
            "ident": np.eye(128, dtype=BF),
            "tri": tri,
        }
        if n_masks:
            m["masks"] = mask_tiles.reshape(n_masks * ST, TB).astype(BF)
        in_maps.append(m)
    return in_maps


def kernel(x, cos, sin, mask, Wq, Wk, Wv, Wo, _trace=False, _result_box=None):
    from concourse.bass_utils import run_bass_kernel_spmd

    mask2d = np.asarray(mask).reshape(T, T).astype(bool)
    plan, mask_tiles = _classify_mask(mask2d)
    n_masks = int(mask_tiles.shape[0])

    key = (plan, n_masks)
    nc = _nc_cache.get(key)
    if nc is None:
        nc = _build(plan, n_masks)
        _nc_cache[key] = nc

    in_maps = _prep_inputs(x, cos, sin, Wq, Wk, Wv, Wo, mask_tiles, n_masks)
    res = run_bass_kernel_spmd(nc, in_maps, core_ids=list(range(8)), trace=_trace)
    if _result_box is not None:
        _result_box.append(res)

    out = np.zeros((B, T, C), dtype=np.float32)
    for core in range(8):
        b = core // NUM_KV_HEADS
        out[b] += res.results[core]["out"]
    return out


# revision 9
# speedup vs baseline: 1.6460x; 1.0223x over previous
"""Grouped-Query Attention (B=2, T=2048, C=2048, 16 Q heads / 4 KV heads,
D=128) on 8 Trainium2 NeuronCores.

Sharding: core (b, g) for b in {0,1}, g in {0..3} handles batch b and KV head
g (= query heads 4g..4g+3). Each core computes its 4 heads' attention plus the
partial output projection against its 512-row slice of Wo; the host sums the
4 partials per batch (the "all-reduce" of the o_proj, done in numpy).

All matmul operands are bf16 (host-cast); PSUM accumulation stays fp32, so
the only precision loss is input rounding (~4e-3 rel err vs the 2e-2 gate).

Layout/scheduling notes (from trace analysis):
- One PSUM pool with 8 [128,512]-f32 bank tags reused across stages (no
  mid-kernel pool releases -> no cross-stage drain bubbles; the PE pstate
  ramp resets on idle gaps, so a dense PE queue is worth ~1.5x clock).
- Startup DMAs interleaved per contraction chunk (wq/wk/wv/x) so the first
  projection matmul unblocks after ~4 transfers instead of all weights.
- Softmax denominator is computed REPLICATED across all 128 partitions
  (lhsT = all-ones [128,128]) so the reciprocal runs as a full-width DVE op
  (~0.65us) instead of a 1-partition op (3.3us) + GpSimd partition
  broadcast; the per-head tail stall on the PE disappears.
- Stage-2 software pipeline: score matmuls run 2 s-tiles ahead of the
  dependent den/PV matmuls so the PE never waits on ScalarE's exp.
- Diagonal (causal-boundary) s-tiles only compute the t-window right of the
  diagonal plus one shared [128,128] triangular 0/1 multiply.
"""
import sys

sys.path.insert(0, "/opt/trn_rl_repo")

import numpy as np
import ml_dtypes

B, T, C = 2, 2048, 2048
NUM_HEADS, NUM_KV_HEADS, HEAD_DIM = 16, 4, 128
G = NUM_HEADS // NUM_KV_HEADS  # 4 query heads per core
SCALE = float(HEAD_DIM) ** -0.5
TB = 512  # t-block (matmul moving free dim)
NTB = T // TB  # 4
ST = 128  # s-tile
NST = T // ST  # 16
NCT = C // 128  # 16 contraction tiles
LA = 2  # stage-2 score-matmul lookahead (s-tiles in flight past exp)

SWAP_MASK = [i ^ 1 for i in range(32)]
BF = ml_dtypes.bfloat16

_nc_cache: dict = {}

# plan entry kinds
FULL, DIAG, GEN = 0, 1, 2


def _classify_mask(mask2d: np.ndarray):
    """mask2d[t, s] bool. Returns (plan, mask_tiles).

    plan[tb] = tuple of (s_tile_idx, w0, kind, mask_id). w0 is the t-window
    start within the t-block (columns < w0 are entirely masked for this
    s-tile). kind: FULL (no mask work), DIAG (shared lower-triangular 0/1
    multiply on the first 128 window columns), GEN (per-tile 0/1 multiply
    over the whole window; mask_id indexes mask_tiles)."""
    tri = (np.arange(ST)[:, None] <= np.arange(ST)[None, :])
    plan = []
    uniq: dict = {}
    tiles = []
    for tb in range(NTB):
        sub_t = mask2d[tb * TB : (tb + 1) * TB]  # [TB(t), T(s)]
        entries = []
        for s in range(NST):
            sub = sub_t[:, s * ST : (s + 1) * ST]  # [TB(t), ST(s)]
            if sub.all():
                entries.append((s, 0, FULL, None))
                continue
            if not sub.any():
                continue
            m = sub.T  # [s, t]
            w0 = 0
            while w0 + ST <= TB and not m[:, w0 : w0 + ST].any():
                w0 += ST
            win = m[:, w0:]
            if (
                win.shape[1] >= ST
                and (win[:, :ST] == tri).all()
                and win[:, ST:].all()
            ):
                entries.append((s, w0, DIAG, None))
                continue
            tile_m = np.zeros((ST, TB), dtype=np.float32)
            tile_m[:, : TB - w0] = win.astype(np.float32)
            key = (w0, tile_m.tobytes())
            mid = uniq.get(key)
            if mid is None:
                mid = len(tiles)
                uniq[key] = mid
                tiles.append(tile_m)
            entries.append((s, w0, GEN, mid))
        plan.append(tuple(entries))
    mask_tiles = (
        np.stack(tiles) if tiles else np.zeros((0, ST, TB), dtype=np.float32)
    )
    return tuple(plan), mask_tiles


def _build(plan, n_masks):
    import concourse.bacc as bacc
    import concourse.mybir as mybir
    import concourse.tile as tile

    F32 = mybir.dt.float32
    BF16 = mybir.dt.bfloat16
    Exp = mybir.ActivationFunctionType.Exp

    nc = bacc.Bacc()

    xT_d = nc.declare_dram_parameter("xT", [C, T], BF16, isOutput=False)
    wq_d = nc.declare_dram_parameter("wq", [C, G * HEAD_DIM], BF16, isOutput=False)
    wk_d = nc.declare_dram_parameter("wk", [C, HEAD_DIM], BF16, isOutput=False)
    wv_d = nc.declare_dram_parameter("wv", [C, HEAD_DIM], BF16, isOutput=False)
    wo_d = nc.declare_dram_parameter("wo", [G * HEAD_DIM, C], BF16, isOutput=False)
    on_d = nc.declare_dram_parameter("ones", [128, 128], BF16, isOutput=False)
    id_d = nc.declare_dram_parameter("ident", [128, 128], BF16, isOutput=False)
    tr_d = nc.declare_dram_parameter("tri", [ST, ST], BF16, isOutput=False)
    ct_d = nc.declare_dram_parameter("ctab", [HEAD_DIM, T], BF16, isOutput=False)
    st_d = nc.declare_dram_parameter("stab", [HEAD_DIM, T], BF16, isOutput=False)
    if n_masks:
        mk_d = nc.declare_dram_parameter(
            "masks", [n_masks * ST, TB], BF16, isOutput=False
        )
    out_d = nc.declare_dram_parameter("out", [T, C], F32, isOutput=True)

    with tile.TileContext(nc) as tc:
        const = tc.alloc_tile_pool(name="const", bufs=1)
        wop = tc.alloc_tile_pool(name="wop", bufs=1)
        qkv = tc.alloc_tile_pool(name="qkv", bufs=1)
        xp = tc.alloc_tile_pool(name="xp", bufs=1)

        # --- interleaved startup DMAs: per-chunk weights + x so the first
        # projection matmuls unblock after a handful of transfers ---
        wq_sb = [wop.tile([128, G * HEAD_DIM], BF16, name=f"wq{i}") for i in range(NCT)]
        wk_sb = [wop.tile([128, HEAD_DIM], BF16, name=f"wk{i}") for i in range(NCT)]
        wv_sb = [wop.tile([128, HEAD_DIM], BF16, name=f"wv{i}") for i in range(NCT)]
        xts = [xp.tile([128, T], BF16, name=f"xt{i}") for i in range(NCT)]
        for i in range(NCT):
            sl = slice(i * 128, (i + 1) * 128)
            nc.sync.dma_start(out=wq_sb[i], in_=wq_d.ap()[sl, :])
            nc.sync.dma_start(out=wk_sb[i], in_=wk_d.ap()[sl, :])
            nc.sync.dma_start(out=wv_sb[i], in_=wv_d.ap()[sl, :])
            nc.sync.dma_start(out=xts[i], in_=xT_d.ap()[sl, :])

        ctab = const.tile([HEAD_DIM, T], BF16, name="ctab")
        stab = const.tile([HEAD_DIM, T], BF16, name="stab")
        nc.sync.dma_start(out=ctab, in_=ct_d.ap())
        nc.sync.dma_start(out=stab, in_=st_d.ap())
        ones_sb = const.tile([128, 128], BF16, name="ones_sb")
        ident = const.tile([128, 128], BF16, name="ident")
        trineg = const.tile([ST, ST], BF16, name="trineg")
        nc.sync.dma_start(out=ones_sb, in_=on_d.ap())
        nc.sync.dma_start(out=ident, in_=id_d.ap())
        nc.sync.dma_start(out=trineg, in_=tr_d.ap())
        if n_masks:
            msk_sb = const.tile([ST, n_masks * TB], BF16, name="msk_sb")
            for i in range(n_masks):
                nc.sync.dma_start(
                    out=msk_sb[:, i * TB : (i + 1) * TB],
                    in_=mk_d.ap()[i * ST : (i + 1) * ST, :],
                )
        wo_sb = [wop.tile([128, C], BF16, name=f"wo{h}") for h in range(G)]
        for h in range(G):
            nc.sync.dma_start(out=wo_sb[h], in_=wo_d.ap()[h * 128 : (h + 1) * 128, :])

        qT = [qkv.tile([128, T], BF16, name=f"qT{h}") for h in range(G)]
        kT = qkv.tile([128, T], BF16, name="kT")
        vT = qkv.tile([128, T], BF16, name="vT")
        vch = [qkv.tile([128, 128], BF16, name=f"v{s}") for s in range(NST)]

        # single PSUM pool: 8 x [128, 512] f32 bank tags, reused across stages
        ps = tc.alloc_tile_pool(name="ps", bufs=1, space="PSUM")

        def bank(tag):
            return ps.tile([128, TB], F32, name=tag, tag=tag)

        rpool = tc.alloc_tile_pool(name="rpool", bufs=3)

        # ---- stage 1: projections + RoPE + v transpose, pipelined per tb ----
        for tb in range(NTB):
            tsl = slice(tb * TB, (tb + 1) * TB)
            q_ps = [bank(f"bk{h}") for h in range(G)]
            k_ps = bank("bk4")
            v_ps = bank("bk5")
            for ci in range(NCT):
                first, last = ci == 0, ci == NCT - 1
                rhs = xts[ci][:, tsl]
                for h in range(G):
                    nc.tensor.matmul(
                        q_ps[h],
                        lhsT=wq_sb[ci][:, h * 128 : (h + 1) * 128],
                        rhs=rhs,
                        start=first,
                        stop=last,
                    )
                nc.tensor.matmul(k_ps, lhsT=wk_sb[ci], rhs=rhs, start=first, stop=last)
                nc.tensor.matmul(v_ps, lhsT=wv_sb[ci], rhs=rhs, start=first, stop=last)

            # v first so the transposes don't queue behind the RoPE backlog,
            # then k (attention waits on it), then qs
            nc.vector.tensor_copy(vT[:, tsl], v_ps)
            for r in range(4):
                s = 4 * tb + r
                vtp = bank(f"bk{6 + (r % 2)}").bitcast(BF16)[:, :128]
                nc.tensor.transpose(vtp, vT[:, s * 128 : (s + 1) * 128], ident)
                nc.vector.tensor_copy(vch[s], vtp)
            for dst, src_ps in [(kT, k_ps)] + [(qT[h], q_ps[h]) for h in range(G)]:
                nc.vector.tensor_copy(dst[:, tsl], src_ps)
                swp = rpool.tile([128, TB], BF16, name="swp", tag="swp")
                tmp = rpool.tile([128, TB], BF16, name="tmp", tag="tmp")
                nc.vector.stream_shuffle(swp, dst[:, tsl], SWAP_MASK)
                nc.vector.tensor_mul(tmp, dst[:, tsl], ctab[:, tsl])
                nc.vector.tensor_mul(swp, swp, stab[:, tsl])
                nc.vector.tensor_add(dst[:, tsl], tmp, swp)

        # ---- stages 2+3: one global software pipeline over (tb, head, s-tile)
        # so head/t-block boundaries never drain the PE. Scores run LA s-tiles
        # ahead of the dependent den/PV matmuls (exp latency hidden). ----
        p2sb = tc.alloc_tile_pool(name="p2sb", bufs=4)
        p3sb = tc.alloc_tile_pool(name="p3sb", bufs=3)
        outp = tc.alloc_tile_pool(name="outp", bufs=10)
        state = {"score": 0, "head": 0, "ncopy": 0}
        oT_live: dict = {}

        def emit_oproj(tb):
            oT_sbs = oT_live.pop(tb)
            for tch in range(TB // 128):
                for cb in range(C // 512):
                    ops = bank(f"bk{6 + (cb % 2)}")
                    for h in range(G):
                        nc.tensor.matmul(
                            ops,
                            lhsT=oT_sbs[h][:, tch * 128 : (tch + 1) * 128],
                            rhs=wo_sb[h][:, cb * 512 : (cb + 1) * 512],
                            start=h == 0,
                            stop=h == G - 1,
                        )
                    osb = p3sb.tile([128, 512], F32, name="osb", tag="osb")
                    if state["ncopy"] % 2 == 0:
                        nc.scalar.copy(osb, ops)
                    else:
                        nc.vector.tensor_copy(osb, ops)
                    state["ncopy"] += 1
                    t0 = tb * TB + tch * 128
                    nc.sync.dma_start(
                        out=out_d.ap()[t0 : t0 + 128, cb * 512 : (cb + 1) * 512],
                        in_=osb,
                    )

        items = []  # (tb, h, idx)
        for tb in range(NTB):
            for h in range(G):
                for idx in range(len(plan[tb])):
                    items.append((tb, h, idx))

        ctx: dict = {}  # (tb,h) -> dict with oT_ps, den, eps

        def emit_score(it):
            tb, h, idx = it
            entries = plan[tb]
            s, w0, kind, mid = entries[idx]
            if idx == 0:
                ctx[(tb, h)] = {
                    "oT": bank(f"bk{3 + (state['head'] % 2)}"),
                    "den": bank("bk5"),
                    "eps": {},
                }
                state["head"] += 1
            stp = bank(f"bk{state['score'] % 3}")
            state["score"] += 1
            diag = kind == DIAG
            nc.tensor.matmul(
                stp[:, w0:],
                lhsT=kT[:, s * 128 : (s + 1) * 128],
                rhs=qT[h][:, tb * TB + w0 : (tb + 1) * TB],
                start=True,
                stop=not diag,
                skip_group_check=diag,
            )
            if diag:
                # additive -512*(s>t) triangular mask folded into the score
                # accumulation on the PE (keeps DVE off the critical path);
                # exp then underflows to ~e-18 which is negligible in den/PV
                nc.tensor.matmul(
                    stp[:, w0 : w0 + ST],
                    lhsT=ident,
                    rhs=trineg,
                    start=False,
                    stop=True,
                    skip_group_check=True,
                )
            ep = p2sb.tile([ST, TB], BF16, name="ep", tag="ep")
            nc.scalar.activation(ep[:, w0:], stp[:, w0:], Exp, scale=SCALE)
            if kind == GEN:
                nc.vector.tensor_mul(
                    ep[:, w0:],
                    ep[:, w0:],
                    msk_sb[:, mid * TB : mid * TB + TB - w0],
                )
            ctx[(tb, h)]["eps"][idx] = ep

        def emit_acc(it):
            tb, h, idx = it
            entries = plan[tb]
            s, w0, kind, mid = entries[idx]
            c = ctx[(tb, h)]
            ep = c["eps"].pop(idx)
            first, last = idx == 0, idx == len(entries) - 1
            nc.tensor.matmul(
                c["den"][:, w0:],
                lhsT=ones_sb,
                rhs=ep[:, w0:],
                start=first,
                stop=last,
                skip_group_check=True,
            )
            nc.tensor.matmul(
                c["oT"][:, w0:],
                lhsT=vch[s],
                rhs=ep[:, w0:],
                start=first,
                stop=last,
                skip_group_check=True,
            )
            if last:
                rcp = p2sb.tile([128, TB], F32, name="rcp", tag="rcp")
                nc.vector.reciprocal(rcp, c["den"])
                oT_sb = outp.tile([128, TB], BF16, name="oT", tag="oT")
                nc.vector.tensor_mul(oT_sb, c["oT"], rcp)
                oT_live.setdefault(tb, []).append(oT_sb)
                del ctx[(tb, h)]
                if h == G - 1:
                    emit_oproj(tb)

        from collections import deque

        pend = deque()
        for it in items:
            emit_score(it)
            pend.append(it)
            if len(pend) > LA:
                emit_acc(pend.popleft())
        while pend:
            emit_acc(pend.popleft())

        outp.release()
        p3sb.release()
        p2sb.release()
        rpool.release()
        ps.release()
        xp.release()
        qkv.release()
        wop.release()
        const.release()

    nc.compile()
    return nc


def _prep_inputs(x, cos, sin, Wq, Wk, Wv, Wo, mask_tiles, n_masks):
    cos = np.asarray(cos, dtype=np.float32).reshape(T, HEAD_DIM // 2)
    sin = np.asarray(sin, dtype=np.float32).reshape(T, HEAD_DIM // 2)
    ctab = np.ascontiguousarray(np.repeat(cos, 2, axis=1).T).astype(BF)  # [128, T]
    s2 = np.repeat(sin, 2, axis=1)
    s2[:, 0::2] *= -1.0
    stab = np.ascontiguousarray(s2.T).astype(BF)
    trineg = (-512.0 * (np.arange(ST)[:, None] > np.arange(ST)[None, :])).astype(BF)

    xTb = [
        np.ascontiguousarray(np.asarray(x[b], dtype=np.float32).T).astype(BF)
        for b in range(B)
    ]
    in_maps = []
    for core in range(8):
        b, g = divmod(core, NUM_KV_HEADS)
        m = {
            "xT": xTb[b],
            "wq": np.ascontiguousarray(Wq[:, g * 512 : (g + 1) * 512]).astype(BF),
            "wk": np.ascontiguousarray(Wk[:, g * 128 : (g + 1) * 128]).astype(BF),
            "wv": np.ascontiguousarray(Wv[:, g * 128 : (g + 1) * 128]).astype(BF),
            "wo": np.ascontiguousarray(Wo[g * 512 : (g + 1) * 512, :]).astype(BF),
            "ctab": ctab,
            "stab": stab,
            "ones": np.ones((128, 128), dtype=BF),
            "ident": np.eye(128, dtype=BF),
            "tri": trineg,
        }
        if n_masks:
            m["masks"] = mask_tiles.reshape(n_masks * ST, TB).astype(BF)
        in_maps.append(m)
    return in_maps


def kernel(x, cos, sin, mask, Wq, Wk, Wv, Wo, _trace=False, _result_box=None):
    from concourse.bass_utils import run_bass_kernel_spmd

    mask2d = np.asarray(mask).reshape(T, T).astype(bool)
    plan, mask_tiles = _classify_mask(mask2d)
    n_masks = int(mask_tiles.shape[0])

    key = (plan, n_masks)
    nc = _nc_cache.get(key)
    if nc is None:
        nc = _build(plan, n_masks)
        _nc_cache[key] = nc

    in_maps = _prep_inputs(x, cos, sin, Wq, Wk, Wv, Wo, mask_tiles, n_masks)
    res = run_bass_kernel_spmd(nc, in_maps, core_ids=list(range(8)), trace=_trace)
    if _result_box is not None:
        _result_box.append(res)

    out = np.zeros((B, T, C), dtype=np.float32)
    for core in range(8):
        b = core // NUM_KV_HEADS
        out[b] += res.results[core]["out"]
    return out


# revision 13
# speedup vs baseline: 1.6668x; 1.0127x over previous
"""Grouped-Query Attention (B=2, T=2048, C=2048, 16 Q heads / 4 KV heads,
D=128) on 8 Trainium2 NeuronCores.

Sharding: core (b, g) for b in {0,1}, g in {0..3} handles batch b and KV head
g (= query heads 4g..4g+3). Each core computes its 4 heads' attention plus the
partial output projection against its 512-row slice of Wo; the host sums the
4 partials per batch (the "all-reduce" of the o_proj, done in numpy).

All matmul operands are bf16 (host-cast); PSUM accumulation stays fp32, so
the only precision loss is input rounding (~4e-3 rel err vs the 2e-2 gate).

Layout/scheduling notes (from trace analysis):
- One PSUM pool with 8 [128,512]-f32 bank tags reused across stages (no
  mid-kernel pool releases -> no cross-stage drain bubbles; the PE pstate
  ramp resets on idle gaps, so a dense PE queue is worth ~1.5x clock).
- Startup DMAs interleaved per contraction chunk (wq/wk/wv/x) so the first
  projection matmul unblocks after ~4 transfers instead of all weights.
- Softmax denominator is computed REPLICATED across all 128 partitions
  (lhsT = all-ones [128,128]) so the reciprocal runs as a full-width DVE op
  (~0.65us) instead of a 1-partition op (3.3us) + GpSimd partition
  broadcast; the per-head tail stall on the PE disappears.
- Stage-2 software pipeline: score matmuls run 2 s-tiles ahead of the
  dependent den/PV matmuls so the PE never waits on ScalarE's exp.
- Diagonal (causal-boundary) s-tiles only compute the t-window right of the
  diagonal plus one shared [128,128] triangular 0/1 multiply.
"""
import sys

sys.path.insert(0, "/opt/trn_rl_repo")

import numpy as np
import ml_dtypes

B, T, C = 2, 2048, 2048
NUM_HEADS, NUM_KV_HEADS, HEAD_DIM = 16, 4, 128
G = NUM_HEADS // NUM_KV_HEADS  # 4 query heads per core
SCALE = float(HEAD_DIM) ** -0.5
TB = 512  # t-block (matmul moving free dim)
NTB = T // TB  # 4
ST = 128  # s-tile
NST = T // ST  # 16
NCT = C // 128  # 16 contraction tiles
LA = 2  # stage-2 score-matmul lookahead (s-tiles in flight past exp)

SWAP_MASK = [i ^ 1 for i in range(32)]
BF = ml_dtypes.bfloat16

_nc_cache: dict = {}

# plan entry kinds
FULL, DIAG, GEN = 0, 1, 2


def _classify_mask(mask2d: np.ndarray):
    """mask2d[t, s] bool. Returns (plan, mask_tiles).

    plan[tb] = tuple of (s_tile_idx, w0, kind, mask_id). w0 is the t-window
    start within the t-block (columns < w0 are entirely masked for this
    s-tile). kind: FULL (no mask work), DIAG (shared lower-triangular 0/1
    multiply on the first 128 window columns), GEN (per-tile 0/1 multiply
    over the whole window; mask_id indexes mask_tiles)."""
    tri = (np.arange(ST)[:, None] <= np.arange(ST)[None, :])
    plan = []
    uniq: dict = {}
    tiles = []
    for tb in range(NTB):
        sub_t = mask2d[tb * TB : (tb + 1) * TB]  # [TB(t), T(s)]
        entries = []
        for s in range(NST):
            sub = sub_t[:, s * ST : (s + 1) * ST]  # [TB(t), ST(s)]
            if sub.all():
                entries.append((s, 0, FULL, None))
                continue
            if not sub.any():
                continue
            m = sub.T  # [s, t]
            w0 = 0
            while w0 + ST <= TB and not m[:, w0 : w0 + ST].any():
                w0 += ST
            win = m[:, w0:]
            if (
                win.shape[1] >= ST
                and (win[:, :ST] == tri).all()
                and win[:, ST:].all()
            ):
                entries.append((s, w0, DIAG, None))
                continue
            tile_m = np.zeros((ST, TB), dtype=np.float32)
            tile_m[:, : TB - w0] = win.astype(np.float32)
            key = (w0, tile_m.tobytes())
            mid = uniq.get(key)
            if mid is None:
                mid = len(tiles)
                uniq[key] = mid
                tiles.append(tile_m)
            entries.append((s, w0, GEN, mid))
        plan.append(tuple(entries))
    mask_tiles = (
        np.stack(tiles) if tiles else np.zeros((0, ST, TB), dtype=np.float32)
    )
    return tuple(plan), mask_tiles


def _build(plan, n_masks):
    import concourse.bacc as bacc
    import concourse.mybir as mybir
    import concourse.tile as tile

    F32 = mybir.dt.float32
    BF16 = mybir.dt.bfloat16
    Exp = mybir.ActivationFunctionType.Exp

    nc = bacc.Bacc()

    xT_d = nc.declare_dram_parameter("xT", [C, T], BF16, isOutput=False)
    wq_d = nc.declare_dram_parameter("wq", [C, G * HEAD_DIM], BF16, isOutput=False)
    wk_d = nc.declare_dram_parameter("wk", [C, HEAD_DIM], BF16, isOutput=False)
    wv_d = nc.declare_dram_parameter("wv", [C, HEAD_DIM], BF16, isOutput=False)
    wo_d = nc.declare_dram_parameter("wo", [G * HEAD_DIM, C], BF16, isOutput=False)
    on_d = nc.declare_dram_parameter("ones", [128, 128], BF16, isOutput=False)
    id_d = nc.declare_dram_parameter("ident", [128, 128], BF16, isOutput=False)
    tr_d = nc.declare_dram_parameter("tri", [ST, ST], BF16, isOutput=False)
    ct_d = nc.declare_dram_parameter("ctab", [HEAD_DIM, T], BF16, isOutput=False)
    st_d = nc.declare_dram_parameter("stab", [HEAD_DIM, T], BF16, isOutput=False)
    if n_masks:
        mk_d = nc.declare_dram_parameter(
            "masks", [n_masks * ST, TB], BF16, isOutput=False
        )
    out_d = nc.declare_dram_parameter("out", [T, C], F32, isOutput=True)

    with tile.TileContext(nc) as tc:
        const = tc.alloc_tile_pool(name="const", bufs=1)
        wop = tc.alloc_tile_pool(name="wop", bufs=1)
        qkv = tc.alloc_tile_pool(name="qkv", bufs=1)
        xp = tc.alloc_tile_pool(name="xp", bufs=1)

        # --- interleaved startup DMAs: per-chunk weights + x so the first
        # projection matmuls unblock after a handful of transfers ---
        wq_sb = [wop.tile([128, G * HEAD_DIM], BF16, name=f"wq{i}") for i in range(NCT)]
        wk_sb = [wop.tile([128, HEAD_DIM], BF16, name=f"wk{i}") for i in range(NCT)]
        wv_sb = [wop.tile([128, HEAD_DIM], BF16, name=f"wv{i}") for i in range(NCT)]
        xts = [xp.tile([128, T], BF16, name=f"xt{i}") for i in range(NCT)]
        for i in range(NCT):
            sl = slice(i * 128, (i + 1) * 128)
            nc.sync.dma_start(out=wq_sb[i], in_=wq_d.ap()[sl, :])
            nc.sync.dma_start(out=wk_sb[i], in_=wk_d.ap()[sl, :])
            nc.sync.dma_start(out=wv_sb[i], in_=wv_d.ap()[sl, :])
            nc.sync.dma_start(out=xts[i], in_=xT_d.ap()[sl, :])

        ctab = const.tile([HEAD_DIM, T], BF16, name="ctab")
        stab = const.tile([HEAD_DIM, T], BF16, name="stab")
        nc.sync.dma_start(out=ctab, in_=ct_d.ap())
        nc.sync.dma_start(out=stab, in_=st_d.ap())
        ones_sb = const.tile([128, 128], BF16, name="ones_sb")
        ident = const.tile([128, 128], BF16, name="ident")
        trineg = const.tile([ST, ST], BF16, name="trineg")
        nc.sync.dma_start(out=ones_sb, in_=on_d.ap())
        nc.sync.dma_start(out=ident, in_=id_d.ap())
        nc.sync.dma_start(out=trineg, in_=tr_d.ap())
        if n_masks:
            msk_sb = const.tile([ST, n_masks * TB], BF16, name="msk_sb")
            for i in range(n_masks):
                nc.sync.dma_start(
                    out=msk_sb[:, i * TB : (i + 1) * TB],
                    in_=mk_d.ap()[i * ST : (i + 1) * ST, :],
                )
        wo_sb = [wop.tile([128, C], BF16, name=f"wo{h}") for h in range(G)]
        for h in range(G):
            nc.sync.dma_start(out=wo_sb[h], in_=wo_d.ap()[h * 128 : (h + 1) * 128, :])

        qT = [qkv.tile([128, T], BF16, name=f"qT{h}") for h in range(G)]
        kT = qkv.tile([128, T], BF16, name="kT")
        vT = qkv.tile([128, T], BF16, name="vT")
        vch = [qkv.tile([128, 128], BF16, name=f"v{s}") for s in range(NST)]

        # single PSUM pool: 8 x [128, 512] f32 bank tags, reused across stages
        ps = tc.alloc_tile_pool(name="ps", bufs=1, space="PSUM")

        def bank(tag):
            return ps.tile([128, TB], F32, name=tag, tag=tag)

        rpool = tc.alloc_tile_pool(name="rpool", bufs=3)

        # ---- stage 1: projections + RoPE + v transpose, pipelined per tb ----
        for tb in range(NTB):
            tsl = slice(tb * TB, (tb + 1) * TB)
            q_ps = [bank(f"bk{h}") for h in range(G)]
            k_ps = bank("bk4")
            v_ps = bank("bk5")
            for ci in range(NCT):
                first, last = ci == 0, ci == NCT - 1
                rhs = xts[ci][:, tsl]
                for h in range(G):
                    nc.tensor.matmul(
                        q_ps[h],
                        lhsT=wq_sb[ci][:, h * 128 : (h + 1) * 128],
                        rhs=rhs,
                        start=first,
                        stop=last,
                    )
                nc.tensor.matmul(k_ps, lhsT=wk_sb[ci], rhs=rhs, start=first, stop=last)
                nc.tensor.matmul(v_ps, lhsT=wv_sb[ci], rhs=rhs, start=first, stop=last)

            # v first so the transposes don't queue behind the RoPE backlog,
            # then k (attention waits on it), then qs
            nc.vector.tensor_copy(vT[:, tsl], v_ps)
            for r in range(4):
                s = 4 * tb + r
                vtp = bank(f"bk{6 + (r % 2)}").bitcast(BF16)[:, :128]
                nc.tensor.transpose(vtp, vT[:, s * 128 : (s + 1) * 128], ident)
                nc.vector.tensor_copy(vch[s], vtp)
            for dst, src_ps in [(kT, k_ps)] + [(qT[h], q_ps[h]) for h in range(G)]:
                nc.vector.tensor_copy(dst[:, tsl], src_ps)
                swp = rpool.tile([128, TB], BF16, name="swp", tag="swp")
                tmp = rpool.tile([128, TB], BF16, name="tmp", tag="tmp")
                nc.vector.stream_shuffle(swp, dst[:, tsl], SWAP_MASK)
                nc.vector.tensor_mul(tmp, dst[:, tsl], ctab[:, tsl])
                nc.vector.tensor_mul(swp, swp, stab[:, tsl])
                nc.vector.tensor_add(dst[:, tsl], tmp, swp)

        # ---- stages 2+3: one global software pipeline over (tb, head, s-tile)
        # so head/t-block boundaries never drain the PE. Scores run LA s-tiles
        # ahead of the dependent den/PV matmuls (exp latency hidden). ----
        p2sb = tc.alloc_tile_pool(name="p2sb", bufs=4)
        p3sb = tc.alloc_tile_pool(name="p3sb", bufs=3)
        outp = tc.alloc_tile_pool(name="outp", bufs=10)
        state = {"score": 0, "head": 0, "ncopy": 0}
        oT_live: dict = {}

        def emit_oproj(tb):
            oT_sbs = oT_live.pop(tb)
            for tch in range(TB // 128):
                for cb in range(C // 512):
                    ops = bank(f"bk{2 + (cb % 2)}")
                    for h in range(G):
                        nc.tensor.matmul(
                            ops,
                            lhsT=oT_sbs[h][:, tch * 128 : (tch + 1) * 128],
                            rhs=wo_sb[h][:, cb * 512 : (cb + 1) * 512],
                            start=h == 0,
                            stop=h == G - 1,
                        )
                    osb = p3sb.tile([128, 512], F32, name="osb", tag="osb")
                    # keep these copies off DVE so rcp (den WAR chain) and
                    # the rescale muls never queue behind them
                    nc.scalar.copy(osb, ops)
                    state["ncopy"] += 1
                    t0 = tb * TB + tch * 128
                    nc.sync.dma_start(
                        out=out_d.ap()[t0 : t0 + 128, cb * 512 : (cb + 1) * 512],
                        in_=osb,
                    )

        items = []  # (tb, h, idx)
        for tb in range(NTB):
            for h in range(G):
                for idx in range(len(plan[tb])):
                    items.append((tb, h, idx))

        ctx: dict = {}  # (tb,h) -> dict with oT_ps, den, eps

        def emit_score(it):
            tb, h, idx = it
            entries = plan[tb]
            s, w0, kind, mid = entries[idx]
            # bank roles chosen so stage-2 tiles reuse the PSUM banks that
            # stage-1's trailing (tb=3) DVE stream releases earliest:
            # v (bk5) and vtp (bk6/7) first -> stp; k (bk4) -> den;
            # q0/q1 (bk0/1) -> oT; q2/q3 (bk2/3) -> o_proj accumulators
            if idx == 0:
                ctx[(tb, h)] = {
                    "oT": bank(f"bk{0 + (state['head'] % 2)}"),
                    "den": bank("bk4"),
                    "eps": {},
                }
                state["head"] += 1
            stp = bank(f"bk{5 + (state['score'] % 3)}")
            state["score"] += 1
            diag = kind == DIAG
            nc.tensor.matmul(
                stp[:, w0:],
                lhsT=kT[:, s * 128 : (s + 1) * 128],
                rhs=qT[h][:, tb * TB + w0 : (tb + 1) * TB],
                start=True,
                stop=not diag,
                skip_group_check=diag,
            )
            if diag:
                # additive -512*(s>t) triangular mask folded into the score
                # accumulation on the PE (keeps DVE off the critical path);
                # exp then underflows to ~e-18 which is negligible in den/PV
                nc.tensor.matmul(
                    stp[:, w0 : w0 + ST],
                    lhsT=ident,
                    rhs=trineg,
                    start=False,
                    stop=True,
                    skip_group_check=True,
                )
            ep = p2sb.tile([ST, TB], BF16, name="ep", tag="ep")
            nc.scalar.activation(ep[:, w0:], stp[:, w0:], Exp, scale=SCALE)
            if kind == GEN:
                nc.vector.tensor_mul(
                    ep[:, w0:],
                    ep[:, w0:],
                    msk_sb[:, mid * TB : mid * TB + TB - w0],
                )
            ctx[(tb, h)]["eps"][idx] = ep

        def emit_acc(it):
            tb, h, idx = it
            entries = plan[tb]
            s, w0, kind, mid = entries[idx]
            c = ctx[(tb, h)]
            ep = c["eps"].pop(idx)
            first, last = idx == 0, idx == len(entries) - 1
            nc.tensor.matmul(
                c["den"][:, w0:],
                lhsT=ones_sb,
                rhs=ep[:, w0:],
                start=first,
                stop=last,
                skip_group_check=True,
            )
            nc.tensor.matmul(
                c["oT"][:, w0:],
                lhsT=vch[s],
                rhs=ep[:, w0:],
                start=first,
                stop=last,
                skip_group_check=True,
            )
            if last:
                rcp = p2sb.tile([128, TB], F32, name="rcp", tag="rcp")
                nc.vector.reciprocal(rcp, c["den"])
                oT_sb = outp.tile([128, TB], BF16, name="oT", tag="oT")
                nc.vector.tensor_mul(oT_sb, c["oT"], rcp)
                oT_live.setdefault(tb, []).append(oT_sb)
                del ctx[(tb, h)]
                if h == G - 1:
                    emit_oproj(tb)

        from collections import deque

        pend = deque()
        for it in items:
            emit_score(it)
            pend.append(it)
            if len(pend) > LA:
                emit_acc(pend.popleft())
        while pend:
            emit_acc(pend.popleft())

        outp.release()
        p3sb.release()
        p2sb.release()
        rpool.release()
        ps.release()
        xp.release()
        qkv.release()
        wop.release()
        const.release()

    nc.compile()
    return nc


def _prep_inputs(x, cos, sin, Wq, Wk, Wv, Wo, mask_tiles, n_masks):
    cos = np.asarray(cos, dtype=np.float32).reshape(T, HEAD_DIM // 2)
    sin = np.asarray(sin, dtype=np.float32).reshape(T, HEAD_DIM // 2)
    ctab = np.ascontiguousarray(np.repeat(cos, 2, axis=1).T).astype(BF)  # [128, T]
    s2 = np.repeat(sin, 2, axis=1)
    s2[:, 0::2] *= -1.0
    stab = np.ascontiguousarray(s2.T).astype(BF)
    trineg = (-512.0 * (np.arange(ST)[:, None] > np.arange(ST)[None, :])).astype(BF)

    xTb = [
        np.ascontiguousarray(np.asarray(x[b], dtype=np.float32).T).astype(BF)
        for b in range(B)
    ]
    in_maps = []
    for core in range(8):
        b, g = divmod(core, NUM_KV_HEADS)
        m = {
            "xT": xTb[b],
            "wq": np.ascontiguousarray(Wq[:, g * 512 : (g + 1) * 512]).astype(BF),
            "wk": np.ascontiguousarray(Wk[:, g * 128 : (g + 1) * 128]).astype(BF),
            "wv": np.ascontiguousarray(Wv[:, g * 128 : (g + 1) * 128]).astype(BF),
            "wo": np.ascontiguousarray(Wo[g * 512 : (g + 1) * 512, :]).astype(BF),
            "ctab": ctab,
            "stab": stab,
            "ones": np.ones((128, 128), dtype=BF),
            "ident": np.eye(128, dtype=BF),
            "tri": trineg,
        }
        if n_masks:
            m["masks"] = mask_tiles.reshape(n_masks * ST, TB).astype(BF)
        in_maps.append(m)
    return in_maps


def kernel(x, cos, sin, mask, Wq, Wk, Wv, Wo, _trace=False, _result_box=None):
    from concourse.bass_utils import run_bass_kernel_spmd

    mask2d = np.asarray(mask).reshape(T, T).astype(bool)
    plan, mask_tiles = _classify_mask(mask2d)
    n_masks = int(mask_tiles.shape[0])

    key = (plan, n_masks)
    nc = _nc_cache.get(key)
    if nc is None:
        nc = _build(plan, n_masks)
        _nc_cache[key] = nc

    in_maps = _prep_inputs(x, cos, sin, Wq, Wk, Wv, Wo, mask_tiles, n_masks)
    res = run_bass_kernel_spmd(nc, in_maps, core_ids=list(range(8)), trace=_trace)
    if _result_box is not None:
        _result_box.append(res)

    out = np.zeros((B, T, C), dtype=np.float32)
    for core in range(8):
        b = core // NUM_KV_HEADS
        out[b] += res.results[core]["out"]
    return out


# revision 17
# speedup vs baseline: 1.8966x; 1.1378x over previous
"""Grouped-Query Attention (B=2, T=2048, C=2048, 16 Q heads / 4 KV heads,
D=128) on 8 Trainium2 NeuronCores.

Sharding: core (b, g) for b in {0,1}, g in {0..3} handles batch b and KV head
g (= query heads 4g..4g+3). Each core computes its 4 heads' attention plus the
partial output projection against its 512-row slice of Wo; the host sums the
4 partials per batch (the "all-reduce" of the o_proj, done in numpy).

All matmul operands are bf16 (host-cast); PSUM accumulation stays fp32, so
the only precision loss is input rounding (~4e-3 rel err vs the 2e-2 gate).

Layout/scheduling notes (from trace analysis):
- One PSUM pool with 8 [128,512]-f32 bank tags reused across stages (no
  mid-kernel pool releases -> no cross-stage drain bubbles; the PE pstate
  ramp resets on idle gaps, so a dense PE queue is worth ~1.5x clock).
- Startup DMAs interleaved per contraction chunk (wq/wk/wv/x) so the first
  projection matmul unblocks after ~4 transfers instead of all weights.
- Softmax denominator is computed REPLICATED across all 128 partitions
  (lhsT = all-ones [128,128]) so the reciprocal runs as a full-width DVE op
  (~0.65us) instead of a 1-partition op (3.3us) + GpSimd partition
  broadcast; the per-head tail stall on the PE disappears.
- Stage-2 software pipeline: score matmuls run 2 s-tiles ahead of the
  dependent den/PV matmuls so the PE never waits on ScalarE's exp.
- Diagonal (causal-boundary) s-tiles only compute the t-window right of the
  diagonal plus one shared [128,128] triangular 0/1 multiply.
"""
import sys

sys.path.insert(0, "/opt/trn_rl_repo")

import numpy as np
import ml_dtypes

B, T, C = 2, 2048, 2048
NUM_HEADS, NUM_KV_HEADS, HEAD_DIM = 16, 4, 128
G = NUM_HEADS // NUM_KV_HEADS  # 4 query heads per core
SCALE = float(HEAD_DIM) ** -0.5
TB = 512  # t-block (matmul moving free dim)
NTB = T // TB  # 4
ST = 128  # s-tile
NST = T // ST  # 16
NCT = C // 128  # 16 contraction tiles
LA = 2  # stage-2 score-matmul lookahead (s-tiles in flight past exp)

SWAP_MASK = [i ^ 1 for i in range(32)]
BF = ml_dtypes.bfloat16

_nc_cache: dict = {}

# plan entry kinds
FULL, DIAG, GEN = 0, 1, 2


def _classify_mask(mask2d: np.ndarray):
    """mask2d[t, s] bool. Returns (plan, mask_tiles).

    plan[tb] = tuple of (s_tile_idx, w0, kind, mask_id). w0 is the t-window
    start within the t-block (columns < w0 are entirely masked for this
    s-tile). kind: FULL (no mask work), DIAG (shared lower-triangular 0/1
    multiply on the first 128 window columns), GEN (per-tile 0/1 multiply
    over the whole window; mask_id indexes mask_tiles)."""
    tri = (np.arange(ST)[:, None] <= np.arange(ST)[None, :])
    plan = []
    uniq: dict = {}
    tiles = []
    for tb in range(NTB):
        sub_t = mask2d[tb * TB : (tb + 1) * TB]  # [TB(t), T(s)]
        entries = []
        for s in range(NST):
            sub = sub_t[:, s * ST : (s + 1) * ST]  # [TB(t), ST(s)]
            if sub.all():
                entries.append((s, 0, FULL, None))
                continue
            if not sub.any():
                continue
            m = sub.T  # [s, t]
            w0 = 0
            while w0 + ST <= TB and not m[:, w0 : w0 + ST].any():
                w0 += ST
            win = m[:, w0:]
            if (
                win.shape[1] >= ST
                and (win[:, :ST] == tri).all()
                and win[:, ST:].all()
            ):
                entries.append((s, w0, DIAG, None))
                continue
            tile_m = np.zeros((ST, TB), dtype=np.float32)
            tile_m[:, : TB - w0] = win.astype(np.float32)
            key = (w0, tile_m.tobytes())
            mid = uniq.get(key)
            if mid is None:
                mid = len(tiles)
                uniq[key] = mid
                tiles.append(tile_m)
            entries.append((s, w0, GEN, mid))
        plan.append(tuple(entries))
    mask_tiles = (
        np.stack(tiles) if tiles else np.zeros((0, ST, TB), dtype=np.float32)
    )
    return tuple(plan), mask_tiles


def _build(plan, n_masks):
    import concourse.bacc as bacc
    import concourse.mybir as mybir
    import concourse.tile as tile

    F32 = mybir.dt.float32
    BF16 = mybir.dt.bfloat16
    Exp = mybir.ActivationFunctionType.Exp

    nc = bacc.Bacc()

    xT_d = nc.declare_dram_parameter("xT", [C, T], BF16, isOutput=False)
    wq_d = nc.declare_dram_parameter("wq", [C, G * HEAD_DIM], BF16, isOutput=False)
    wk_d = nc.declare_dram_parameter("wk", [C, HEAD_DIM], BF16, isOutput=False)
    wv_d = nc.declare_dram_parameter("wv", [C, HEAD_DIM], BF16, isOutput=False)
    wo_d = nc.declare_dram_parameter("wo", [G * HEAD_DIM, C], BF16, isOutput=False)
    on_d = nc.declare_dram_parameter("ones", [128, 128], BF16, isOutput=False)
    id_d = nc.declare_dram_parameter("ident", [128, 128], BF16, isOutput=False)
    tr_d = nc.declare_dram_parameter("tri", [ST, ST], BF16, isOutput=False)
    ct_d = nc.declare_dram_parameter("ctab", [HEAD_DIM, T], BF16, isOutput=False)
    st_d = nc.declare_dram_parameter("stab", [HEAD_DIM, T], BF16, isOutput=False)
    if n_masks:
        mk_d = nc.declare_dram_parameter(
            "masks", [n_masks * ST, TB], BF16, isOutput=False
        )
    out_d = nc.declare_dram_parameter("out", [T, C], F32, isOutput=True)

    with tile.TileContext(nc) as tc:
        const = tc.alloc_tile_pool(name="const", bufs=1)
        wop = tc.alloc_tile_pool(name="wop", bufs=1)
        qkv = tc.alloc_tile_pool(name="qkv", bufs=1)
        xp = tc.alloc_tile_pool(name="xp", bufs=1)

        # --- interleaved startup DMAs: per-chunk weights + x so the first
        # projection matmuls unblock after a handful of transfers ---
        wq_sb = [wop.tile([128, G * HEAD_DIM], BF16, name=f"wq{i}") for i in range(NCT)]
        wk_sb = [wop.tile([128, HEAD_DIM], BF16, name=f"wk{i}") for i in range(NCT)]
        wv_sb = [wop.tile([128, HEAD_DIM], BF16, name=f"wv{i}") for i in range(NCT)]
        xts = [xp.tile([128, T], BF16, name=f"xt{i}") for i in range(NCT)]
        for i in range(NCT):
            sl = slice(i * 128, (i + 1) * 128)
            nc.sync.dma_start(out=wq_sb[i], in_=wq_d.ap()[sl, :])
            nc.sync.dma_start(out=wk_sb[i], in_=wk_d.ap()[sl, :])
            nc.sync.dma_start(out=wv_sb[i], in_=wv_d.ap()[sl, :])
            nc.sync.dma_start(out=xts[i], in_=xT_d.ap()[sl, :])

        ctab = const.tile([HEAD_DIM, T], BF16, name="ctab")
        stab = const.tile([HEAD_DIM, T], BF16, name="stab")
        nc.sync.dma_start(out=ctab, in_=ct_d.ap())
        nc.sync.dma_start(out=stab, in_=st_d.ap())
        ones_sb = const.tile([128, 128], BF16, name="ones_sb")
        ident = const.tile([128, 128], BF16, name="ident")
        trineg = const.tile([ST, ST], BF16, name="trineg")
        nc.sync.dma_start(out=ones_sb, in_=on_d.ap())
        nc.sync.dma_start(out=ident, in_=id_d.ap())
        nc.sync.dma_start(out=trineg, in_=tr_d.ap())
        if n_masks:
            msk_sb = const.tile([ST, n_masks * TB], BF16, name="msk_sb")
            for i in range(n_masks):
                nc.sync.dma_start(
                    out=msk_sb[:, i * TB : (i + 1) * TB],
                    in_=mk_d.ap()[i * ST : (i + 1) * ST, :],
                )
        wo_sb = [wop.tile([128, C], BF16, name=f"wo{h}") for h in range(G)]
        for h in range(G):
            nc.sync.dma_start(out=wo_sb[h], in_=wo_d.ap()[h * 128 : (h + 1) * 128, :])

        # per-t-block tiles (not one [128, T] tile) so stage-2 readers only
        # depend on the t-blocks they actually use — tile-granular dependency
        # tracking would otherwise serialize stage 2 behind ALL RoPE work
        qT = [
            [qkv.tile([128, TB], BF16, name=f"qT{h}_{tb}") for tb in range(NTB)]
            for h in range(G)
        ]
        kT = [qkv.tile([128, TB], BF16, name=f"kT{tb}") for tb in range(NTB)]
        vT = [qkv.tile([128, TB], BF16, name=f"vT{tb}") for tb in range(NTB)]
        vch = [qkv.tile([128, 128], BF16, name=f"v{s}") for s in range(NST)]

        # single PSUM pool: 8 x [128, 512] f32 bank tags, reused across stages
        ps = tc.alloc_tile_pool(name="ps", bufs=1, space="PSUM")

        def bank(tag):
            return ps.tile([128, TB], F32, name=tag, tag=tag)

        rpool = tc.alloc_tile_pool(name="rpool", bufs=3)

        # ---- stage 1: projections + RoPE + v transpose, pipelined per tb ----
        for tb in range(NTB):
            tsl = slice(tb * TB, (tb + 1) * TB)
            q_ps = [bank(f"bk{h}") for h in range(G)]
            k_ps = bank("bk4")
            v_ps = bank("bk5")
            for ci in range(NCT):
                first, last = ci == 0, ci == NCT - 1
                rhs = xts[ci][:, tsl]
                for h in range(G):
                    nc.tensor.matmul(
                        q_ps[h],
                        lhsT=wq_sb[ci][:, h * 128 : (h + 1) * 128],
                        rhs=rhs,
                        start=first,
                        stop=last,
                    )
                nc.tensor.matmul(k_ps, lhsT=wk_sb[ci], rhs=rhs, start=first, stop=last)
                nc.tensor.matmul(v_ps, lhsT=wv_sb[ci], rhs=rhs, start=first, stop=last)

            # v first so the transposes don't queue behind the RoPE backlog,
            # then k (attention waits on it), then qs
            nc.vector.tensor_copy(vT[tb], v_ps)
            for r in range(4):
                s = 4 * tb + r
                vtp = bank(f"bk{6 + (r % 2)}").bitcast(BF16)[:, :128]
                nc.tensor.transpose(vtp, vT[tb][:, r * 128 : (r + 1) * 128], ident)
                nc.vector.tensor_copy(vch[s], vtp)
            for dst, src_ps in [(kT[tb], k_ps)] + [(qT[h][tb], q_ps[h]) for h in range(G)]:
                nc.vector.tensor_copy(dst, src_ps)
                swp = rpool.tile([128, TB], BF16, name="swp", tag="swp")
                tmp = rpool.tile([128, TB], BF16, name="tmp", tag="tmp")
                nc.vector.stream_shuffle(swp, dst, SWAP_MASK)
                nc.vector.tensor_mul(tmp, dst, ctab[:, tsl])
                nc.vector.tensor_mul(swp, swp, stab[:, tsl])
                nc.vector.tensor_add(dst, tmp, swp)

        # ---- stages 2+3: one global software pipeline over (tb, head, s-tile)
        # so head/t-block boundaries never drain the PE. Scores run LA s-tiles
        # ahead of the dependent den/PV matmuls (exp latency hidden). ----
        p2sb = tc.alloc_tile_pool(name="p2sb", bufs=4)
        p3sb = tc.alloc_tile_pool(name="p3sb", bufs=3)
        outp = tc.alloc_tile_pool(name="outp", bufs=10)
        state = {"score": 0, "head": 0, "ncopy": 0}
        oT_live: dict = {}

        def emit_oproj(tb):
            oT_sbs = oT_live.pop(tb)
            for tch in range(TB // 128):
                for cb in range(C // 512):
                    ops = bank(f"bk{2 + (cb % 2)}")
                    for h in range(G):
                        nc.tensor.matmul(
                            ops,
                            lhsT=oT_sbs[h][:, tch * 128 : (tch + 1) * 128],
                            rhs=wo_sb[h][:, cb * 512 : (cb + 1) * 512],
                            start=h == 0,
                            stop=h == G - 1,
                        )
                    osb = p3sb.tile([128, 512], F32, name="osb", tag="osb")
                    # keep these copies off DVE so rcp (den WAR chain) and
                    # the rescale muls never queue behind them
                    nc.scalar.copy(osb, ops)
                    state["ncopy"] += 1
                    t0 = tb * TB + tch * 128
                    nc.sync.dma_start(
                        out=out_d.ap()[t0 : t0 + 128, cb * 512 : (cb + 1) * 512],
                        in_=osb,
                    )

        items = []  # (tb, h, idx)
        for tb in range(NTB):
            for h in range(G):
                for idx in range(len(plan[tb])):
                    items.append((tb, h, idx))

        ctx: dict = {}  # (tb,h) -> dict with oT_ps, den, eps

        def emit_score(it):
            tb, h, idx = it
            entries = plan[tb]
            s, w0, kind, mid = entries[idx]
            # bank roles chosen so stage-2 tiles reuse the PSUM banks that
            # stage-1's trailing (tb=3) DVE stream releases earliest:
            # v (bk5) and vtp (bk6/7) first -> stp; k (bk4) -> den;
            # q0/q1 (bk0/1) -> oT; q2/q3 (bk2/3) -> o_proj accumulators
            if idx == 0:
                ctx[(tb, h)] = {
                    "oT": bank(f"bk{0 + (state['head'] % 2)}"),
                    "den": bank("bk4"),
                    "eps": {},
                }
                state["head"] += 1
            stp = bank(f"bk{5 + (state['score'] % 3)}")
            state["score"] += 1
            diag = kind == DIAG
            nc.tensor.matmul(
                stp[:, w0:],
                lhsT=kT[s // 4][:, (s % 4) * 128 : (s % 4 + 1) * 128],
                rhs=qT[h][tb][:, w0:],
                start=True,
                stop=not diag,
                skip_group_check=diag,
            )
            if diag:
                # additive -512*(s>t) triangular mask folded into the score
                # accumulation on the PE (keeps DVE off the critical path);
                # exp then underflows to ~e-18 which is negligible in den/PV
                nc.tensor.matmul(
                    stp[:, w0 : w0 + ST],
                    lhsT=ident,
                    rhs=trineg,
                    start=False,
                    stop=True,
                    skip_group_check=True,
                )
            ep = p2sb.tile([ST, TB], BF16, name="ep", tag="ep")
            nc.scalar.activation(ep[:, w0:], stp[:, w0:], Exp, scale=SCALE)
            if kind == GEN:
                nc.vector.tensor_mul(
                    ep[:, w0:],
                    ep[:, w0:],
                    msk_sb[:, mid * TB : mid * TB + TB - w0],
                )
            ctx[(tb, h)]["eps"][idx] = ep

        def emit_acc(it):
            tb, h, idx = it
            entries = plan[tb]
            s, w0, kind, mid = entries[idx]
            c = ctx[(tb, h)]
            ep = c["eps"].pop(idx)
            first, last = idx == 0, idx == len(entries) - 1
            nc.tensor.matmul(
                c["den"][:, w0:],
                lhsT=ones_sb,
                rhs=ep[:, w0:],
                start=first,
                stop=last,
                skip_group_check=True,
            )
            nc.tensor.matmul(
                c["oT"][:, w0:],
                lhsT=vch[s],
                rhs=ep[:, w0:],
                start=first,
                stop=last,
                skip_group_check=True,
            )
            if last:
                rcp = p2sb.tile([128, TB], F32, name="rcp", tag="rcp")
                # ~51-ULP approx is ample for the softmax denominator and 5x
                # faster than reciprocal() — this sits on the den-bank WAR
                # chain the PE waits on at each head switch
                nc.vector.reciprocal_approx_fast(rcp, c["den"])
                oT_sb = outp.tile([128, TB], BF16, name="oT", tag="oT")
                nc.vector.tensor_mul(oT_sb, c["oT"], rcp)
                oT_live.setdefault(tb, []).append(oT_sb)
                del ctx[(tb, h)]
                if h == G - 1:
                    emit_oproj(tb)

        from collections import deque

        pend = deque()
        for it in items:
            emit_score(it)
            pend.append(it)
            if len(pend) > LA:
                emit_acc(pend.popleft())
        while pend:
            emit_acc(pend.popleft())

        outp.release()
        p3sb.release()
        p2sb.release()
        rpool.release()
        ps.release()
        xp.release()
        qkv.release()
        wop.release()
        const.release()

    nc.compile()
    return nc


def _prep_inputs(x, cos, sin, Wq, Wk, Wv, Wo, mask_tiles, n_masks):
    cos = np.asarray(cos, dtype=np.float32).reshape(T, HEAD_DIM // 2)
    sin = np.asarray(sin, dtype=np.float32).reshape(T, HEAD_DIM // 2)
    ctab = np.ascontiguousarray(np.repeat(cos, 2, axis=1).T).astype(BF)  # [128, T]
    s2 = np.repeat(sin, 2, axis=1)
    s2[:, 0::2] *= -1.0
    stab = np.ascontiguousarray(s2.T).astype(BF)
    trineg = (-512.0 * (np.arange(ST)[:, None] > np.arange(ST)[None, :])).astype(BF)

    xTb = [
        np.ascontiguousarray(np.asarray(x[b], dtype=np.float32).T).astype(BF)
        for b in range(B)
    ]
    in_maps = []
    for core in range(8):
        b, g = divmod(core, NUM_KV_HEADS)
        m = {
            "xT": xTb[b],
            "wq": np.ascontiguousarray(Wq[:, g * 512 : (g + 1) * 512]).astype(BF),
            "wk": np.ascontiguousarray(Wk[:, g * 128 : (g + 1) * 128]).astype(BF),
            "wv": np.ascontiguousarray(Wv[:, g * 128 : (g + 1) * 128]).astype(BF),
            "wo": np.ascontiguousarray(Wo[g * 512 : (g + 1) * 512, :]).astype(BF),
            "ctab": ctab,
            "stab": stab,
            "ones": np.ones((128, 128), dtype=BF),
            "ident": np.eye(128, dtype=BF),
            "tri": trineg,
        }
        if n_masks:
            m["masks"] = mask_tiles.reshape(n_masks * ST, TB).astype(BF)
        in_maps.append(m)
    return in_maps


def kernel(x, cos, sin, mask, Wq, Wk, Wv, Wo, _trace=False, _result_box=None):
    from concourse.bass_utils import run_bass_kernel_spmd

    mask2d = np.asarray(mask).reshape(T, T).astype(bool)
    plan, mask_tiles = _classify_mask(mask2d)
    n_masks = int(mask_tiles.shape[0])

    key = (plan, n_masks)
    nc = _nc_cache.get(key)
    if nc is None:
        nc = _build(plan, n_masks)
        _nc_cache[key] = nc

    in_maps = _prep_inputs(x, cos, sin, Wq, Wk, Wv, Wo, mask_tiles, n_masks)
    res = run_bass_kernel_spmd(nc, in_maps, core_ids=list(range(8)), trace=_trace)
    if _result_box is not None:
        _result_box.append(res)

    out = np.zeros((B, T, C), dtype=np.float32)
    for core in range(8):
        b = core // NUM_KV_HEADS
        out[b] += res.results[core]["out"]
    return out


# revision 21
# speedup vs baseline: 1.9160x; 1.0103x over previous
"""Grouped-Query Attention (B=2, T=2048, C=2048, 16 Q heads / 4 KV heads,
D=128) on 8 Trainium2 NeuronCores.

Sharding: core (b, g) for b in {0,1}, g in {0..3} handles batch b and KV head
g (= query heads 4g..4g+3). Each core computes its 4 heads' attention plus the
partial output projection against its 512-row slice of Wo; the host sums the
4 partials per batch (the "all-reduce" of the o_proj, done in numpy).

All matmul operands are bf16 (host-cast); PSUM accumulation stays fp32, so
the only precision loss is input rounding (~4e-3 rel err vs the 2e-2 gate).

Layout/scheduling notes (from trace analysis):
- One PSUM pool with 8 [128,512]-f32 bank tags reused across stages (no
  mid-kernel pool releases -> no cross-stage drain bubbles; the PE pstate
  ramp resets on idle gaps, so a dense PE queue is worth ~1.5x clock).
- Startup DMAs interleaved per contraction chunk (wq/wk/wv/x) so the first
  projection matmul unblocks after ~4 transfers instead of all weights.
- Softmax denominator is computed REPLICATED across all 128 partitions
  (lhsT = all-ones [128,128]) so the reciprocal runs as a full-width DVE op
  (~0.65us) instead of a 1-partition op (3.3us) + GpSimd partition
  broadcast; the per-head tail stall on the PE disappears.
- Stage-2 software pipeline: score matmuls run 2 s-tiles ahead of the
  dependent den/PV matmuls so the PE never waits on ScalarE's exp.
- Diagonal (causal-boundary) s-tiles only compute the t-window right of the
  diagonal plus one shared [128,128] triangular 0/1 multiply.
"""
import sys

sys.path.insert(0, "/opt/trn_rl_repo")

import numpy as np
import ml_dtypes

B, T, C = 2, 2048, 2048
NUM_HEADS, NUM_KV_HEADS, HEAD_DIM = 16, 4, 128
G = NUM_HEADS // NUM_KV_HEADS  # 4 query heads per core
SCALE = float(HEAD_DIM) ** -0.5
TB = 512  # t-block (matmul moving free dim)
NTB = T // TB  # 4
ST = 128  # s-tile
NST = T // ST  # 16
NCT = C // 128  # 16 contraction tiles
LA = 2  # stage-2 score-matmul lookahead (s-tiles in flight past exp)

SWAP_MASK = [i ^ 1 for i in range(32)]
BF = ml_dtypes.bfloat16

_nc_cache: dict = {}

# plan entry kinds
FULL, DIAG, GEN = 0, 1, 2


def _classify_mask(mask2d: np.ndarray):
    """mask2d[t, s] bool. Returns (plan, mask_tiles).

    plan[tb] = tuple of (s_tile_idx, w0, kind, mask_id). w0 is the t-window
    start within the t-block (columns < w0 are entirely masked for this
    s-tile). kind: FULL (no mask work), DIAG (shared lower-triangular 0/1
    multiply on the first 128 window columns), GEN (per-tile 0/1 multiply
    over the whole window; mask_id indexes mask_tiles)."""
    tri = (np.arange(ST)[:, None] <= np.arange(ST)[None, :])
    plan = []
    uniq: dict = {}
    tiles = []
    for tb in range(NTB):
        sub_t = mask2d[tb * TB : (tb + 1) * TB]  # [TB(t), T(s)]
        entries = []
        for s in range(NST):
            sub = sub_t[:, s * ST : (s + 1) * ST]  # [TB(t), ST(s)]
            if sub.all():
                entries.append((s, 0, FULL, None))
                continue
            if not sub.any():
                continue
            m = sub.T  # [s, t]
            w0 = 0
            while w0 + ST <= TB and not m[:, w0 : w0 + ST].any():
                w0 += ST
            win = m[:, w0:]
            if (
                win.shape[1] >= ST
                and (win[:, :ST] == tri).all()
                and win[:, ST:].all()
            ):
                entries.append((s, w0, DIAG, None))
                continue
            tile_m = np.zeros((ST, TB), dtype=np.float32)
            tile_m[:, : TB - w0] = win.astype(np.float32)
            key = (w0, tile_m.tobytes())
            mid = uniq.get(key)
            if mid is None:
                mid = len(tiles)
                uniq[key] = mid
                tiles.append(tile_m)
            entries.append((s, w0, GEN, mid))
        plan.append(tuple(entries))
    mask_tiles = (
        np.stack(tiles) if tiles else np.zeros((0, ST, TB), dtype=np.float32)
    )
    return tuple(plan), mask_tiles


def _build(plan, n_masks):
    import concourse.bacc as bacc
    import concourse.mybir as mybir
    import concourse.tile as tile

    F32 = mybir.dt.float32
    BF16 = mybir.dt.bfloat16
    Exp = mybir.ActivationFunctionType.Exp

    nc = bacc.Bacc()

    xT_d = nc.declare_dram_parameter("xT", [C, T], BF16, isOutput=False)
    wq_d = nc.declare_dram_parameter("wq", [C, G * HEAD_DIM], BF16, isOutput=False)
    wk_d = nc.declare_dram_parameter("wk", [C, HEAD_DIM], BF16, isOutput=False)
    wv_d = nc.declare_dram_parameter("wv", [C, HEAD_DIM], BF16, isOutput=False)
    wo_d = nc.declare_dram_parameter("wo", [G * HEAD_DIM, C], BF16, isOutput=False)
    on_d = nc.declare_dram_parameter("ones", [128, 128], BF16, isOutput=False)
    id_d = nc.declare_dram_parameter("ident", [128, 128], BF16, isOutput=False)
    tr_d = nc.declare_dram_parameter("tri", [ST, ST], BF16, isOutput=False)
    ct_d = nc.declare_dram_parameter("ctab", [HEAD_DIM, T], BF16, isOutput=False)
    st_d = nc.declare_dram_parameter("stab", [HEAD_DIM, T], BF16, isOutput=False)
    if n_masks:
        mk_d = nc.declare_dram_parameter(
            "masks", [n_masks * ST, TB], BF16, isOutput=False
        )
    out_d = nc.declare_dram_parameter("out", [T, C], F32, isOutput=True)

    with tile.TileContext(nc) as tc:
        const = tc.alloc_tile_pool(name="const", bufs=1)
        wop = tc.alloc_tile_pool(name="wop", bufs=1)
        qkv = tc.alloc_tile_pool(name="qkv", bufs=1)
        xp = tc.alloc_tile_pool(name="xp", bufs=1)

        # --- interleaved startup DMAs: per-chunk weights + x so the first
        # projection matmuls unblock after a handful of transfers ---
        wq_sb = [wop.tile([128, G * HEAD_DIM], BF16, name=f"wq{i}") for i in range(NCT)]
        wk_sb = [wop.tile([128, HEAD_DIM], BF16, name=f"wk{i}") for i in range(NCT)]
        wv_sb = [wop.tile([128, HEAD_DIM], BF16, name=f"wv{i}") for i in range(NCT)]
        # x split into tb0-slice + rest tiles (separate tiles, deps are
        # tile-granular) so tb0's projection pass only waits on 4.7MB
        # (weights + tb0 x slices), not the full 11MB
        xt0 = [xp.tile([128, TB], BF16, name=f"xt0_{i}") for i in range(NCT)]
        xtr = [xp.tile([128, T - TB], BF16, name=f"xtr{i}") for i in range(NCT)]
        for i in range(NCT):
            sl = slice(i * 128, (i + 1) * 128)
            nc.sync.dma_start(out=wq_sb[i], in_=wq_d.ap()[sl, :])
            nc.sync.dma_start(out=wk_sb[i], in_=wk_d.ap()[sl, :])
            nc.sync.dma_start(out=wv_sb[i], in_=wv_d.ap()[sl, :])
            nc.sync.dma_start(out=xt0[i], in_=xT_d.ap()[sl, :TB])
        for i in range(NCT):
            sl = slice(i * 128, (i + 1) * 128)
            nc.sync.dma_start(out=xtr[i], in_=xT_d.ap()[sl, TB:])

        ctab = const.tile([HEAD_DIM, T], BF16, name="ctab")
        stab = const.tile([HEAD_DIM, T], BF16, name="stab")
        nc.sync.dma_start(out=ctab, in_=ct_d.ap())
        nc.sync.dma_start(out=stab, in_=st_d.ap())
        ones_sb = const.tile([128, 128], BF16, name="ones_sb")
        ident = const.tile([128, 128], BF16, name="ident")
        trineg = const.tile([ST, ST], BF16, name="trineg")
        nc.sync.dma_start(out=ones_sb, in_=on_d.ap())
        nc.sync.dma_start(out=ident, in_=id_d.ap())
        nc.sync.dma_start(out=trineg, in_=tr_d.ap())
        if n_masks:
            msk_sb = const.tile([ST, n_masks * TB], BF16, name="msk_sb")
            for i in range(n_masks):
                nc.sync.dma_start(
                    out=msk_sb[:, i * TB : (i + 1) * TB],
                    in_=mk_d.ap()[i * ST : (i + 1) * ST, :],
                )
        wo_sb = [wop.tile([128, C], BF16, name=f"wo{h}") for h in range(G)]
        for h in range(G):
            nc.sync.dma_start(out=wo_sb[h], in_=wo_d.ap()[h * 128 : (h + 1) * 128, :])

        # per-t-block tiles (not one [128, T] tile) so stage-2 readers only
        # depend on the t-blocks they actually use — tile-granular dependency
        # tracking would otherwise serialize stage 2 behind ALL RoPE work
        qT = [
            [qkv.tile([128, TB], BF16, name=f"qT{h}_{tb}") for tb in range(NTB)]
            for h in range(G)
        ]
        kT = [qkv.tile([128, TB], BF16, name=f"kT{tb}") for tb in range(NTB)]
        vT = [qkv.tile([128, TB], BF16, name=f"vT{tb}") for tb in range(NTB)]
        vch = [qkv.tile([128, 128], BF16, name=f"v{s}") for s in range(NST)]

        # single PSUM pool: 8 x [128, 512] f32 bank tags, reused across stages
        ps = tc.alloc_tile_pool(name="ps", bufs=1, space="PSUM")

        def bank(tag):
            return ps.tile([128, TB], F32, name=tag, tag=tag)

        rpool = tc.alloc_tile_pool(name="rpool", bufs=3)

        # ---- stage 1: projections + RoPE + v transpose, pipelined per tb ----
        for tb in range(NTB):
            tsl = slice(tb * TB, (tb + 1) * TB)
            q_ps = [bank(f"bk{h}") for h in range(G)]
            k_ps = bank("bk4")
            v_ps = bank("bk5")
            for ci in range(NCT):
                first, last = ci == 0, ci == NCT - 1
                rhs = (
                    xt0[ci]
                    if tb == 0
                    else xtr[ci][:, (tb - 1) * TB : tb * TB]
                )
                for h in range(G):
                    nc.tensor.matmul(
                        q_ps[h],
                        lhsT=wq_sb[ci][:, h * 128 : (h + 1) * 128],
                        rhs=rhs,
                        start=first,
                        stop=last,
                    )
                nc.tensor.matmul(k_ps, lhsT=wk_sb[ci], rhs=rhs, start=first, stop=last)
                nc.tensor.matmul(v_ps, lhsT=wv_sb[ci], rhs=rhs, start=first, stop=last)

            # v first so the transposes don't queue behind the RoPE backlog,
            # then k (attention waits on it), then qs
            nc.vector.tensor_copy(vT[tb], v_ps)
            for r in range(4):
                s = 4 * tb + r
                vtp = bank(f"bk{6 + (r % 2)}").bitcast(BF16)[:, :128]
                nc.tensor.transpose(vtp, vT[tb][:, r * 128 : (r + 1) * 128], ident)
                nc.vector.tensor_copy(vch[s], vtp)
            for dst, src_ps in [(kT[tb], k_ps)] + [(qT[h][tb], q_ps[h]) for h in range(G)]:
                nc.vector.tensor_copy(dst, src_ps)
                swp = rpool.tile([128, TB], BF16, name="swp", tag="swp")
                tmp = rpool.tile([128, TB], BF16, name="tmp", tag="tmp")
                nc.vector.stream_shuffle(swp, dst, SWAP_MASK)
                nc.vector.tensor_mul(tmp, dst, ctab[:, tsl])
                nc.vector.tensor_mul(swp, swp, stab[:, tsl])
                nc.vector.tensor_add(dst, tmp, swp)

        # ---- stages 2+3: one global software pipeline over (tb, head, s-tile)
        # so head/t-block boundaries never drain the PE. Scores run LA s-tiles
        # ahead of the dependent den/PV matmuls (exp latency hidden). ----
        p2sb = tc.alloc_tile_pool(name="p2sb", bufs=4)
        p3sb = tc.alloc_tile_pool(name="p3sb", bufs=3)
        outp = tc.alloc_tile_pool(name="outp", bufs=10)
        state = {"score": 0, "head": 0, "ncopy": 0}
        oT_live: dict = {}

        def emit_oproj(tb):
            oT_sbs = oT_live.pop(tb)
            for tch in range(TB // 128):
                for cb in range(C // 512):
                    ops = bank(f"bk{2 + (cb % 2)}")
                    for h in range(G):
                        nc.tensor.matmul(
                            ops,
                            lhsT=oT_sbs[h][:, tch * 128 : (tch + 1) * 128],
                            rhs=wo_sb[h][:, cb * 512 : (cb + 1) * 512],
                            start=h == 0,
                            stop=h == G - 1,
                        )
                    osb = p3sb.tile([128, 512], F32, name="osb", tag="osb")
                    # keep these copies off DVE so rcp (den WAR chain) and
                    # the rescale muls never queue behind them
                    nc.scalar.copy(osb, ops)
                    state["ncopy"] += 1
                    t0 = tb * TB + tch * 128
                    nc.sync.dma_start(
                        out=out_d.ap()[t0 : t0 + 128, cb * 512 : (cb + 1) * 512],
                        in_=osb,
                    )

        items = []  # (tb, h, idx)
        for tb in range(NTB):
            for h in range(G):
                for idx in range(len(plan[tb])):
                    items.append((tb, h, idx))

        ctx: dict = {}  # (tb,h) -> dict with oT_ps, den, eps

        def emit_score(it):
            tb, h, idx = it
            entries = plan[tb]
            s, w0, kind, mid = entries[idx]
            # bank roles chosen so stage-2 tiles reuse the PSUM banks that
            # stage-1's trailing (tb=3) DVE stream releases earliest:
            # v (bk5) and vtp (bk6/7) first -> stp; k (bk4) -> den;
            # q0/q1 (bk0/1) -> oT; q2/q3 (bk2/3) -> o_proj accumulators
            if idx == 0:
                ctx[(tb, h)] = {
                    "oT": bank(f"bk{0 + (state['head'] % 2)}"),
                    "den": bank("bk4"),
                    "eps": {},
                }
                state["head"] += 1
            stp = bank(f"bk{5 + (state['score'] % 3)}")
            state["score"] += 1
            diag = kind == DIAG
            nc.tensor.matmul(
                stp[:, w0:],
                lhsT=kT[s // 4][:, (s % 4) * 128 : (s % 4 + 1) * 128],
                rhs=qT[h][tb][:, w0:],
                start=True,
                stop=not diag,
                skip_group_check=diag,
            )
            if diag:
                # additive -512*(s>t) triangular mask folded into the score
                # accumulation on the PE (keeps DVE off the critical path);
                # exp then underflows to ~e-18 which is negligible in den/PV
                nc.tensor.matmul(
                    stp[:, w0 : w0 + ST],
                    lhsT=ident,
                    rhs=trineg,
                    start=False,
                    stop=True,
                    skip_group_check=True,
                )
            ep = p2sb.tile([ST, TB], BF16, name="ep", tag="ep")
            nc.scalar.activation(ep[:, w0:], stp[:, w0:], Exp, scale=SCALE)
            if kind == GEN:
                nc.vector.tensor_mul(
                    ep[:, w0:],
                    ep[:, w0:],
                    msk_sb[:, mid * TB : mid * TB + TB - w0],
                )
            ctx[(tb, h)]["eps"][idx] = ep

        def emit_acc(it):
            tb, h, idx = it
            entries = plan[tb]
            s, w0, kind, mid = entries[idx]
            c = ctx[(tb, h)]
            ep = c["eps"].pop(idx)
            first, last = idx == 0, idx == len(entries) - 1
            nc.tensor.matmul(
                c["den"][:, w0:],
                lhsT=ones_sb,
                rhs=ep[:, w0:],
                start=first,
                stop=last,
                skip_group_check=True,
            )
            nc.tensor.matmul(
                c["oT"][:, w0:],
                lhsT=vch[s],
                rhs=ep[:, w0:],
                start=first,
                stop=last,
                skip_group_check=True,
            )
            if last:
                rcp = p2sb.tile([128, TB], F32, name="rcp", tag="rcp")
                # ~51-ULP approx is ample for the softmax denominator and 5x
                # faster than reciprocal() — this sits on the den-bank WAR
                # chain the PE waits on at each head switch
                nc.vector.reciprocal_approx_fast(rcp, c["den"])
                oT_sb = outp.tile([128, TB], BF16, name="oT", tag="oT")
                nc.vector.tensor_mul(oT_sb, c["oT"], rcp)
                oT_live.setdefault(tb, []).append(oT_sb)
                del ctx[(tb, h)]
                # o_proj for t-block tb is emitted one head LATER (during
                # (tb+1, h0)'s attention) so its lhsT never waits on the
                # rescale chain of tb's last head
                if h == 0 and tb > 0:
                    emit_oproj(tb - 1)
                if tb == NTB - 1 and h == G - 1:
                    emit_oproj(tb)

        from collections import deque

        pend = deque()
        for it in items:
            emit_score(it)
            pend.append(it)
            if len(pend) > LA:
                emit_acc(pend.popleft())
        while pend:
            emit_acc(pend.popleft())

        outp.release()
        p3sb.release()
        p2sb.release()
        rpool.release()
        ps.release()
        xp.release()
        qkv.release()
        wop.release()
        const.release()

    nc.compile()
    return nc


def _prep_inputs(x, cos, sin, Wq, Wk, Wv, Wo, mask_tiles, n_masks):
    cos = np.asarray(cos, dtype=np.float32).reshape(T, HEAD_DIM // 2)
    sin = np.asarray(sin, dtype=np.float32).reshape(T, HEAD_DIM // 2)
    ctab = np.ascontiguousarray(np.repeat(cos, 2, axis=1).T).astype(BF)  # [128, T]
    s2 = np.repeat(sin, 2, axis=1)
    s2[:, 0::2] *= -1.0
    stab = np.ascontiguousarray(s2.T).astype(BF)
    trineg = (-512.0 * (np.arange(ST)[:, None] > np.arange(ST)[None, :])).astype(BF)

    xTb = [
        np.ascontiguousarray(np.asarray(x[b], dtype=np.float32).T).astype(BF)
        for b in range(B)
    ]
    in_maps = []
    for core in range(8):
        b, g = divmod(core, NUM_KV_HEADS)
        m = {
            "xT": xTb[b],
            "wq": np.ascontiguousarray(Wq[:, g * 512 : (g + 1) * 512]).astype(BF),
            "wk": np.ascontiguousarray(Wk[:, g * 128 : (g + 1) * 128]).astype(BF),
            "wv": np.ascontiguousarray(Wv[:, g * 128 : (g + 1) * 128]).astype(BF),
            "wo": np.ascontiguousarray(Wo[g * 512 : (g + 1) * 512, :]).astype(BF),
            "ctab": ctab,
            "stab": stab,
            "ones": np.ones((128, 128), dtype=BF),
            "ident": np.eye(128, dtype=BF),
            "tri": trineg,
        }
        if n_masks:
            m["masks"] = mask_tiles.reshape(n_masks * ST, TB).astype(BF)
        in_maps.append(m)
    return in_maps


def kernel(x, cos, sin, mask, Wq, Wk, Wv, Wo, _trace=False, _result_box=None):
    from concourse.bass_utils import run_bass_kernel_spmd

    mask2d = np.asarray(mask).reshape(T, T).astype(bool)
    plan, mask_tiles = _classify_mask(mask2d)
    n_masks = int(mask_tiles.shape[0])

    key = (plan, n_masks)
    nc = _nc_cache.get(key)
    if nc is None:
        nc = _build(plan, n_masks)
        _nc_cache[key] = nc

    in_maps = _prep_inputs(x, cos, sin, Wq, Wk, Wv, Wo, mask_tiles, n_masks)
    res = run_bass_kernel_spmd(nc, in_maps, core_ids=list(range(8)), trace=_trace)
    if _result_box is not None:
        _result_box.append(res)

    out = np.zeros((B, T, C), dtype=np.float32)
    for core in range(8):
        b = core // NUM_KV_HEADS
        out[b] += res.results[core]["out"]
    return out


# revision 26
# speedup vs baseline: 2.0100x; 1.0491x over previous
"""Grouped-Query Attention (B=2, T=2048, C=2048, 16 Q heads / 4 KV heads,
D=128) on 8 Trainium2 NeuronCores.

Sharding: core (b, g) for b in {0,1}, g in {0..3} handles batch b and KV head
g (= query heads 4g..4g+3). Each core computes its 4 heads' attention plus the
partial output projection against its 512-row slice of Wo; the host sums the
4 partials per batch (the "all-reduce" of the o_proj, done in numpy).

All matmul operands are bf16 (host-cast); PSUM accumulation stays fp32, so
the only precision loss is input rounding (~4e-3 rel err vs the 2e-2 gate).

Layout/scheduling notes (from trace analysis):
- One PSUM pool with 8 [128,512]-f32 bank tags reused across stages (no
  mid-kernel pool releases -> no cross-stage drain bubbles; the PE pstate
  ramp resets on idle gaps, so a dense PE queue is worth ~1.5x clock).
- Startup DMAs interleaved per contraction chunk (wq/wk/wv/x) so the first
  projection matmul unblocks after ~4 transfers instead of all weights.
- Softmax denominator is computed REPLICATED across all 128 partitions
  (lhsT = all-ones [128,128]) so the reciprocal runs as a full-width DVE op
  (~0.65us) instead of a 1-partition op (3.3us) + GpSimd partition
  broadcast; the per-head tail stall on the PE disappears.
- Stage-2 software pipeline: score matmuls run 2 s-tiles ahead of the
  dependent den/PV matmuls so the PE never waits on ScalarE's exp.
- Diagonal (causal-boundary) s-tiles only compute the t-window right of the
  diagonal plus one shared [128,128] triangular 0/1 multiply.
"""
import sys

sys.path.insert(0, "/opt/trn_rl_repo")

import numpy as np
import ml_dtypes

B, T, C = 2, 2048, 2048
NUM_HEADS, NUM_KV_HEADS, HEAD_DIM = 16, 4, 128
G = NUM_HEADS // NUM_KV_HEADS  # 4 query heads per core
SCALE = float(HEAD_DIM) ** -0.5
TB = 512  # t-block (matmul moving free dim)
NTB = T // TB  # 4
ST = 128  # s-tile
NST = T // ST  # 16
NCT = C // 128  # 16 contraction tiles
LA = 2  # stage-2 score-matmul lookahead (s-tiles in flight past exp)

SWAP_MASK = [i ^ 1 for i in range(32)]
BF = ml_dtypes.bfloat16

_nc_cache: dict = {}

# plan entry kinds
FULL, DIAG, GEN = 0, 1, 2


def _classify_mask(mask2d: np.ndarray):
    """mask2d[t, s] bool. Returns (plan, mask_tiles).

    plan[tb] = tuple of (s_tile_idx, w0, kind, mask_id). w0 is the t-window
    start within the t-block (columns < w0 are entirely masked for this
    s-tile). kind: FULL (no mask work), DIAG (shared lower-triangular 0/1
    multiply on the first 128 window columns), GEN (per-tile 0/1 multiply
    over the whole window; mask_id indexes mask_tiles)."""
    tri = (np.arange(ST)[:, None] <= np.arange(ST)[None, :])
    plan = []
    uniq: dict = {}
    tiles = []
    for tb in range(NTB):
        sub_t = mask2d[tb * TB : (tb + 1) * TB]  # [TB(t), T(s)]
        entries = []
        for s in range(NST):
            sub = sub_t[:, s * ST : (s + 1) * ST]  # [TB(t), ST(s)]
            if sub.all():
                entries.append((s, 0, FULL, None))
                continue
            if not sub.any():
                continue
            m = sub.T  # [s, t]
            w0 = 0
            while w0 + ST <= TB and not m[:, w0 : w0 + ST].any():
                w0 += ST
            win = m[:, w0:]
            if (
                win.shape[1] >= ST
                and (win[:, :ST] == tri).all()
                and win[:, ST:].all()
            ):
                entries.append((s, w0, DIAG, None))
                continue
            tile_m = np.zeros((ST, TB), dtype=np.float32)
            tile_m[:, : TB - w0] = win.astype(np.float32)
            key = (w0, tile_m.tobytes())
            mid = uniq.get(key)
            if mid is None:
                mid = len(tiles)
                uniq[key] = mid
                tiles.append(tile_m)
            entries.append((s, w0, GEN, mid))
        plan.append(tuple(entries))
    mask_tiles = (
        np.stack(tiles) if tiles else np.zeros((0, ST, TB), dtype=np.float32)
    )
    return tuple(plan), mask_tiles


def _build(plan, n_masks):
    import concourse.bacc as bacc
    import concourse.mybir as mybir
    import concourse.tile as tile

    F32 = mybir.dt.float32
    BF16 = mybir.dt.bfloat16
    Exp = mybir.ActivationFunctionType.Exp

    nc = bacc.Bacc()

    xT_d = nc.declare_dram_parameter("xT", [C, T], BF16, isOutput=False)
    # wqkv = [Wq | Wk | Wv] columns, one DMA per 128-row chunk
    wqkv_d = nc.declare_dram_parameter(
        "wqkv", [C, (G + 2) * HEAD_DIM], BF16, isOutput=False
    )
    wo_d = nc.declare_dram_parameter("wo", [G * HEAD_DIM, C], BF16, isOutput=False)
    on_d = nc.declare_dram_parameter("ones", [128, 128], BF16, isOutput=False)
    id_d = nc.declare_dram_parameter("ident", [128, 128], BF16, isOutput=False)
    tr_d = nc.declare_dram_parameter("tri", [ST, ST], BF16, isOutput=False)
    ct_d = nc.declare_dram_parameter("ctab", [HEAD_DIM, T], BF16, isOutput=False)
    st_d = nc.declare_dram_parameter("stab", [HEAD_DIM, T], BF16, isOutput=False)
    if n_masks:
        mk_d = nc.declare_dram_parameter(
            "masks", [n_masks * ST, TB], BF16, isOutput=False
        )
    out_d = nc.declare_dram_parameter("out", [T, C], F32, isOutput=True)

    with tile.TileContext(nc) as tc:
        const = tc.alloc_tile_pool(name="const", bufs=1)
        wop = tc.alloc_tile_pool(name="wop", bufs=1)
        qkv = tc.alloc_tile_pool(name="qkv", bufs=1)
        xp = tc.alloc_tile_pool(name="xp", bufs=1)

        # --- interleaved startup DMAs: per-chunk weights + x so the first
        # projection matmuls unblock after a handful of transfers ---
        wqkv_sb = [
            wop.tile([128, (G + 2) * HEAD_DIM], BF16, name=f"wqkv{i}")
            for i in range(NCT)
        ]
        # x split into tb0-slice + rest tiles (separate tiles, deps are
        # tile-granular) so tb0's projection pass only waits on 4.7MB
        # (weights + tb0 x slices), not the full 11MB
        xt0 = [xp.tile([128, TB], BF16, name=f"xt0_{i}") for i in range(NCT)]
        xtr = [xp.tile([128, T - TB], BF16, name=f"xtr{i}") for i in range(NCT)]
        for i in range(NCT):
            sl = slice(i * 128, (i + 1) * 128)
            nc.sync.dma_start(out=wqkv_sb[i], in_=wqkv_d.ap()[sl, :])
            nc.sync.dma_start(out=xt0[i], in_=xT_d.ap()[sl, :TB])
        for i in range(NCT):
            sl = slice(i * 128, (i + 1) * 128)
            nc.sync.dma_start(out=xtr[i], in_=xT_d.ap()[sl, TB:])

        ctab = const.tile([HEAD_DIM, T], BF16, name="ctab")
        stab = const.tile([HEAD_DIM, T], BF16, name="stab")
        nc.sync.dma_start(out=ctab, in_=ct_d.ap())
        nc.sync.dma_start(out=stab, in_=st_d.ap())
        ones_sb = const.tile([128, 128], BF16, name="ones_sb")
        ident = const.tile([128, 128], BF16, name="ident")
        trineg = const.tile([ST, ST], BF16, name="trineg")
        nc.sync.dma_start(out=ones_sb, in_=on_d.ap())
        nc.sync.dma_start(out=ident, in_=id_d.ap())
        nc.sync.dma_start(out=trineg, in_=tr_d.ap())
        if n_masks:
            msk_sb = const.tile([ST, n_masks * TB], BF16, name="msk_sb")
            for i in range(n_masks):
                nc.sync.dma_start(
                    out=msk_sb[:, i * TB : (i + 1) * TB],
                    in_=mk_d.ap()[i * ST : (i + 1) * ST, :],
                )
        wo_sb = [wop.tile([128, C], BF16, name=f"wo{h}") for h in range(G)]
        for h in range(G):
            nc.sync.dma_start(out=wo_sb[h], in_=wo_d.ap()[h * 128 : (h + 1) * 128, :])

        # per-t-block tiles (not one [128, T] tile) so stage-2 readers only
        # depend on the t-blocks they actually use — tile-granular dependency
        # tracking would otherwise serialize stage 2 behind ALL RoPE work
        qT = [
            [qkv.tile([128, TB], BF16, name=f"qT{h}_{tb}") for tb in range(NTB)]
            for h in range(G)
        ]
        kT = [qkv.tile([128, TB], BF16, name=f"kT{tb}") for tb in range(NTB)]
        vT = [qkv.tile([128, TB], BF16, name=f"vT{tb}") for tb in range(NTB)]
        vch = [qkv.tile([128, 128], BF16, name=f"v{s}") for s in range(NST)]

        # single PSUM pool: 8 x [128, 512] f32 bank tags, reused across stages
        ps = tc.alloc_tile_pool(name="ps", bufs=1, space="PSUM")

        def bank(tag):
            return ps.tile([128, TB], F32, name=tag, tag=tag)

        rpool = tc.alloc_tile_pool(name="rpool", bufs=3)

        # ---- stage 1: projections + RoPE + v transpose, pipelined per tb ----
        for tb in range(NTB):
            tsl = slice(tb * TB, (tb + 1) * TB)
            q_ps = [bank(f"bk{h}") for h in range(G)]
            k_ps = bank("bk4")
            v_ps = bank("bk5")
            for ci in range(NCT):
                first, last = ci == 0, ci == NCT - 1
                rhs = (
                    xt0[ci]
                    if tb == 0
                    else xtr[ci][:, (tb - 1) * TB : tb * TB]
                )
                w = wqkv_sb[ci]
                # v first, then k: their consumers (transpose chain, scores)
                # unblock before the q projections finish
                nc.tensor.matmul(
                    v_ps, lhsT=w[:, 5 * 128 : 6 * 128], rhs=rhs, start=first, stop=last
                )
                nc.tensor.matmul(
                    k_ps, lhsT=w[:, 4 * 128 : 5 * 128], rhs=rhs, start=first, stop=last
                )
                for h in range(G):
                    nc.tensor.matmul(
                        q_ps[h],
                        lhsT=w[:, h * 128 : (h + 1) * 128],
                        rhs=rhs,
                        start=first,
                        stop=last,
                    )

            # v first so the transposes don't queue behind the RoPE backlog,
            # then k (attention waits on it), then qs
            nc.vector.tensor_copy(vT[tb], v_ps)
            for r in range(4):
                s = 4 * tb + r
                vtp = bank(f"bk{6 + (r % 2)}").bitcast(BF16)[:, :128]
                nc.tensor.transpose(vtp, vT[tb][:, r * 128 : (r + 1) * 128], ident)
                nc.vector.tensor_copy(vch[s], vtp)
            for dst, src_ps in [(kT[tb], k_ps)] + [(qT[h][tb], q_ps[h]) for h in range(G)]:
                nc.vector.tensor_copy(dst, src_ps)
                swp = rpool.tile([128, TB], BF16, name="swp", tag="swp")
                tmp = rpool.tile([128, TB], BF16, name="tmp", tag="tmp")
                nc.vector.stream_shuffle(swp, dst, SWAP_MASK)
                nc.vector.tensor_mul(tmp, dst, ctab[:, tsl])
                nc.vector.tensor_mul(swp, swp, stab[:, tsl])
                nc.vector.tensor_add(dst, tmp, swp)

        # ---- stages 2+3: one global software pipeline over (tb, head, s-tile)
        # so head/t-block boundaries never drain the PE. Scores run LA s-tiles
        # ahead of the dependent den/PV matmuls (exp latency hidden). ----
        p2sb = tc.alloc_tile_pool(name="p2sb", bufs=4)
        p3sb = tc.alloc_tile_pool(name="p3sb", bufs=3)
        outp = tc.alloc_tile_pool(name="outp", bufs=10)
        state = {"score": 0, "head": 0, "ncopy": 0}
        oT_live: dict = {}

        def emit_oproj(tb):
            oT_sbs = oT_live.pop(tb)
            for tch in range(TB // 128):
                for cb in range(C // 512):
                    ops = bank(f"bk{2 + (cb % 2)}")
                    for h in range(G):
                        nc.tensor.matmul(
                            ops,
                            lhsT=oT_sbs[h][:, tch * 128 : (tch + 1) * 128],
                            rhs=wo_sb[h][:, cb * 512 : (cb + 1) * 512],
                            start=h == 0,
                            stop=h == G - 1,
                        )
                    osb = p3sb.tile([128, 512], F32, name="osb", tag="osb")
                    # keep these copies off DVE so rcp (den WAR chain) and
                    # the rescale muls never queue behind them
                    nc.scalar.copy(osb, ops)
                    state["ncopy"] += 1
                    t0 = tb * TB + tch * 128
                    nc.sync.dma_start(
                        out=out_d.ap()[t0 : t0 + 128, cb * 512 : (cb + 1) * 512],
                        in_=osb,
                    )

        items = []  # (tb, h, idx)
        for tb in range(NTB):
            for h in range(G):
                for idx in range(len(plan[tb])):
                    items.append((tb, h, idx))

        ctx: dict = {}  # (tb,h) -> dict with oT_ps, den, eps

        def emit_score(it):
            tb, h, idx = it
            entries = plan[tb]
            s, w0, kind, mid = entries[idx]
            # bank roles chosen so stage-2 tiles reuse the PSUM banks that
            # stage-1's trailing (tb=3) DVE stream releases earliest:
            # v (bk5) and vtp (bk6/7) first -> stp; k (bk4) -> den;
            # q0/q1 (bk0/1) -> oT; q2/q3 (bk2/3) -> o_proj accumulators
            if idx == 0:
                ctx[(tb, h)] = {
                    "oT": bank(f"bk{0 + (state['head'] % 2)}"),
                    "den": bank("bk4"),
                    "eps": {},
                }
                state["head"] += 1
            stp = bank(f"bk{5 + (state['score'] % 3)}")
            state["score"] += 1
            diag = kind == DIAG
            nc.tensor.matmul(
                stp[:, w0:],
                lhsT=kT[s // 4][:, (s % 4) * 128 : (s % 4 + 1) * 128],
                rhs=qT[h][tb][:, w0:],
                start=True,
                stop=not diag,
                skip_group_check=diag,
            )
            if diag:
                # additive -512*(s>t) triangular mask folded into the score
                # accumulation on the PE (keeps DVE off the critical path);
                # exp then underflows to ~e-18 which is negligible in den/PV
                nc.tensor.matmul(
                    stp[:, w0 : w0 + ST],
                    lhsT=ident,
                    rhs=trineg,
                    start=False,
                    stop=True,
                    skip_group_check=True,
                )
            ep = p2sb.tile([ST, TB], BF16, name="ep", tag="ep")
            nc.scalar.activation(ep[:, w0:], stp[:, w0:], Exp, scale=SCALE)
            if kind == GEN:
                nc.vector.tensor_mul(
                    ep[:, w0:],
                    ep[:, w0:],
                    msk_sb[:, mid * TB : mid * TB + TB - w0],
                )
            ctx[(tb, h)]["eps"][idx] = ep

        def emit_acc(it):
            tb, h, idx = it
            entries = plan[tb]
            s, w0, kind, mid = entries[idx]
            c = ctx[(tb, h)]
            ep = c["eps"].pop(idx)
            first, last = idx == 0, idx == len(entries) - 1
            nc.tensor.matmul(
                c["den"][:, w0:],
                lhsT=ones_sb,
                rhs=ep[:, w0:],
                start=first,
                stop=last,
                skip_group_check=True,
            )
            nc.tensor.matmul(
                c["oT"][:, w0:],
                lhsT=vch[s],
                rhs=ep[:, w0:],
                start=first,
                stop=last,
                skip_group_check=True,
            )
            if last:
                # free both PSUM banks via ScalarE copies (short queue) so the
                # PE's WAR on them never waits behind the DVE backlog; the
                # reciprocal + rescale then run on SBUF off the critical path
                den_sb = p2sb.tile([128, TB], F32, name="den_sb", tag="den_sb")
                nc.scalar.copy(den_sb, c["den"])
                oT_f = p2sb.tile([128, TB], F32, name="oT_f", tag="oT_f")
                nc.scalar.copy(oT_f, c["oT"])
                rcp = p2sb.tile([128, TB], F32, name="rcp", tag="rcp")
                # ~51-ULP approx is ample for the softmax denominator
                nc.vector.reciprocal_approx_fast(rcp, den_sb)
                oT_sb = outp.tile([128, TB], BF16, name="oT", tag="oT")
                nc.vector.tensor_mul(oT_sb, oT_f, rcp)
                oT_live.setdefault(tb, []).append(oT_sb)
                del ctx[(tb, h)]
                # o_proj for t-block tb is emitted one head LATER (during
                # (tb+1, h0)'s attention) so its lhsT never waits on the
                # rescale chain of tb's last head
                if h == 0 and tb > 0:
                    emit_oproj(tb - 1)
                if tb == NTB - 1 and h == G - 1:
                    emit_oproj(tb)

        from collections import deque

        pend = deque()
        for it in items:
            emit_score(it)
            pend.append(it)
            if len(pend) > LA:
                emit_acc(pend.popleft())
        while pend:
            emit_acc(pend.popleft())

        outp.release()
        p3sb.release()
        p2sb.release()
        rpool.release()
        ps.release()
        xp.release()
        qkv.release()
        wop.release()
        const.release()

    nc.compile()
    return nc


def _prep_inputs(x, cos, sin, Wq, Wk, Wv, Wo, mask_tiles, n_masks):
    cos = np.asarray(cos, dtype=np.float32).reshape(T, HEAD_DIM // 2)
    sin = np.asarray(sin, dtype=np.float32).reshape(T, HEAD_DIM // 2)
    ctab = np.ascontiguousarray(np.repeat(cos, 2, axis=1).T).astype(BF)  # [128, T]
    s2 = np.repeat(sin, 2, axis=1)
    s2[:, 0::2] *= -1.0
    stab = np.ascontiguousarray(s2.T).astype(BF)
    trineg = (-512.0 * (np.arange(ST)[:, None] > np.arange(ST)[None, :])).astype(BF)

    xTb = [
        np.ascontiguousarray(np.asarray(x[b], dtype=np.float32).T).astype(BF)
        for b in range(B)
    ]
    in_maps = []
    for core in range(8):
        b, g = divmod(core, NUM_KV_HEADS)
        wqkv = np.concatenate(
            [
                Wq[:, g * 512 : (g + 1) * 512],
                Wk[:, g * 128 : (g + 1) * 128],
                Wv[:, g * 128 : (g + 1) * 128],
            ],
            axis=1,
        )
        m = {
            "xT": xTb[b],
            "wqkv": np.ascontiguousarray(wqkv).astype(BF),
            "wo": np.ascontiguousarray(Wo[g * 512 : (g + 1) * 512, :]).astype(BF),
            "ctab": ctab,
            "stab": stab,
            "ones": np.ones((128, 128), dtype=BF),
            "ident": np.eye(128, dtype=BF),
            "tri": trineg,
        }
        if n_masks:
            m["masks"] = mask_tiles.reshape(n_masks * ST, TB).astype(BF)
        in_maps.append(m)
    return in_maps


def kernel(x, cos, sin, mask, Wq, Wk, Wv, Wo, _trace=False, _result_box=None):
    from concourse.bass_utils import run_bass_kernel_spmd

    mask2d = np.asarray(mask).reshape(T, T).astype(bool)
    plan, mask_tiles = _classify_mask(mask2d)
    n_masks = int(mask_tiles.shape[0])

    key = (plan, n_masks)
    nc = _nc_cache.get(key)
    if nc is None:
        nc = _build(plan, n_masks)
        _nc_cache[key] = nc

    in_maps = _prep_inputs(x, cos, sin, Wq, Wk, Wv, Wo, mask_tiles, n_masks)
    res = run_bass_kernel_spmd(nc, in_maps, core_ids=list(range(8)), trace=_trace)
    if _result_box is not None:
        _result_box.append(res)

    out = np.zeros((B, T, C), dtype=np.float32)
    for core in range(8):
        b = core // NUM_KV_HEADS
        out[b] += res.results[core]["out"]
    return out


# revision 30
# speedup vs baseline: 2.0969x; 1.0432x over previous
"""Grouped-Query Attention (B=2, T=2048, C=2048, 16 Q heads / 4 KV heads,
D=128) on 8 Trainium2 NeuronCores.

Sharding: core (b, g) for b in {0,1}, g in {0..3} handles batch b and KV head
g (= query heads 4g..4g+3). Each core computes its 4 heads' attention plus the
partial output projection against its 512-row slice of Wo; the host sums the
4 partials per batch (the "all-reduce" of the o_proj, done in numpy).

All matmul operands are bf16 (host-cast); PSUM accumulation stays fp32, so
the only precision loss is input rounding (~4e-3 rel err vs the 2e-2 gate).

Layout/scheduling notes (from trace analysis):
- One PSUM pool with 8 [128,512]-f32 bank tags reused across stages (no
  mid-kernel pool releases -> no cross-stage drain bubbles; the PE pstate
  ramp resets on idle gaps, so a dense PE queue is worth ~1.5x clock).
- Startup DMAs interleaved per contraction chunk (wq/wk/wv/x) so the first
  projection matmul unblocks after ~4 transfers instead of all weights.
- Softmax denominator is computed REPLICATED across all 128 partitions
  (lhsT = all-ones [128,128]) so the reciprocal runs as a full-width DVE op
  (~0.65us) instead of a 1-partition op (3.3us) + GpSimd partition
  broadcast; the per-head tail stall on the PE disappears.
- Stage-2 software pipeline: score matmuls run 2 s-tiles ahead of the
  dependent den/PV matmuls so the PE never waits on ScalarE's exp.
- Diagonal (causal-boundary) s-tiles only compute the t-window right of the
  diagonal plus one shared [128,128] triangular 0/1 multiply.
"""
import sys

sys.path.insert(0, "/opt/trn_rl_repo")

import numpy as np
import ml_dtypes

B, T, C = 2, 2048, 2048
NUM_HEADS, NUM_KV_HEADS, HEAD_DIM = 16, 4, 128
G = NUM_HEADS // NUM_KV_HEADS  # 4 query heads per core
SCALE = float(HEAD_DIM) ** -0.5
TB = 512  # t-block (matmul moving free dim)
NTB = T // TB  # 4
ST = 128  # s-tile
NST = T // ST  # 16
NCT = C // 128  # 16 contraction tiles
LA = 2  # stage-2 score-matmul lookahead (s-tiles in flight past exp)

SWAP_MASK = [i ^ 1 for i in range(32)]
BF = ml_dtypes.bfloat16

_nc_cache: dict = {}

# plan entry kinds
FULL, DIAG, GEN = 0, 1, 2


def _classify_mask(mask2d: np.ndarray):
    """mask2d[t, s] bool. Returns (plan, mask_tiles).

    plan[tb] = tuple of (s_tile_idx, w0, kind, mask_id). w0 is the t-window
    start within the t-block (columns < w0 are entirely masked for this
    s-tile). kind: FULL (no mask work), DIAG (shared lower-triangular 0/1
    multiply on the first 128 window columns), GEN (per-tile 0/1 multiply
    over the whole window; mask_id indexes mask_tiles)."""
    tri = (np.arange(ST)[:, None] <= np.arange(ST)[None, :])
    plan = []
    uniq: dict = {}
    tiles = []
    for tb in range(NTB):
        sub_t = mask2d[tb * TB : (tb + 1) * TB]  # [TB(t), T(s)]
        entries = []
        for s in range(NST):
            sub = sub_t[:, s * ST : (s + 1) * ST]  # [TB(t), ST(s)]
            if sub.all():
                entries.append((s, 0, FULL, None))
                continue
            if not sub.any():
                continue
            m = sub.T  # [s, t]
            w0 = 0
            while w0 + ST <= TB and not m[:, w0 : w0 + ST].any():
                w0 += ST
            win = m[:, w0:]
            if (
                win.shape[1] >= ST
                and (win[:, :ST] == tri).all()
                and win[:, ST:].all()
            ):
                entries.append((s, w0, DIAG, None))
                continue
            tile_m = np.zeros((ST, TB), dtype=np.float32)
            tile_m[:, : TB - w0] = win.astype(np.float32)
            key = (w0, tile_m.tobytes())
            mid = uniq.get(key)
            if mid is None:
                mid = len(tiles)
                uniq[key] = mid
                tiles.append(tile_m)
            entries.append((s, w0, GEN, mid))
        plan.append(tuple(entries))
    mask_tiles = (
        np.stack(tiles) if tiles else np.zeros((0, ST, TB), dtype=np.float32)
    )
    return tuple(plan), mask_tiles


def _build(plan, n_masks):
    import concourse.bacc as bacc
    import concourse.mybir as mybir
    import concourse.tile as tile

    F32 = mybir.dt.float32
    BF16 = mybir.dt.bfloat16
    Exp = mybir.ActivationFunctionType.Exp

    nc = bacc.Bacc()

    xT_d = nc.declare_dram_parameter("xT", [C, T], BF16, isOutput=False)
    # wqkv = [Wq | Wk | Wv] columns, one DMA per 128-row chunk
    wqkv_d = nc.declare_dram_parameter(
        "wqkv", [C, (G + 2) * HEAD_DIM], BF16, isOutput=False
    )
    wo_d = nc.declare_dram_parameter("wo", [G * HEAD_DIM, C], BF16, isOutput=False)
    on_d = nc.declare_dram_parameter("ones", [128, 128], BF16, isOutput=False)
    id_d = nc.declare_dram_parameter("ident", [128, 128], BF16, isOutput=False)
    tr_d = nc.declare_dram_parameter("tri", [ST, ST], BF16, isOutput=False)
    ct_d = nc.declare_dram_parameter("ctab", [HEAD_DIM, T], BF16, isOutput=False)
    st_d = nc.declare_dram_parameter("stab", [HEAD_DIM, T], BF16, isOutput=False)
    if n_masks:
        mk_d = nc.declare_dram_parameter(
            "masks", [n_masks * ST, TB], BF16, isOutput=False
        )
    out_d = nc.declare_dram_parameter("out", [T, C], BF16, isOutput=True)

    with tile.TileContext(nc) as tc:
        const = tc.alloc_tile_pool(name="const", bufs=1)
        wop = tc.alloc_tile_pool(name="wop", bufs=1)
        qkv = tc.alloc_tile_pool(name="qkv", bufs=1)
        xp = tc.alloc_tile_pool(name="xp", bufs=1)

        # --- interleaved startup DMAs: per-chunk weights + x so the first
        # projection matmuls unblock after a handful of transfers ---
        wqkv_sb = [
            wop.tile([128, (G + 2) * HEAD_DIM], BF16, name=f"wqkv{i}")
            for i in range(NCT)
        ]
        # x split into tb0-slice + rest tiles (separate tiles, deps are
        # tile-granular) so tb0's projection pass only waits on 4.7MB
        # (weights + tb0 x slices), not the full 11MB
        xt0 = [xp.tile([128, TB], BF16, name=f"xt0_{i}") for i in range(NCT)]
        xtr = [xp.tile([128, T - TB], BF16, name=f"xtr{i}") for i in range(NCT)]
        for i in range(NCT):
            sl = slice(i * 128, (i + 1) * 128)
            nc.sync.dma_start(out=wqkv_sb[i], in_=wqkv_d.ap()[sl, :])
            nc.sync.dma_start(out=xt0[i], in_=xT_d.ap()[sl, :TB])
        for i in range(NCT):
            sl = slice(i * 128, (i + 1) * 128)
            nc.sync.dma_start(out=xtr[i], in_=xT_d.ap()[sl, TB:])

        ctab = const.tile([HEAD_DIM, T], BF16, name="ctab")
        stab = const.tile([HEAD_DIM, T], BF16, name="stab")
        nc.sync.dma_start(out=ctab, in_=ct_d.ap())
        nc.sync.dma_start(out=stab, in_=st_d.ap())
        ones_sb = const.tile([128, 128], BF16, name="ones_sb")
        ident = const.tile([128, 128], BF16, name="ident")
        trineg = const.tile([ST, ST], BF16, name="trineg")
        nc.sync.dma_start(out=ones_sb, in_=on_d.ap())
        nc.sync.dma_start(out=ident, in_=id_d.ap())
        nc.sync.dma_start(out=trineg, in_=tr_d.ap())
        if n_masks:
            msk_sb = const.tile([ST, n_masks * TB], BF16, name="msk_sb")
            for i in range(n_masks):
                nc.sync.dma_start(
                    out=msk_sb[:, i * TB : (i + 1) * TB],
                    in_=mk_d.ap()[i * ST : (i + 1) * ST, :],
                )
        wo_sb = [wop.tile([128, C], BF16, name=f"wo{h}") for h in range(G)]
        for h in range(G):
            nc.sync.dma_start(out=wo_sb[h], in_=wo_d.ap()[h * 128 : (h + 1) * 128, :])

        # per-t-block tiles (not one [128, T] tile) so stage-2 readers only
        # depend on the t-blocks they actually use — tile-granular dependency
        # tracking would otherwise serialize stage 2 behind ALL RoPE work
        qT = [
            [qkv.tile([128, TB], BF16, name=f"qT{h}_{tb}") for tb in range(NTB)]
            for h in range(G)
        ]
        kT = [qkv.tile([128, TB], BF16, name=f"kT{tb}") for tb in range(NTB)]
        vT = [qkv.tile([128, TB], BF16, name=f"vT{tb}") for tb in range(NTB)]
        vch = [qkv.tile([128, 128], BF16, name=f"v{s}") for s in range(NST)]

        # single PSUM pool: 8 x [128, 512] f32 bank tags, reused across stages
        ps = tc.alloc_tile_pool(name="ps", bufs=1, space="PSUM")

        def bank(tag):
            return ps.tile([128, TB], F32, name=tag, tag=tag)

        rpool = tc.alloc_tile_pool(name="rpool", bufs=3)

        # ---- stage 1: projections + RoPE + v transpose, pipelined per tb ----
        def emit_v_post(tb, v_ps):
            nc.vector.tensor_copy(vT[tb], v_ps)
            for r in range(4):
                s = 4 * tb + r
                vtp = bank(f"bk{6 + (r % 2)}").bitcast(BF16)[:, :128]
                nc.tensor.transpose(vtp, vT[tb][:, r * 128 : (r + 1) * 128], ident)
                nc.vector.tensor_copy(vch[s], vtp)

        def emit_rope(tb, dst, src_ps):
            tsl = slice(tb * TB, (tb + 1) * TB)
            nc.vector.tensor_copy(dst, src_ps)
            swp = rpool.tile([128, TB], BF16, name="swp", tag="swp")
            tmp = rpool.tile([128, TB], BF16, name="tmp", tag="tmp")
            nc.vector.stream_shuffle(swp, dst, SWAP_MASK)
            nc.vector.tensor_mul(tmp, dst, ctab[:, tsl])
            nc.vector.tensor_mul(swp, swp, stab[:, tsl])
            nc.vector.tensor_add(dst, tmp, swp)

        for tb in range(NTB):
            q_ps = [bank(f"bk{h}") for h in range(G)]
            k_ps = bank("bk4")
            v_ps = bank("bk5")

            def rhs_for(ci):
                return (
                    xt0[ci] if tb == 0 else xtr[ci][:, (tb - 1) * TB : tb * TB]
                )

            if tb == 0:
                # ci-major: tb0 is paced by the input DMA, so touch each
                # freshly-arrived chunk with all 6 matmuls at once
                for ci in range(NCT):
                    first, last = ci == 0, ci == NCT - 1
                    rhs = rhs_for(ci)
                    w = wqkv_sb[ci]
                    nc.tensor.matmul(
                        v_ps, lhsT=w[:, 640:768], rhs=rhs, start=first, stop=last
                    )
                    nc.tensor.matmul(
                        k_ps, lhsT=w[:, 512:640], rhs=rhs, start=first, stop=last
                    )
                    for h in range(G):
                        nc.tensor.matmul(
                            q_ps[h],
                            lhsT=w[:, h * 128 : (h + 1) * 128],
                            rhs=rhs,
                            start=first,
                            stop=last,
                        )
                emit_v_post(tb, v_ps)
                emit_rope(tb, kT[tb], k_ps)
                for h in range(G):
                    emit_rope(tb, qT[h][tb], q_ps[h])
            else:
                # per-tensor passes with the DVE consumer emitted right after
                # each pass: RoPE/transposes overlap THIS t-block's remaining
                # projections instead of piling up at the t-block boundary
                passes = [
                    (v_ps, slice(640, 768), lambda: emit_v_post(tb, v_ps)),
                    (k_ps, slice(512, 640), lambda: emit_rope(tb, kT[tb], k_ps)),
                ] + [
                    (
                        q_ps[h],
                        slice(h * 128, (h + 1) * 128),
                        (lambda h=h: emit_rope(tb, qT[h][tb], q_ps[h])),
                    )
                    for h in range(G)
                ]
                for ps_bank, wsl, post in passes:
                    for ci in range(NCT):
                        nc.tensor.matmul(
                            ps_bank,
                            lhsT=wqkv_sb[ci][:, wsl],
                            rhs=rhs_for(ci),
                            start=ci == 0,
                            stop=ci == NCT - 1,
                        )
                    post()

        # ---- stages 2+3: one global software pipeline over (tb, head, s-tile)
        # so head/t-block boundaries never drain the PE. Scores run LA s-tiles
        # ahead of the dependent den/PV matmuls (exp latency hidden). ----
        p2sb = tc.alloc_tile_pool(name="p2sb", bufs=4)
        p3sb = tc.alloc_tile_pool(name="p3sb", bufs=3)
        outp = tc.alloc_tile_pool(name="outp", bufs=10)
        state = {"score": 0, "head": 0, "ncopy": 0}
        oT_live: dict = {}

        def emit_oproj(tb):
            oT_sbs = oT_live.pop(tb)
            for tch in range(TB // 128):
                for cb in range(C // 512):
                    ops = bank(f"bk{2 + (cb % 2)}")
                    for h in range(G):
                        nc.tensor.matmul(
                            ops,
                            lhsT=oT_sbs[h][:, tch * 128 : (tch + 1) * 128],
                            rhs=wo_sb[h][:, cb * 512 : (cb + 1) * 512],
                            start=h == 0,
                            stop=h == G - 1,
                        )
                    osb = p3sb.tile([128, 512], BF16, name="osb", tag="osb")
                    # DVE is light in stage 2+3; ScalarE stays dedicated to
                    # exp + the PSUM-freeing den/oT copies
                    nc.vector.tensor_copy(osb, ops)
                    state["ncopy"] += 1
                    t0 = tb * TB + tch * 128
                    nc.sync.dma_start(
                        out=out_d.ap()[t0 : t0 + 128, cb * 512 : (cb + 1) * 512],
                        in_=osb,
                    )

        items = []  # (tb, h, idx)
        for tb in range(NTB):
            for h in range(G):
                for idx in range(len(plan[tb])):
                    items.append((tb, h, idx))

        ctx: dict = {}  # (tb,h) -> dict with oT_ps, den, eps

        def emit_score(it):
            tb, h, idx = it
            entries = plan[tb]
            s, w0, kind, mid = entries[idx]
            # bank roles chosen so stage-2 tiles reuse the PSUM banks that
            # stage-1's trailing (tb=3) DVE stream releases earliest:
            # v (bk5) and vtp (bk6/7) first -> stp; k (bk4) -> den;
            # q0/q1 (bk0/1) -> oT; q2/q3 (bk2/3) -> o_proj accumulators
            if idx == 0:
                ctx[(tb, h)] = {
                    "oT": bank(f"bk{0 + (state['head'] % 2)}"),
                    "den": bank("bk4"),
                    "eps": {},
                }
                state["head"] += 1
            stp = bank(f"bk{5 + (state['score'] % 3)}")
            state["score"] += 1
            diag = kind == DIAG
            nc.tensor.matmul(
                stp[:, w0:],
                lhsT=kT[s // 4][:, (s % 4) * 128 : (s % 4 + 1) * 128],
                rhs=qT[h][tb][:, w0:],
                start=True,
                stop=not diag,
                skip_group_check=diag,
            )
            if diag:
                # additive -512*(s>t) triangular mask folded into the score
                # accumulation on the PE (keeps DVE off the critical path);
                # exp then underflows to ~e-18 which is negligible in den/PV
                nc.tensor.matmul(
                    stp[:, w0 : w0 + ST],
                    lhsT=ident,
                    rhs=trineg,
                    start=False,
                    stop=True,
                    skip_group_check=True,
                )
            ep = p2sb.tile([ST, TB], BF16, name="ep", tag="ep")
            nc.scalar.activation(ep[:, w0:], stp[:, w0:], Exp, scale=SCALE)
            if kind == GEN:
                nc.vector.tensor_mul(
                    ep[:, w0:],
                    ep[:, w0:],
                    msk_sb[:, mid * TB : mid * TB + TB - w0],
                )
            ctx[(tb, h)]["eps"][idx] = ep

        def emit_acc(it):
            tb, h, idx = it
            entries = plan[tb]
            s, w0, kind, mid = entries[idx]
            c = ctx[(tb, h)]
            ep = c["eps"].pop(idx)
            first, last = idx == 0, idx == len(entries) - 1
            nc.tensor.matmul(
                c["den"][:, w0:],
                lhsT=ones_sb,
                rhs=ep[:, w0:],
                start=first,
                stop=last,
                skip_group_check=True,
            )
            nc.tensor.matmul(
                c["oT"][:, w0:],
                lhsT=vch[s],
                rhs=ep[:, w0:],
                start=first,
                stop=last,
                skip_group_check=True,
            )
            if last:
                # free both PSUM banks via ScalarE copies (short queue) so the
                # PE's WAR on them never waits behind the DVE backlog; the
                # reciprocal + rescale then run on SBUF off the critical path
                den_sb = p2sb.tile([128, TB], F32, name="den_sb", tag="den_sb")
                nc.scalar.copy(den_sb, c["den"])
                oT_f = p2sb.tile([128, TB], F32, name="oT_f", tag="oT_f")
                nc.scalar.copy(oT_f, c["oT"])
                rcp = p2sb.tile([128, TB], F32, name="rcp", tag="rcp")
                # ~51-ULP approx is ample for the softmax denominator
                nc.vector.reciprocal_approx_fast(rcp, den_sb)
                oT_sb = outp.tile([128, TB], BF16, name="oT", tag="oT")
                nc.vector.tensor_mul(oT_sb, oT_f, rcp)
                oT_live.setdefault(tb, []).append(oT_sb)
                del ctx[(tb, h)]
                # o_proj for t-block tb is emitted one head LATER (during
                # (tb+1, h0)'s attention) so its lhsT never waits on the
                # rescale chain of tb's last head
                if h == 0 and tb > 0:
                    emit_oproj(tb - 1)
                if tb == NTB - 1 and h == G - 1:
                    emit_oproj(tb)

        from collections import deque

        pend = deque()
        for it in items:
            emit_score(it)
            pend.append(it)
            if len(pend) > LA:
                emit_acc(pend.popleft())
        while pend:
            emit_acc(pend.popleft())

        outp.release()
        p3sb.release()
        p2sb.release()
        rpool.release()
        ps.release()
        xp.release()
        qkv.release()
        wop.release()
        const.release()

    nc.compile()
    return nc


def _prep_inputs(x, cos, sin, Wq, Wk, Wv, Wo, mask_tiles, n_masks):
    cos = np.asarray(cos, dtype=np.float32).reshape(T, HEAD_DIM // 2)
    sin = np.asarray(sin, dtype=np.float32).reshape(T, HEAD_DIM // 2)
    ctab = np.ascontiguousarray(np.repeat(cos, 2, axis=1).T).astype(BF)  # [128, T]
    s2 = np.repeat(sin, 2, axis=1)
    s2[:, 0::2] *= -1.0
    stab = np.ascontiguousarray(s2.T).astype(BF)
    trineg = (-512.0 * (np.arange(ST)[:, None] > np.arange(ST)[None, :])).astype(BF)

    xTb = [
        np.ascontiguousarray(np.asarray(x[b], dtype=np.float32).T).astype(BF)
        for b in range(B)
    ]
    in_maps = []
    for core in range(8):
        b, g = divmod(core, NUM_KV_HEADS)
        wqkv = np.concatenate(
            [
                Wq[:, g * 512 : (g + 1) * 512],
                Wk[:, g * 128 : (g + 1) * 128],
                Wv[:, g * 128 : (g + 1) * 128],
            ],
            axis=1,
        )
        m = {
            "xT": xTb[b],
            "wqkv": np.ascontiguousarray(wqkv).astype(BF),
            "wo": np.ascontiguousarray(Wo[g * 512 : (g + 1) * 512, :]).astype(BF),
            "ctab": ctab,
            "stab": stab,
            "ones": np.ones((128, 128), dtype=BF),
            "ident": np.eye(128, dtype=BF),
            "tri": trineg,
        }
        if n_masks:
            m["masks"] = mask_tiles.reshape(n_masks * ST, TB).astype(BF)
        in_maps.append(m)
    return in_maps


def kernel(x, cos, sin, mask, Wq, Wk, Wv, Wo, _trace=False, _result_box=None):
    from concourse.bass_utils import run_bass_kernel_spmd

    mask2d = np.asarray(mask).reshape(T, T).astype(bool)
    plan, mask_tiles = _classify_mask(mask2d)
    n_masks = int(mask_tiles.shape[0])

    key = (plan, n_masks)
    nc = _nc_cache.get(key)
    if nc is None:
        nc = _build(plan, n_masks)
        _nc_cache[key] = nc

    in_maps = _prep_inputs(x, cos, sin, Wq, Wk, Wv, Wo, mask_tiles, n_masks)
    res = run_bass_kernel_spmd(nc, in_maps, core_ids=list(range(8)), trace=_trace)
    if _result_box is not None:
        _result_box.append(res)

    out = np.zeros((B, T, C), dtype=np.float32)
    for core in range(8):
        b = core // NUM_KV_HEADS
        out[b] += res.results[core]["out"].astype(np.float32)
    return out


# revision 36
# speedup vs baseline: 2.1601x; 1.0302x over previous
"""Grouped-Query Attention (B=2, T=2048, C=2048, 16 Q heads / 4 KV heads,
D=128) on 8 Trainium2 NeuronCores.

Sharding: core (b, g) for b in {0,1}, g in {0..3} handles batch b and KV head
g (= query heads 4g..4g+3). Each core computes its 4 heads' attention plus the
partial output projection against its 512-row slice of Wo; the host sums the
4 partials per batch (the "all-reduce" of the o_proj, done in numpy).

All matmul operands are bf16 (host-cast); PSUM accumulation stays fp32, so
the only precision loss is input rounding (~4e-3 rel err vs the 2e-2 gate).

Layout/scheduling notes (from trace analysis):
- One PSUM pool with 8 [128,512]-f32 bank tags reused across stages (no
  mid-kernel pool releases -> no cross-stage drain bubbles; the PE pstate
  ramp resets on idle gaps, so a dense PE queue is worth ~1.5x clock).
- Startup DMAs interleaved per contraction chunk (wq/wk/wv/x) so the first
  projection matmul unblocks after ~4 transfers instead of all weights.
- Softmax denominator is computed REPLICATED across all 128 partitions
  (lhsT = all-ones [128,128]) so the reciprocal runs as a full-width DVE op
  (~0.65us) instead of a 1-partition op (3.3us) + GpSimd partition
  broadcast; the per-head tail stall on the PE disappears.
- Stage-2 software pipeline: score matmuls run 2 s-tiles ahead of the
  dependent den/PV matmuls so the PE never waits on ScalarE's exp.
- Diagonal (causal-boundary) s-tiles only compute the t-window right of the
  diagonal plus one shared [128,128] triangular 0/1 multiply.
"""
import sys

sys.path.insert(0, "/opt/trn_rl_repo")

import numpy as np
import ml_dtypes

B, T, C = 2, 2048, 2048
NUM_HEADS, NUM_KV_HEADS, HEAD_DIM = 16, 4, 128
G = NUM_HEADS // NUM_KV_HEADS  # 4 query heads per core
SCALE = float(HEAD_DIM) ** -0.5
TB = 512  # t-block (matmul moving free dim)
NTB = T // TB  # 4
ST = 128  # s-tile
NST = T // ST  # 16
NCT = C // 128  # 16 contraction tiles
LA = 3  # stage-2 score-matmul lookahead (s-tiles in flight past exp)

SWAP_MASK = [i ^ 1 for i in range(32)]
BF = ml_dtypes.bfloat16

_nc_cache: dict = {}

# plan entry kinds
FULL, DIAG, GEN = 0, 1, 2


def _classify_mask(mask2d: np.ndarray):
    """mask2d[t, s] bool. Returns (plan, mask_tiles).

    plan[tb] = tuple of (s_tile_idx, w0, kind, mask_id). w0 is the t-window
    start within the t-block (columns < w0 are entirely masked for this
    s-tile). kind: FULL (no mask work), DIAG (shared lower-triangular 0/1
    multiply on the first 128 window columns), GEN (per-tile 0/1 multiply
    over the whole window; mask_id indexes mask_tiles)."""
    tri = (np.arange(ST)[:, None] <= np.arange(ST)[None, :])
    plan = []
    uniq: dict = {}
    tiles = []
    for tb in range(NTB):
        sub_t = mask2d[tb * TB : (tb + 1) * TB]  # [TB(t), T(s)]
        entries = []
        for s in range(NST):
            sub = sub_t[:, s * ST : (s + 1) * ST]  # [TB(t), ST(s)]
            if sub.all():
                entries.append((s, 0, FULL, None))
                continue
            if not sub.any():
                continue
            m = sub.T  # [s, t]
            w0 = 0
            while w0 + ST <= TB and not m[:, w0 : w0 + ST].any():
                w0 += ST
            win = m[:, w0:]
            if (
                win.shape[1] >= ST
                and (win[:, :ST] == tri).all()
                and win[:, ST:].all()
            ):
                entries.append((s, w0, DIAG, None))
                continue
            tile_m = np.zeros((ST, TB), dtype=np.float32)
            tile_m[:, : TB - w0] = win.astype(np.float32)
            key = (w0, tile_m.tobytes())
            mid = uniq.get(key)
            if mid is None:
                mid = len(tiles)
                uniq[key] = mid
                tiles.append(tile_m)
            entries.append((s, w0, GEN, mid))
        plan.append(tuple(entries))
    mask_tiles = (
        np.stack(tiles) if tiles else np.zeros((0, ST, TB), dtype=np.float32)
    )
    return tuple(plan), mask_tiles


def _build(plan, n_masks):
    import concourse.bacc as bacc
    import concourse.mybir as mybir
    import concourse.tile as tile

    F32 = mybir.dt.float32
    BF16 = mybir.dt.bfloat16
    Exp = mybir.ActivationFunctionType.Exp

    nc = bacc.Bacc()

    xT_d = nc.declare_dram_parameter("xT", [C, T], BF16, isOutput=False)
    # wqkv = [Wq | Wk | Wv] columns, one DMA per 128-row chunk
    wqkv_d = nc.declare_dram_parameter(
        "wqkv", [C, (G + 2) * HEAD_DIM], BF16, isOutput=False
    )
    wo_d = nc.declare_dram_parameter("wo", [G * HEAD_DIM, C], BF16, isOutput=False)
    on_d = nc.declare_dram_parameter("ones", [128, 128], BF16, isOutput=False)
    id_d = nc.declare_dram_parameter("ident", [128, 128], BF16, isOutput=False)
    tr_d = nc.declare_dram_parameter("tri", [ST, ST], BF16, isOutput=False)
    ct_d = nc.declare_dram_parameter("ctab", [HEAD_DIM, T], BF16, isOutput=False)
    st_d = nc.declare_dram_parameter("stab", [HEAD_DIM, T], BF16, isOutput=False)
    if n_masks:
        mk_d = nc.declare_dram_parameter(
            "masks", [n_masks * ST, TB], BF16, isOutput=False
        )
    out_d = nc.declare_dram_parameter("out", [T, C], BF16, isOutput=True)

    with tile.TileContext(nc) as tc:
        const = tc.alloc_tile_pool(name="const", bufs=1)
        wop = tc.alloc_tile_pool(name="wop", bufs=1)
        qkv = tc.alloc_tile_pool(name="qkv", bufs=1)
        xp = tc.alloc_tile_pool(name="xp", bufs=1)

        # --- interleaved startup DMAs: per-chunk weights + x so the first
        # projection matmuls unblock after a handful of transfers ---
        wqkv_sb = [
            wop.tile([128, (G + 2) * HEAD_DIM], BF16, name=f"wqkv{i}")
            for i in range(NCT)
        ]
        # x split into tb0-slice + rest tiles (separate tiles, deps are
        # tile-granular) so tb0's projection pass only waits on 4.7MB
        # (weights + tb0 x slices), not the full 11MB
        xt0 = [xp.tile([128, TB], BF16, name=f"xt0_{i}") for i in range(NCT)]
        xtr = [xp.tile([128, T - TB], BF16, name=f"xtr{i}") for i in range(NCT)]
        for i in range(NCT):
            sl = slice(i * 128, (i + 1) * 128)
            nc.sync.dma_start(out=wqkv_sb[i], in_=wqkv_d.ap()[sl, :])
            nc.sync.dma_start(out=xt0[i], in_=xT_d.ap()[sl, :TB])
        for i in range(NCT):
            sl = slice(i * 128, (i + 1) * 128)
            nc.sync.dma_start(out=xtr[i], in_=xT_d.ap()[sl, TB:])

        ctab = const.tile([HEAD_DIM, T], BF16, name="ctab")
        stab = const.tile([HEAD_DIM, T], BF16, name="stab")
        nc.sync.dma_start(out=ctab, in_=ct_d.ap())
        nc.sync.dma_start(out=stab, in_=st_d.ap())
        ones_sb = const.tile([128, 128], BF16, name="ones_sb")
        ident = const.tile([128, 128], BF16, name="ident")
        trineg = const.tile([ST, ST], BF16, name="trineg")
        nc.sync.dma_start(out=ones_sb, in_=on_d.ap())
        nc.sync.dma_start(out=ident, in_=id_d.ap())
        nc.sync.dma_start(out=trineg, in_=tr_d.ap())
        if n_masks:
            msk_sb = const.tile([ST, n_masks * TB], BF16, name="msk_sb")
            for i in range(n_masks):
                nc.sync.dma_start(
                    out=msk_sb[:, i * TB : (i + 1) * TB],
                    in_=mk_d.ap()[i * ST : (i + 1) * ST, :],
                )
        wo_sb = [wop.tile([128, C], BF16, name=f"wo{h}") for h in range(G)]
        for h in range(G):
            nc.sync.dma_start(out=wo_sb[h], in_=wo_d.ap()[h * 128 : (h + 1) * 128, :])

        # per-t-block tiles (not one [128, T] tile) so stage-2 readers only
        # depend on the t-blocks they actually use — tile-granular dependency
        # tracking would otherwise serialize stage 2 behind ALL RoPE work
        qT = [
            [qkv.tile([128, TB], BF16, name=f"qT{h}_{tb}") for tb in range(NTB)]
            for h in range(G)
        ]
        kT = [qkv.tile([128, TB], BF16, name=f"kT{tb}") for tb in range(NTB)]
        vT = [qkv.tile([128, TB], BF16, name=f"vT{tb}") for tb in range(NTB)]
        vch = [qkv.tile([128, 128], BF16, name=f"v{s}") for s in range(NST)]

        # single PSUM pool: 8 x [128, 512] f32 bank tags, reused across stages
        ps = tc.alloc_tile_pool(name="ps", bufs=1, space="PSUM")

        def bank(tag):
            return ps.tile([128, TB], F32, name=tag, tag=tag)

        rpool = tc.alloc_tile_pool(name="rpool", bufs=3)

        # ---- stage 1: projections + RoPE + v transpose, pipelined per tb ----
        def emit_v_post(tb, v_ps):
            nc.vector.tensor_copy(vT[tb], v_ps)
            for r in range(4):
                s = 4 * tb + r
                vtp = bank(f"bk{6 + (r % 2)}").bitcast(BF16)[:, :128]
                nc.tensor.transpose(vtp, vT[tb][:, r * 128 : (r + 1) * 128], ident)
                nc.vector.tensor_copy(vch[s], vtp)

        def emit_rope(tb, dst, src_ps):
            tsl = slice(tb * TB, (tb + 1) * TB)
            nc.vector.tensor_copy(dst, src_ps)
            swp = rpool.tile([128, TB], BF16, name="swp", tag="swp")
            tmp = rpool.tile([128, TB], BF16, name="tmp", tag="tmp")
            nc.vector.stream_shuffle(swp, dst, SWAP_MASK)
            nc.vector.tensor_mul(tmp, dst, ctab[:, tsl])
            nc.vector.tensor_mul(swp, swp, stab[:, tsl])
            nc.vector.tensor_add(dst, tmp, swp)

        for tb in range(NTB):
            q_ps = [bank(f"bk{h}") for h in range(G)]
            k_ps = bank("bk4")
            v_ps = bank("bk5")

            def rhs_for(ci):
                return (
                    xt0[ci] if tb == 0 else xtr[ci][:, (tb - 1) * TB : tb * TB]
                )

            if tb == 0:
                # ci-major: tb0 is paced by the input DMA, so touch each
                # freshly-arrived chunk with all 6 matmuls at once
                for ci in range(NCT):
                    first, last = ci == 0, ci == NCT - 1
                    rhs = rhs_for(ci)
                    w = wqkv_sb[ci]
                    nc.tensor.matmul(
                        v_ps, lhsT=w[:, 640:768], rhs=rhs, start=first, stop=last
                    )
                    nc.tensor.matmul(
                        k_ps, lhsT=w[:, 512:640], rhs=rhs, start=first, stop=last
                    )
                    for h in range(G):
                        nc.tensor.matmul(
                            q_ps[h],
                            lhsT=w[:, h * 128 : (h + 1) * 128],
                            rhs=rhs,
                            start=first,
                            stop=last,
                        )
                emit_v_post(tb, v_ps)
                emit_rope(tb, kT[tb], k_ps)
                for h in range(G):
                    emit_rope(tb, qT[h][tb], q_ps[h])
            else:
                # per-tensor passes with the DVE consumer emitted right after
                # each pass: RoPE/transposes overlap THIS t-block's remaining
                # projections instead of piling up at the t-block boundary
                passes = [
                    (v_ps, slice(640, 768), lambda: emit_v_post(tb, v_ps)),
                    (k_ps, slice(512, 640), lambda: emit_rope(tb, kT[tb], k_ps)),
                ] + [
                    (
                        q_ps[h],
                        slice(h * 128, (h + 1) * 128),
                        (lambda h=h: emit_rope(tb, qT[h][tb], q_ps[h])),
                    )
                    for h in range(G)
                ]
                for ps_bank, wsl, post in passes:
                    for ci in range(NCT):
                        nc.tensor.matmul(
                            ps_bank,
                            lhsT=wqkv_sb[ci][:, wsl],
                            rhs=rhs_for(ci),
                            start=ci == 0,
                            stop=ci == NCT - 1,
                        )
                    post()

        # ---- stages 2+3: one global software pipeline over (tb, head, s-tile)
        # so head/t-block boundaries never drain the PE. Scores run LA s-tiles
        # ahead of the dependent den/PV matmuls (exp latency hidden). ----
        p2sb = tc.alloc_tile_pool(name="p2sb", bufs=6)  # ep tiles (LA+3 live)
        phd = tc.alloc_tile_pool(name="phd", bufs=2)  # per-head den/oT/rcp
        p3sb = tc.alloc_tile_pool(name="p3sb", bufs=3)
        outp = tc.alloc_tile_pool(name="outp", bufs=10)
        state = {"score": 0, "head": 0, "ncopy": 0}
        oT_live: dict = {}

        def emit_oproj(tb):
            oT_sbs = oT_live.pop(tb)
            for tch in range(TB // 128):
                # stage the full [128, C] row block in SBUF and ship it as ONE
                # DMA (4KB contiguous per partition -> fat packets; the tail
                # after the last matmul drains ~4x faster)
                osb = p3sb.tile([128, C], BF16, name="osb", tag="osb")
                for cb in range(C // 512):
                    ops = bank(f"bk{2 + (cb % 2)}")
                    for h in range(G):
                        nc.tensor.matmul(
                            ops,
                            lhsT=oT_sbs[h][:, tch * 128 : (tch + 1) * 128],
                            rhs=wo_sb[h][:, cb * 512 : (cb + 1) * 512],
                            start=h == 0,
                            stop=h == G - 1,
                        )
                    dst = osb[:, cb * 512 : (cb + 1) * 512]
                    # alternate the PSUM->SBUF copies between ScalarE and DVE
                    # so neither queue builds a backlog
                    if state["ncopy"] % 2 == 0:
                        nc.scalar.copy(dst, ops)
                    else:
                        nc.vector.tensor_copy(dst, ops)
                    state["ncopy"] += 1
                t0 = tb * TB + tch * 128
                nc.sync.dma_start(out=out_d.ap()[t0 : t0 + 128, :], in_=osb)

        items = []  # (tb, h, idx)
        for tb in range(NTB):
            for h in range(G):
                for idx in range(len(plan[tb])):
                    items.append((tb, h, idx))

        ctx: dict = {}  # (tb,h) -> dict with oT_ps, den, eps

        def emit_score(it):
            tb, h, idx = it
            entries = plan[tb]
            s, w0, kind, mid = entries[idx]
            # bank roles chosen so stage-2 tiles reuse the PSUM banks that
            # stage-1's trailing (tb=3) DVE stream releases earliest:
            # v (bk5) and vtp (bk6/7) first -> stp; k (bk4) -> den;
            # q0/q1 (bk0/1) -> oT; q2/q3 (bk2/3) -> o_proj accumulators
            if idx == 0:
                ctx[(tb, h)] = {
                    "oT": bank(f"bk{0 + (state['head'] % 2)}"),
                    "den": bank("bk4"),
                    "eps": {},
                }
                state["head"] += 1
            stp = bank(f"bk{5 + (state['score'] % 3)}")
            state["score"] += 1
            diag = kind == DIAG
            nc.tensor.matmul(
                stp[:, w0:],
                lhsT=kT[s // 4][:, (s % 4) * 128 : (s % 4 + 1) * 128],
                rhs=qT[h][tb][:, w0:],
                start=True,
                stop=not diag,
                skip_group_check=diag,
            )
            if diag:
                # additive -512*(s>t) triangular mask folded into the score
                # accumulation on the PE (keeps DVE off the critical path);
                # exp then underflows to ~e-18 which is negligible in den/PV
                nc.tensor.matmul(
                    stp[:, w0 : w0 + ST],
                    lhsT=ident,
                    rhs=trineg,
                    start=False,
                    stop=True,
                    skip_group_check=True,
                )
            ep = p2sb.tile([ST, TB], BF16, name="ep", tag="ep")
            nc.scalar.activation(ep[:, w0:], stp[:, w0:], Exp, scale=SCALE)
            if kind == GEN:
                nc.vector.tensor_mul(
                    ep[:, w0:],
                    ep[:, w0:],
                    msk_sb[:, mid * TB : mid * TB + TB - w0],
                )
            ctx[(tb, h)]["eps"][idx] = ep

        def emit_acc(it):
            tb, h, idx = it
            entries = plan[tb]
            s, w0, kind, mid = entries[idx]
            c = ctx[(tb, h)]
            ep = c["eps"].pop(idx)
            first, last = idx == 0, idx == len(entries) - 1
            nc.tensor.matmul(
                c["den"][:, w0:],
                lhsT=ones_sb,
                rhs=ep[:, w0:],
                start=first,
                stop=last,
                skip_group_check=True,
            )
            nc.tensor.matmul(
                c["oT"][:, w0:],
                lhsT=vch[s],
                rhs=ep[:, w0:],
                start=first,
                stop=last,
                skip_group_check=True,
            )
            if last:
                # free both PSUM banks via ScalarE copies (short queue) so the
                # PE's WAR on them never waits behind the DVE backlog; the
                # reciprocal + rescale then run on SBUF off the critical path
                den_sb = phd.tile([128, TB], F32, name="den_sb", tag="den_sb")
                nc.scalar.copy(den_sb, c["den"])
                oT_f = phd.tile([128, TB], F32, name="oT_f", tag="oT_f")
                nc.scalar.copy(oT_f, c["oT"])
                rcp = phd.tile([128, TB], F32, name="rcp", tag="rcp")
                # ~51-ULP approx is ample for the softmax denominator
                nc.vector.reciprocal_approx_fast(rcp, den_sb)
                oT_sb = outp.tile([128, TB], BF16, name="oT", tag="oT")
                nc.vector.tensor_mul(oT_sb, oT_f, rcp)
                oT_live.setdefault(tb, []).append(oT_sb)
                del ctx[(tb, h)]
                # o_proj for t-block tb is emitted one head LATER (during
                # (tb+1, h0)'s attention) so its lhsT never waits on the
                # rescale chain of tb's last head
                if h == 0 and tb > 0:
                    emit_oproj(tb - 1)
                if tb == NTB - 1 and h == G - 1:
                    emit_oproj(tb)

        from collections import deque

        pend = deque()
        for it in items:
            emit_score(it)
            pend.append(it)
            if len(pend) > LA:
                emit_acc(pend.popleft())
        while pend:
            emit_acc(pend.popleft())

        outp.release()
        p3sb.release()
        phd.release()
        p2sb.release()
        rpool.release()
        ps.release()
        xp.release()
        qkv.release()
        wop.release()
        const.release()

    nc.compile()
    return nc


def _prep_inputs(x, cos, sin, Wq, Wk, Wv, Wo, mask_tiles, n_masks):
    cos = np.asarray(cos, dtype=np.float32).reshape(T, HEAD_DIM // 2)
    sin = np.asarray(sin, dtype=np.float32).reshape(T, HEAD_DIM // 2)
    ctab = np.ascontiguousarray(np.repeat(cos, 2, axis=1).T).astype(BF)  # [128, T]
    s2 = np.repeat(sin, 2, axis=1)
    s2[:, 0::2] *= -1.0
    stab = np.ascontiguousarray(s2.T).astype(BF)
    trineg = (-512.0 * (np.arange(ST)[:, None] > np.arange(ST)[None, :])).astype(BF)

    xTb = [
        np.ascontiguousarray(np.asarray(x[b], dtype=np.float32).T).astype(BF)
        for b in range(B)
    ]
    in_maps = []
    for core in range(8):
        b, g = divmod(core, NUM_KV_HEADS)
        wqkv = np.concatenate(
            [
                Wq[:, g * 512 : (g + 1) * 512],
                Wk[:, g * 128 : (g + 1) * 128],
                Wv[:, g * 128 : (g + 1) * 128],
            ],
            axis=1,
        )
        m = {
            "xT": xTb[b],
            "wqkv": np.ascontiguousarray(wqkv).astype(BF),
            "wo": np.ascontiguousarray(Wo[g * 512 : (g + 1) * 512, :]).astype(BF),
            "ctab": ctab,
            "stab": stab,
            "ones": np.ones((128, 128), dtype=BF),
            "ident": np.eye(128, dtype=BF),
            "tri": trineg,
        }
        if n_masks:
            m["masks"] = mask_tiles.reshape(n_masks * ST, TB).astype(BF)
        in_maps.append(m)
    return in_maps


def kernel(x, cos, sin, mask, Wq, Wk, Wv, Wo, _trace=False, _result_box=None):
    from concourse.bass_utils import run_bass_kernel_spmd

    mask2d = np.asarray(mask).reshape(T, T).astype(bool)
    plan, mask_tiles = _classify_mask(mask2d)
    n_masks = int(mask_tiles.shape[0])

    key = (plan, n_masks)
    nc = _nc_cache.get(key)
    if nc is None:
        nc = _build(plan, n_masks)
        _nc_cache[key] = nc

    in_maps = _prep_inputs(x, cos, sin, Wq, Wk, Wv, Wo, mask_tiles, n_masks)
    res = run_bass_kernel_spmd(nc, in_maps, core_ids=list(range(8)), trace=_trace)
    if _result_box is not None:
        _result_box.append(res)

    out = np.zeros((B, T, C), dtype=np.float32)
    for core in range(8):
        b = core // NUM_KV_HEADS
        out[b] += res.results[core]["out"].astype(np.float32)
    return out


# revision 38
# speedup vs baseline: 2.1613x; 1.0006x over previous
"""Grouped-Query Attention (B=2, T=2048, C=2048, 16 Q heads / 4 KV heads,
D=128) on 8 Trainium2 NeuronCores.

Sharding: core (b, g) for b in {0,1}, g in {0..3} handles batch b and KV head
g (= query heads 4g..4g+3). Each core computes its 4 heads' attention plus the
partial output projection against its 512-row slice of Wo; the host sums the
4 partials per batch (the "all-reduce" of the o_proj, done in numpy).

All matmul operands are bf16 (host-cast); PSUM accumulation stays fp32, so
the only precision loss is input rounding (~4e-3 rel err vs the 2e-2 gate).

Layout/scheduling notes (from trace analysis):
- One PSUM pool with 8 [128,512]-f32 bank tags reused across stages (no
  mid-kernel pool releases -> no cross-stage drain bubbles; the PE pstate
  ramp resets on idle gaps, so a dense PE queue is worth ~1.5x clock).
- Startup DMAs interleaved per contraction chunk (wq/wk/wv/x) so the first
  projection matmul unblocks after ~4 transfers instead of all weights.
- Softmax denominator is computed REPLICATED across all 128 partitions
  (lhsT = all-ones [128,128]) so the reciprocal runs as a full-width DVE op
  (~0.65us) instead of a 1-partition op (3.3us) + GpSimd partition
  broadcast; the per-head tail stall on the PE disappears.
- Stage-2 software pipeline: score matmuls run 2 s-tiles ahead of the
  dependent den/PV matmuls so the PE never waits on ScalarE's exp.
- Diagonal (causal-boundary) s-tiles only compute the t-window right of the
  diagonal plus one shared [128,128] triangular 0/1 multiply.
"""
import sys

sys.path.insert(0, "/opt/trn_rl_repo")

import numpy as np
import ml_dtypes

B, T, C = 2, 2048, 2048
NUM_HEADS, NUM_KV_HEADS, HEAD_DIM = 16, 4, 128
G = NUM_HEADS // NUM_KV_HEADS  # 4 query heads per core
SCALE = float(HEAD_DIM) ** -0.5
TB = 512  # t-block (matmul moving free dim)
NTB = T // TB  # 4
ST = 128  # s-tile
NST = T // ST  # 16
NCT = C // 128  # 16 contraction tiles
LA = 3  # stage-2 score-matmul lookahead (s-tiles in flight past exp)

SWAP_MASK = [i ^ 1 for i in range(32)]
BF = ml_dtypes.bfloat16

_nc_cache: dict = {}

# plan entry kinds
FULL, DIAG, GEN = 0, 1, 2


def _classify_mask(mask2d: np.ndarray):
    """mask2d[t, s] bool. Returns (plan, mask_tiles).

    plan[tb] = tuple of (s_tile_idx, w0, kind, mask_id). w0 is the t-window
    start within the t-block (columns < w0 are entirely masked for this
    s-tile). kind: FULL (no mask work), DIAG (shared lower-triangular 0/1
    multiply on the first 128 window columns), GEN (per-tile 0/1 multiply
    over the whole window; mask_id indexes mask_tiles)."""
    tri = (np.arange(ST)[:, None] <= np.arange(ST)[None, :])
    plan = []
    uniq: dict = {}
    tiles = []
    for tb in range(NTB):
        sub_t = mask2d[tb * TB : (tb + 1) * TB]  # [TB(t), T(s)]
        entries = []
        for s in range(NST):
            sub = sub_t[:, s * ST : (s + 1) * ST]  # [TB(t), ST(s)]
            if sub.all():
                entries.append((s, 0, FULL, None))
                continue
            if not sub.any():
                continue
            m = sub.T  # [s, t]
            w0 = 0
            while w0 + ST <= TB and not m[:, w0 : w0 + ST].any():
                w0 += ST
            win = m[:, w0:]
            if (
                win.shape[1] >= ST
                and (win[:, :ST] == tri).all()
                and win[:, ST:].all()
            ):
                entries.append((s, w0, DIAG, None))
                continue
            tile_m = np.zeros((ST, TB), dtype=np.float32)
            tile_m[:, : TB - w0] = win.astype(np.float32)
            key = (w0, tile_m.tobytes())
            mid = uniq.get(key)
            if mid is None:
                mid = len(tiles)
                uniq[key] = mid
                tiles.append(tile_m)
            entries.append((s, w0, GEN, mid))
        plan.append(tuple(entries))
    mask_tiles = (
        np.stack(tiles) if tiles else np.zeros((0, ST, TB), dtype=np.float32)
    )
    return tuple(plan), mask_tiles


def _build(plan, n_masks):
    import concourse.bacc as bacc
    import concourse.mybir as mybir
    import concourse.tile as tile

    F32 = mybir.dt.float32
    BF16 = mybir.dt.bfloat16
    Exp = mybir.ActivationFunctionType.Exp

    nc = bacc.Bacc()

    xT_d = nc.declare_dram_parameter("xT", [C, T], BF16, isOutput=False)
    # wqkv = [Wq | Wk | Wv] columns, one DMA per 128-row chunk
    wqkv_d = nc.declare_dram_parameter(
        "wqkv", [C, (G + 2) * HEAD_DIM], BF16, isOutput=False
    )
    wo_d = nc.declare_dram_parameter("wo", [G * HEAD_DIM, C], BF16, isOutput=False)
    on_d = nc.declare_dram_parameter("ones", [128, 128], BF16, isOutput=False)
    id_d = nc.declare_dram_parameter("ident", [128, 128], BF16, isOutput=False)
    tr_d = nc.declare_dram_parameter("tri", [ST, ST], BF16, isOutput=False)
    ct_d = nc.declare_dram_parameter("ctab", [HEAD_DIM, T], BF16, isOutput=False)
    st_d = nc.declare_dram_parameter("stab", [HEAD_DIM, T], BF16, isOutput=False)
    if n_masks:
        mk_d = nc.declare_dram_parameter(
            "masks", [n_masks * ST, TB], BF16, isOutput=False
        )
    out_d = nc.declare_dram_parameter("out", [T, C], BF16, isOutput=True)

    with tile.TileContext(nc) as tc:
        const = tc.alloc_tile_pool(name="const", bufs=1)
        wop = tc.alloc_tile_pool(name="wop", bufs=1)
        qkv = tc.alloc_tile_pool(name="qkv", bufs=1)
        xp = tc.alloc_tile_pool(name="xp", bufs=1)

        # --- interleaved startup DMAs: per-chunk weights + x so the first
        # projection matmuls unblock after a handful of transfers ---
        wqkv_sb = [
            wop.tile([128, (G + 2) * HEAD_DIM], BF16, name=f"wqkv{i}")
            for i in range(NCT)
        ]
        # x split into tb0-slice + rest tiles (separate tiles, deps are
        # tile-granular) so tb0's projection pass only waits on 4.7MB
        # (weights + tb0 x slices), not the full 11MB
        xt0 = [xp.tile([128, TB], BF16, name=f"xt0_{i}") for i in range(NCT)]
        xtr = [xp.tile([128, T - TB], BF16, name=f"xtr{i}") for i in range(NCT)]
        for i in range(NCT):
            sl = slice(i * 128, (i + 1) * 128)
            nc.sync.dma_start(out=wqkv_sb[i], in_=wqkv_d.ap()[sl, :])
            nc.sync.dma_start(out=xt0[i], in_=xT_d.ap()[sl, :TB])
        for i in range(NCT):
            sl = slice(i * 128, (i + 1) * 128)
            nc.sync.dma_start(out=xtr[i], in_=xT_d.ap()[sl, TB:])

        ctab = const.tile([HEAD_DIM, T], BF16, name="ctab")
        stab = const.tile([HEAD_DIM, T], BF16, name="stab")
        nc.sync.dma_start(out=ctab, in_=ct_d.ap())
        nc.sync.dma_start(out=stab, in_=st_d.ap())
        ones_sb = const.tile([128, 128], BF16, name="ones_sb")
        ident = const.tile([128, 128], BF16, name="ident")
        trineg = const.tile([ST, ST], BF16, name="trineg")
        nc.sync.dma_start(out=ones_sb, in_=on_d.ap())
        nc.sync.dma_start(out=ident, in_=id_d.ap())
        nc.sync.dma_start(out=trineg, in_=tr_d.ap())
        if n_masks:
            msk_sb = const.tile([ST, n_masks * TB], BF16, name="msk_sb")
            for i in range(n_masks):
                nc.sync.dma_start(
                    out=msk_sb[:, i * TB : (i + 1) * TB],
                    in_=mk_d.ap()[i * ST : (i + 1) * ST, :],
                )
        wo_sb = [wop.tile([128, C], BF16, name=f"wo{h}") for h in range(G)]
        for h in range(G):
            nc.sync.dma_start(out=wo_sb[h], in_=wo_d.ap()[h * 128 : (h + 1) * 128, :])

        # per-t-block tiles (not one [128, T] tile) so stage-2 readers only
        # depend on the t-blocks they actually use — tile-granular dependency
        # tracking would otherwise serialize stage 2 behind ALL RoPE work
        qT = [
            [qkv.tile([128, TB], BF16, name=f"qT{h}_{tb}") for tb in range(NTB)]
            for h in range(G)
        ]
        kT = [qkv.tile([128, TB], BF16, name=f"kT{tb}") for tb in range(NTB)]
        vT = [qkv.tile([128, TB], BF16, name=f"vT{tb}") for tb in range(NTB)]
        vch = [qkv.tile([128, 128], BF16, name=f"v{s}") for s in range(NST)]

        # single PSUM pool: 8 x [128, 512] f32 bank tags, reused across stages
        ps = tc.alloc_tile_pool(name="ps", bufs=1, space="PSUM")

        def bank(tag):
            return ps.tile([128, TB], F32, name=tag, tag=tag)

        rpool = tc.alloc_tile_pool(name="rpool", bufs=3)

        # ---- stage 1: projections + RoPE + v transpose, pipelined per tb ----
        def emit_v_post(tb, v_ps):
            nc.vector.tensor_copy(vT[tb], v_ps)
            for r in range(4):
                s = 4 * tb + r
                vtp = bank(f"bk{6 + (r % 2)}").bitcast(BF16)[:, :128]
                nc.tensor.transpose(vtp, vT[tb][:, r * 128 : (r + 1) * 128], ident)
                nc.vector.tensor_copy(vch[s], vtp)

        def emit_rope(tb, dst, src_ps):
            tsl = slice(tb * TB, (tb + 1) * TB)
            nc.vector.tensor_copy(dst, src_ps)
            swp = rpool.tile([128, TB], BF16, name="swp", tag="swp")
            tmp = rpool.tile([128, TB], BF16, name="tmp", tag="tmp")
            nc.vector.stream_shuffle(swp, dst, SWAP_MASK)
            nc.vector.tensor_mul(tmp, dst, ctab[:, tsl])
            nc.vector.tensor_mul(swp, swp, stab[:, tsl])
            nc.vector.tensor_add(dst, tmp, swp)

        for tb in range(NTB):
            q_ps = [bank(f"bk{h}") for h in range(G)]
            k_ps = bank("bk4")
            v_ps = bank("bk5")

            def rhs_for(ci):
                return (
                    xt0[ci] if tb == 0 else xtr[ci][:, (tb - 1) * TB : tb * TB]
                )

            if tb == 0:
                # ci-major: tb0 is paced by the input DMA, so touch each
                # freshly-arrived chunk with all 6 matmuls at once
                for ci in range(NCT):
                    first, last = ci == 0, ci == NCT - 1
                    rhs = rhs_for(ci)
                    w = wqkv_sb[ci]
                    nc.tensor.matmul(
                        v_ps, lhsT=w[:, 640:768], rhs=rhs, start=first, stop=last
                    )
                    nc.tensor.matmul(
                        k_ps, lhsT=w[:, 512:640], rhs=rhs, start=first, stop=last
                    )
                    for h in range(G):
                        nc.tensor.matmul(
                            q_ps[h],
                            lhsT=w[:, h * 128 : (h + 1) * 128],
                            rhs=rhs,
                            start=first,
                            stop=last,
                        )
                emit_v_post(tb, v_ps)
                emit_rope(tb, kT[tb], k_ps)
                for h in range(G):
                    emit_rope(tb, qT[h][tb], q_ps[h])
            else:
                # per-tensor passes with the DVE consumer emitted right after
                # each pass: RoPE/transposes overlap THIS t-block's remaining
                # projections instead of piling up at the t-block boundary
                passes = [
                    (v_ps, slice(640, 768), lambda: emit_v_post(tb, v_ps)),
                    (k_ps, slice(512, 640), lambda: emit_rope(tb, kT[tb], k_ps)),
                ] + [
                    (
                        q_ps[h],
                        slice(h * 128, (h + 1) * 128),
                        (lambda h=h: emit_rope(tb, qT[h][tb], q_ps[h])),
                    )
                    for h in range(G)
                ]
                for ps_bank, wsl, post in passes:
                    for ci in range(NCT):
                        nc.tensor.matmul(
                            ps_bank,
                            lhsT=wqkv_sb[ci][:, wsl],
                            rhs=rhs_for(ci),
                            start=ci == 0,
                            stop=ci == NCT - 1,
                        )
                    post()

        # ---- stages 2+3: one global software pipeline over (tb, head, s-tile)
        # so head/t-block boundaries never drain the PE. Scores run LA s-tiles
        # ahead of the dependent den/PV matmuls (exp latency hidden). ----
        p2sb = tc.alloc_tile_pool(name="p2sb", bufs=6)  # ep tiles (LA+3 live)
        phd = tc.alloc_tile_pool(name="phd", bufs=2)  # per-head den/oT/rcp
        p3sb = tc.alloc_tile_pool(name="p3sb", bufs=3)
        outp = tc.alloc_tile_pool(name="outp", bufs=10)
        state = {"score": 0, "head": 0, "ncopy": 0}
        oT_live: dict = {}

        def emit_oproj(tb):
            oT_sbs = oT_live.pop(tb)
            for tch in range(TB // 128):
                # stage the full [128, C] row block in SBUF and ship it as ONE
                # DMA (4KB contiguous per partition -> fat packets; the tail
                # after the last matmul drains ~4x faster)
                osb = p3sb.tile([128, C], BF16, name="osb", tag="osb")
                for cb in range(C // 512):
                    ops = bank(f"bk{2 + (cb % 2)}")
                    for h in range(G):
                        nc.tensor.matmul(
                            ops,
                            lhsT=oT_sbs[h][:, tch * 128 : (tch + 1) * 128],
                            rhs=wo_sb[h][:, cb * 512 : (cb + 1) * 512],
                            start=h == 0,
                            stop=h == G - 1,
                        )
                    dst = osb[:, cb * 512 : (cb + 1) * 512]
                    # alternate the PSUM->SBUF copies between ScalarE and DVE
                    # so neither queue builds a backlog
                    if state["ncopy"] % 2 == 0:
                        nc.scalar.copy(dst, ops)
                    else:
                        nc.vector.tensor_copy(dst, ops)
                    state["ncopy"] += 1
                t0 = tb * TB + tch * 128
                # 4 partition-range DMAs: parallel queues AND 4KB packets
                for q in range(4):
                    nc.sync.dma_start(
                        out=out_d.ap()[t0 + q * 32 : t0 + (q + 1) * 32, :],
                        in_=osb[q * 32 : (q + 1) * 32, :],
                    )

        items = []  # (tb, h, idx)
        for tb in range(NTB):
            for h in range(G):
                for idx in range(len(plan[tb])):
                    items.append((tb, h, idx))

        ctx: dict = {}  # (tb,h) -> dict with oT_ps, den, eps

        def emit_score(it):
            tb, h, idx = it
            entries = plan[tb]
            s, w0, kind, mid = entries[idx]
            # bank roles chosen so stage-2 tiles reuse the PSUM banks that
            # stage-1's trailing (tb=3) DVE stream releases earliest:
            # v (bk5) and vtp (bk6/7) first -> stp; k (bk4) -> den;
            # q0/q1 (bk0/1) -> oT; q2/q3 (bk2/3) -> o_proj accumulators
            if idx == 0:
                ctx[(tb, h)] = {
                    "oT": bank(f"bk{0 + (state['head'] % 2)}"),
                    "den": bank("bk4"),
                    "eps": {},
                }
                state["head"] += 1
            stp = bank(f"bk{5 + (state['score'] % 3)}")
            state["score"] += 1
            diag = kind == DIAG
            nc.tensor.matmul(
                stp[:, w0:],
                lhsT=kT[s // 4][:, (s % 4) * 128 : (s % 4 + 1) * 128],
                rhs=qT[h][tb][:, w0:],
                start=True,
                stop=not diag,
                skip_group_check=diag,
            )
            if diag:
                # additive -512*(s>t) triangular mask folded into the score
                # accumulation on the PE (keeps DVE off the critical path);
                # exp then underflows to ~e-18 which is negligible in den/PV
                nc.tensor.matmul(
                    stp[:, w0 : w0 + ST],
                    lhsT=ident,
                    rhs=trineg,
                    start=False,
                    stop=True,
                    skip_group_check=True,
                )
            ep = p2sb.tile([ST, TB], BF16, name="ep", tag="ep")
            nc.scalar.activation(ep[:, w0:], stp[:, w0:], Exp, scale=SCALE)
            if kind == GEN:
                nc.vector.tensor_mul(
                    ep[:, w0:],
                    ep[:, w0:],
                    msk_sb[:, mid * TB : mid * TB + TB - w0],
                )
            ctx[(tb, h)]["eps"][idx] = ep

        def emit_acc(it):
            tb, h, idx = it
            entries = plan[tb]
            s, w0, kind, mid = entries[idx]
            c = ctx[(tb, h)]
            ep = c["eps"].pop(idx)
            first, last = idx == 0, idx == len(entries) - 1
            nc.tensor.matmul(
                c["den"][:, w0:],
                lhsT=ones_sb,
                rhs=ep[:, w0:],
                start=first,
                stop=last,
                skip_group_check=True,
            )
            nc.tensor.matmul(
                c["oT"][:, w0:],
                lhsT=vch[s],
                rhs=ep[:, w0:],
                start=first,
                stop=last,
                skip_group_check=True,
            )
            if last:
                # free both PSUM banks via ScalarE copies (short queue) so the
                # PE's WAR on them never waits behind the DVE backlog; the
                # reciprocal + rescale then run on SBUF off the critical path
                den_sb = phd.tile([128, TB], F32, name="den_sb", tag="den_sb")
                nc.scalar.copy(den_sb, c["den"])
                oT_f = phd.tile([128, TB], F32, name="oT_f", tag="oT_f")
                nc.scalar.copy(oT_f, c["oT"])
                rcp = phd.tile([128, TB], F32, name="rcp", tag="rcp")
                # ~51-ULP approx is ample for the softmax denominator
                nc.vector.reciprocal_approx_fast(rcp, den_sb)
                oT_sb = outp.tile([128, TB], BF16, name="oT", tag="oT")
                nc.vector.tensor_mul(oT_sb, oT_f, rcp)
                oT_live.setdefault(tb, []).append(oT_sb)
                del ctx[(tb, h)]
                # o_proj for t-block tb is emitted two heads LATER (during
                # (tb+1, h1)'s attention) so its lhsT never waits on the
                # rescale chain of tb's last head
                if h == 1 and tb > 0:
                    emit_oproj(tb - 1)
                if tb == NTB - 1 and h == G - 1:
                    emit_oproj(tb)

        from collections import deque

        pend = deque()
        for it in items:
            emit_score(it)
            pend.append(it)
            if len(pend) > LA:
                emit_acc(pend.popleft())
        while pend:
            emit_acc(pend.popleft())

        outp.release()
        p3sb.release()
        phd.release()
        p2sb.release()
        rpool.release()
        ps.release()
        xp.release()
        qkv.release()
        wop.release()
        const.release()

    nc.compile()
    return nc


def _prep_inputs(x, cos, sin, Wq, Wk, Wv, Wo, mask_tiles, n_masks):
    cos = np.asarray(cos, dtype=np.float32).reshape(T, HEAD_DIM // 2)
    sin = np.asarray(sin, dtype=np.float32).reshape(T, HEAD_DIM // 2)
    ctab = np.ascontiguousarray(np.repeat(cos, 2, axis=1).T).astype(BF)  # [128, T]
    s2 = np.repeat(sin, 2, axis=1)
    s2[:, 0::2] *= -1.0
    stab = np.ascontiguousarray(s2.T).astype(BF)
    trineg = (-512.0 * (np.arange(ST)[:, None] > np.arange(ST)[None, :])).astype(BF)

    xTb = [
        np.ascontiguousarray(np.asarray(x[b], dtype=np.float32).T).astype(BF)
        for b in range(B)
    ]
    in_maps = []
    for core in range(8):
        b, g = divmod(core, NUM_KV_HEADS)
        wqkv = np.concatenate(
            [
                Wq[:, g * 512 : (g + 1) * 512],
                Wk[:, g * 128 : (g + 1) * 128],
                Wv[:, g * 128 : (g + 1) * 128],
            ],
            axis=1,
        )
        m = {
            "xT": xTb[b],
            "wqkv": np.ascontiguousarray(wqkv).astype(BF),
            "wo": np.ascontiguousarray(Wo[g * 512 : (g + 1) * 512, :]).astype(BF),
            "ctab": ctab,
            "stab": stab,
            "ones": np.ones((128, 128), dtype=BF),
            "ident": np.eye(128, dtype=BF),
            "tri": trineg,
        }
        if n_masks:
            m["masks"] = mask_tiles.reshape(n_masks * ST, TB).astype(BF)
        in_maps.append(m)
    return in_maps


def kernel(x, cos, sin, mask, Wq, Wk, Wv, Wo, _trace=False, _result_box=None):
    from concourse.bass_utils import run_bass_kernel_spmd

    mask2d = np.asarray(mask).reshape(T, T).astype(bool)
    plan, mask_tiles = _classify_mask(mask2d)
    n_masks = int(mask_tiles.shape[0])

    key = (plan, n_masks)
    nc = _nc_cache.get(key)
    if nc is None:
        nc = _build(plan, n_masks)
        _nc_cache[key] = nc

    in_maps = _prep_inputs(x, cos, sin, Wq, Wk, Wv, Wo, mask_tiles, n_masks)
    res = run_bass_kernel_spmd(nc, in_maps, core_ids=list(range(8)), trace=_trace)
    if _result_box is not None:
        _result_box.append(res)

    out = np.zeros((B, T, C), dtype=np.float32)
    for core in range(8):
        b = core // NUM_KV_HEADS
        out[b] += res.results[core]["out"].astype(np.float32)
    return out


# revision 47
# speedup vs baseline: 2.1826x; 1.0098x over previous
"""Grouped-Query Attention (B=2, T=2048, C=2048, 16 Q heads / 4 KV heads,
D=128) on 8 Trainium2 NeuronCores.

Sharding: core (b, g) for b in {0,1}, g in {0..3} handles batch b and KV head
g (= query heads 4g..4g+3). Each core computes its 4 heads' attention plus the
partial output projection against its 512-row slice of Wo; the host sums the
4 partials per batch (the "all-reduce" of the o_proj, done in numpy).

All matmul operands are bf16 (host-cast); PSUM accumulation stays fp32, so
the only precision loss is input rounding (~4e-3 rel err vs the 2e-2 gate).

Layout/scheduling notes (from trace analysis):
- One PSUM pool with 8 [128,512]-f32 bank tags reused across stages (no
  mid-kernel pool releases -> no cross-stage drain bubbles; the PE pstate
  ramp resets on idle gaps, so a dense PE queue is worth ~1.5x clock).
- Startup DMAs interleaved per contraction chunk (wq/wk/wv/x) so the first
  projection matmul unblocks after ~4 transfers instead of all weights.
- Softmax denominator is computed REPLICATED across all 128 partitions
  (lhsT = all-ones [128,128]) so the reciprocal runs as a full-width DVE op
  (~0.65us) instead of a 1-partition op (3.3us) + GpSimd partition
  broadcast; the per-head tail stall on the PE disappears.
- Stage-2 software pipeline: score matmuls run 2 s-tiles ahead of the
  dependent den/PV matmuls so the PE never waits on ScalarE's exp.
- Diagonal (causal-boundary) s-tiles only compute the t-window right of the
  diagonal plus one shared [128,128] triangular 0/1 multiply.
"""
import sys

sys.path.insert(0, "/opt/trn_rl_repo")

import numpy as np
import ml_dtypes

B, T, C = 2, 2048, 2048
NUM_HEADS, NUM_KV_HEADS, HEAD_DIM = 16, 4, 128
G = NUM_HEADS // NUM_KV_HEADS  # 4 query heads per core
SCALE = float(HEAD_DIM) ** -0.5
TB = 512  # t-block (matmul moving free dim)
NTB = T // TB  # 4
ST = 128  # s-tile
NST = T // ST  # 16
NCT = C // 128  # 16 contraction tiles
LA = 3  # stage-2 score-matmul lookahead (s-tiles in flight past exp)

SWAP_MASK = [i ^ 1 for i in range(32)]
BF = ml_dtypes.bfloat16

_nc_cache: dict = {}

# plan entry kinds
FULL, DIAG, GEN = 0, 1, 2


def _classify_mask(mask2d: np.ndarray):
    """mask2d[t, s] bool. Returns (plan, mask_tiles).

    plan[tb] = tuple of (s_tile_idx, w0, kind, mask_id). w0 is the t-window
    start within the t-block (columns < w0 are entirely masked for this
    s-tile). kind: FULL (no mask work), DIAG (shared lower-triangular 0/1
    multiply on the first 128 window columns), GEN (per-tile 0/1 multiply
    over the whole window; mask_id indexes mask_tiles)."""
    tri = (np.arange(ST)[:, None] <= np.arange(ST)[None, :])
    plan = []
    uniq: dict = {}
    tiles = []
    for tb in range(NTB):
        sub_t = mask2d[tb * TB : (tb + 1) * TB]  # [TB(t), T(s)]
        entries = []
        for s in range(NST):
            sub = sub_t[:, s * ST : (s + 1) * ST]  # [TB(t), ST(s)]
            if sub.all():
                entries.append((s, 0, FULL, None))
                continue
            if not sub.any():
                continue
            m = sub.T  # [s, t]
            w0 = 0
            while w0 + ST <= TB and not m[:, w0 : w0 + ST].any():
                w0 += ST
            win = m[:, w0:]
            if (
                win.shape[1] >= ST
                and (win[:, :ST] == tri).all()
                and win[:, ST:].all()
            ):
                entries.append((s, w0, DIAG, None))
                continue
            tile_m = np.zeros((ST, TB), dtype=np.float32)
            tile_m[:, : TB - w0] = win.astype(np.float32)
            key = (w0, tile_m.tobytes())
            mid = uniq.get(key)
            if mid is None:
                mid = len(tiles)
                uniq[key] = mid
                tiles.append(tile_m)
            entries.append((s, w0, GEN, mid))
        plan.append(tuple(entries))
    mask_tiles = (
        np.stack(tiles) if tiles else np.zeros((0, ST, TB), dtype=np.float32)
    )
    return tuple(plan), mask_tiles


def _build(plan, n_masks):
    import concourse.bacc as bacc
    import concourse.mybir as mybir
    import concourse.tile as tile

    F32 = mybir.dt.float32
    BF16 = mybir.dt.bfloat16
    F8 = mybir.dt.float8e4
    Exp = mybir.ActivationFunctionType.Exp

    nc = bacc.Bacc()

    xT_d = nc.declare_dram_parameter("xT", [C, T], BF16, isOutput=False)
    # wqkv = [Wq | Wk | Wv] columns, one DMA per 128-row chunk
    wqkv_d = nc.declare_dram_parameter(
        "wqkv", [C, (G + 2) * HEAD_DIM], BF16, isOutput=False
    )
    wo_d = nc.declare_dram_parameter("wo", [G * HEAD_DIM, C], BF16, isOutput=False)
    on_d = nc.declare_dram_parameter("ones", [128, 2 * 128], F8, isOutput=False)
    id_d = nc.declare_dram_parameter("ident", [128, 128], BF16, isOutput=False)
    tr_d = nc.declare_dram_parameter("tri", [ST, ST], BF16, isOutput=False)
    ct_d = nc.declare_dram_parameter("ctab", [HEAD_DIM, T], BF16, isOutput=False)
    st_d = nc.declare_dram_parameter("stab", [HEAD_DIM, T], BF16, isOutput=False)
    if n_masks:
        mk_d = nc.declare_dram_parameter(
            "masks", [n_masks * ST, TB], BF16, isOutput=False
        )
    out_d = nc.declare_dram_parameter("out", [T, C], BF16, isOutput=True)

    with tile.TileContext(nc) as tc:
        const = tc.alloc_tile_pool(name="const", bufs=1)
        wop = tc.alloc_tile_pool(name="wop", bufs=1)
        qkv = tc.alloc_tile_pool(name="qkv", bufs=1)
        xp = tc.alloc_tile_pool(name="xp", bufs=1)

        # --- interleaved startup DMAs: per-chunk weights + x so the first
        # projection matmuls unblock after a handful of transfers ---
        wqkv_sb = [
            wop.tile([128, (G + 2) * HEAD_DIM], BF16, name=f"wqkv{i}")
            for i in range(NCT)
        ]
        # x split into tb0-slice + rest tiles (separate tiles, deps are
        # tile-granular) so tb0's projection pass only waits on 4.7MB
        # (weights + tb0 x slices), not the full 11MB
        xt0 = [xp.tile([128, TB], BF16, name=f"xt0_{i}") for i in range(NCT)]
        xtr = [xp.tile([128, T - TB], BF16, name=f"xtr{i}") for i in range(NCT)]
        for i in range(NCT):
            sl = slice(i * 128, (i + 1) * 128)
            nc.sync.dma_start(out=wqkv_sb[i], in_=wqkv_d.ap()[sl, :])
            nc.sync.dma_start(out=xt0[i], in_=xT_d.ap()[sl, :TB])
        for i in range(NCT):
            sl = slice(i * 128, (i + 1) * 128)
            nc.sync.dma_start(out=xtr[i], in_=xT_d.ap()[sl, TB:])

        ctab = const.tile([HEAD_DIM, T], BF16, name="ctab")
        stab = const.tile([HEAD_DIM, T], BF16, name="stab")
        nc.sync.dma_start(out=ctab, in_=ct_d.ap())
        nc.sync.dma_start(out=stab, in_=st_d.ap())
        # P (exp output) is fp8e4: den matmuls on full s-tile PAIRS run in
        # DoubleRow mode (2 tiles per 256-cycle matmul); PV reads the fp8 P
        # against bf16 V. exp bias=-2 keeps P <= ~30, far from fp8 max 448.
        ones8 = const.tile([128, 2, 128], F8, name="ones8")
        ident = const.tile([128, 128], BF16, name="ident")
        trineg = const.tile([ST, ST], BF16, name="trineg")
        nbias = const.tile([128, 1], F32, name="nbias")
        nc.gpsimd.memset(nbias, -2.0)
        nc.sync.dma_start(out=ones8, in_=on_d.ap())
        nc.sync.dma_start(out=ident, in_=id_d.ap())
        nc.sync.dma_start(out=trineg, in_=tr_d.ap())
        if n_masks:
            msk_sb = const.tile([ST, n_masks * TB], BF16, name="msk_sb")
            for i in range(n_masks):
                nc.sync.dma_start(
                    out=msk_sb[:, i * TB : (i + 1) * TB],
                    in_=mk_d.ap()[i * ST : (i + 1) * ST, :],
                )
        wo_sb = [wop.tile([128, C], BF16, name=f"wo{h}") for h in range(G)]
        for h in range(G):
            nc.sync.dma_start(out=wo_sb[h], in_=wo_d.ap()[h * 128 : (h + 1) * 128, :])

        # per-t-block tiles (not one [128, T] tile) so stage-2 readers only
        # depend on the t-blocks they actually use — tile-granular dependency
        # tracking would otherwise serialize stage 2 behind ALL RoPE work
        qT = [
            [qkv.tile([128, TB], BF16, name=f"qT{h}_{tb}") for tb in range(NTB)]
            for h in range(G)
        ]
        kT = [qkv.tile([128, TB], BF16, name=f"kT{tb}") for tb in range(NTB)]
        vT = [qkv.tile([128, TB], BF16, name=f"vT{tb}") for tb in range(NTB)]
        vch = [qkv.tile([128, 128], BF16, name=f"v{s}") for s in range(NST)]

        # single PSUM pool: 8 x [128, 512] f32 bank tags, reused across stages
        ps = tc.alloc_tile_pool(name="ps", bufs=1, space="PSUM")

        def bank(tag):
            return ps.tile([128, TB], F32, name=tag, tag=tag)

        rpool = tc.alloc_tile_pool(name="rpool", bufs=3)

        # ---- stage 1: projections + RoPE + v transpose, pipelined per tb ----
        def emit_v_post(tb, v_ps):
            nc.vector.tensor_copy(vT[tb], v_ps)
            for r in range(4):
                s = 4 * tb + r
                vtp = bank(f"bk{6 + (r % 2)}").bitcast(BF16)[:, :128]
                nc.tensor.transpose(vtp, vT[tb][:, r * 128 : (r + 1) * 128], ident)
                nc.vector.tensor_copy(vch[s], vtp)

        def emit_rope(tb, dst, src_ps):
            tsl = slice(tb * TB, (tb + 1) * TB)
            nc.vector.tensor_copy(dst, src_ps)
            swp = rpool.tile([128, TB], BF16, name="swp", tag="swp")
            tmp = rpool.tile([128, TB], BF16, name="tmp", tag="tmp")
            nc.vector.stream_shuffle(swp, dst, SWAP_MASK)
            nc.vector.tensor_mul(tmp, dst, ctab[:, tsl])
            nc.vector.tensor_mul(swp, swp, stab[:, tsl])
            nc.vector.tensor_add(dst, tmp, swp)

        for tb in range(NTB):
            q_ps = [bank(f"bk{h}") for h in range(G)]
            k_ps = bank("bk4")
            v_ps = bank("bk5")

            def rhs_for(ci):
                return (
                    xt0[ci] if tb == 0 else xtr[ci][:, (tb - 1) * TB : tb * TB]
                )

            if tb == 0:
                # ci-major: tb0 is paced by the input DMA, so touch each
                # freshly-arrived chunk with all 6 matmuls at once
                for ci in range(NCT):
                    first, last = ci == 0, ci == NCT - 1
                    rhs = rhs_for(ci)
                    w = wqkv_sb[ci]
                    nc.tensor.matmul(
                        v_ps, lhsT=w[:, 640:768], rhs=rhs, start=first, stop=last
                    )
                    nc.tensor.matmul(
                        k_ps, lhsT=w[:, 512:640], rhs=rhs, start=first, stop=last
                    )
                    for h in range(G):
                        nc.tensor.matmul(
                            q_ps[h],
                            lhsT=w[:, h * 128 : (h + 1) * 128],
                            rhs=rhs,
                            start=first,
                            stop=last,
                        )
                emit_v_post(tb, v_ps)
                emit_rope(tb, kT[tb], k_ps)
                for h in range(G):
                    emit_rope(tb, qT[h][tb], q_ps[h])
            else:
                # per-tensor passes with the DVE consumer emitted right after
                # each pass: RoPE/transposes overlap THIS t-block's remaining
                # projections instead of piling up at the t-block boundary
                passes = [
                    (v_ps, slice(640, 768), lambda: emit_v_post(tb, v_ps)),
                    (k_ps, slice(512, 640), lambda: emit_rope(tb, kT[tb], k_ps)),
                ] + [
                    (
                        q_ps[h],
                        slice(h * 128, (h + 1) * 128),
                        (lambda h=h: emit_rope(tb, qT[h][tb], q_ps[h])),
                    )
                    for h in range(G)
                ]
                for ps_bank, wsl, post in passes:
                    for ci in range(NCT):
                        nc.tensor.matmul(
                            ps_bank,
                            lhsT=wqkv_sb[ci][:, wsl],
                            rhs=rhs_for(ci),
                            start=ci == 0,
                            stop=ci == NCT - 1,
                        )
                    post()

        # ---- stages 2+3: one global software pipeline over (tb, head, s-tile)
        # so head/t-block boundaries never drain the PE. Scores run LA s-tiles
        # ahead of the dependent den/PV matmuls (exp latency hidden). ----
        p2sb = tc.alloc_tile_pool(name="p2sb", bufs=6)  # ep tiles (LA+3 live)
        phd = tc.alloc_tile_pool(name="phd", bufs=2)  # per-head den/oT/rcp
        p3sb = tc.alloc_tile_pool(name="p3sb", bufs=3)
        outp = tc.alloc_tile_pool(name="outp", bufs=10)
        state = {"score": 0, "head": 0, "ncopy": 0}
        oT_live: dict = {}

        def emit_oproj(tb):
            oT_sbs = oT_live.pop(tb)
            for tch in range(TB // 128):
                # stage the full [128, C] row block in SBUF and ship it as ONE
                # DMA (4KB contiguous per partition -> fat packets; the tail
                # after the last matmul drains ~4x faster)
                osb = p3sb.tile([128, C], BF16, name="osb", tag="osb")
                for cb in range(C // 512):
                    ops = bank(f"bk{2 + (cb % 2)}")
                    for h in range(G):
                        nc.tensor.matmul(
                            ops,
                            lhsT=oT_sbs[h][:, tch * 128 : (tch + 1) * 128],
                            rhs=wo_sb[h][:, cb * 512 : (cb + 1) * 512],
                            start=h == 0,
                            stop=h == G - 1,
                        )
                    dst = osb[:, cb * 512 : (cb + 1) * 512]
                    # alternate the PSUM->SBUF copies between ScalarE and DVE
                    # so neither queue builds a backlog
                    if state["ncopy"] % 2 == 0:
                        nc.scalar.copy(dst, ops)
                    else:
                        nc.vector.tensor_copy(dst, ops)
                    state["ncopy"] += 1
                t0 = tb * TB + tch * 128
                # 4 partition-range DMAs: parallel queues AND 4KB packets
                for q in range(4):
                    nc.sync.dma_start(
                        out=out_d.ap()[t0 + q * 32 : t0 + (q + 1) * 32, :],
                        in_=osb[q * 32 : (q + 1) * 32, :],
                    )

        items = []  # (tb, h, idx)
        for tb in range(NTB):
            for h in range(G):
                for idx in range(len(plan[tb])):
                    items.append((tb, h, idx))

        ctx: dict = {}  # (tb,h) -> dict with oT_ps, den, eps

        def emit_score(it):
            tb, h, idx = it
            entries = plan[tb]
            s, w0, kind, mid = entries[idx]
            # bank roles chosen so stage-2 tiles reuse the PSUM banks that
            # stage-1's trailing (tb=3) DVE stream releases earliest:
            # v (bk5) and vtp (bk6/7) first -> stp; k (bk4) -> den;
            # q0/q1 (bk0/1) -> oT; q2/q3 (bk2/3) -> o_proj accumulators
            if idx == 0:
                # den-op schedule: full tiles pair up into DoubleRow den
                # matmuls (emitted at the 2nd member's acc); odd leftover and
                # diag/gen tiles get single fp8 den matmuls
                den_at = {}
                fulls = [i for i, e in enumerate(entries) if e[2] == FULL]
                for a, b2 in zip(fulls[::2], fulls[1::2]):
                    den_at[b2] = "pair"
                if len(fulls) % 2:
                    den_at[fulls[-1]] = "single"
                for i, e in enumerate(entries):
                    if e[2] != FULL:
                        den_at[i] = "single"
                ctx[(tb, h)] = {
                    "oT": bank(f"bk{0 + (state['head'] % 2)}"),
                    "den": bank("bk4"),
                    "eps": {},
                    "plane": 0,
                    "pair": None,
                    "den_at": den_at,
                    "den_last": max(den_at),
                    "den_started": False,
                }
                state["head"] += 1
            stp = bank(f"bk{5 + (state['score'] % 3)}")
            state["score"] += 1
            diag = kind == DIAG
            nc.tensor.matmul(
                stp[:, w0:],
                lhsT=kT[s // 4][:, (s % 4) * 128 : (s % 4 + 1) * 128],
                rhs=qT[h][tb][:, w0:],
                start=True,
                stop=not diag,
                skip_group_check=diag,
            )
            if diag:
                # additive -512*(s>t) triangular mask folded into the score
                # accumulation on the PE (keeps DVE off the critical path);
                # exp then underflows to ~e-18 which is negligible in den/PV
                nc.tensor.matmul(
                    stp[:, w0 : w0 + ST],
                    lhsT=ident,
                    rhs=trineg,
                    start=False,
                    stop=True,
                    skip_group_check=True,
                )
            c = ctx[(tb, h)]
            if kind == FULL:
                if c["plane"] == 0:
                    c["pair"] = p2sb.tile([ST, 2, TB], F8, name="ep2", tag="ep2")
                ep_ap = c["pair"][:, c["plane"], :]
                c["eps"][idx] = ("f", c["pair"], c["plane"])
                c["plane"] ^= 1
            else:
                ep = p2sb.tile([ST, TB], F8, name="epd", tag="epd")
                ep_ap = ep[:, w0:]
                c["eps"][idx] = ("d", ep, w0)
            nc.scalar.activation(ep_ap, stp[:, w0:], Exp, scale=SCALE, bias=nbias)
            if kind == GEN:
                nc.vector.tensor_mul(
                    ep_ap, ep_ap, msk_sb[:, mid * TB : mid * TB + TB - w0]
                )

        from concourse import mybir as _mybir

        def emit_acc(it):
            tb, h, idx = it
            entries = plan[tb]
            s, w0, kind, mid = entries[idx]
            c = ctx[(tb, h)]
            rec = c["eps"].pop(idx)
            first, last = idx == 0, idx == len(entries) - 1
            if rec[0] == "f":
                _, pair, pl = rec
                pv_rhs = pair[:, pl, :]
                pv_out = c["oT"]
            else:
                _, epd, _w0 = rec
                pv_rhs = epd[:, w0:]
                pv_out = c["oT"][:, w0:]
            den_kind = c["den_at"].get(idx)
            if den_kind is not None:
                dstart = not c["den_started"]
                dstop = idx == c["den_last"]
                c["den_started"] = True
                if den_kind == "pair":
                    nc.tensor.matmul(
                        c["den"],
                        lhsT=ones8,
                        rhs=pair,
                        start=dstart,
                        stop=dstop,
                        perf_mode=_mybir.MatmulPerfMode.DoubleRow,
                        skip_group_check=True,
                    )
                elif rec[0] == "f":
                    nc.tensor.matmul(
                        c["den"],
                        lhsT=ones8[:, 0, :],
                        rhs=pair[:, pl, :],
                        start=dstart,
                        stop=dstop,
                        skip_group_check=True,
                    )
                else:
                    nc.tensor.matmul(
                        c["den"][:, w0:],
                        lhsT=ones8[:, 0, :],
                        rhs=epd[:, w0:],
                        start=dstart,
                        stop=dstop,
                        skip_group_check=True,
                    )
            nc.tensor.matmul(
                pv_out,
                lhsT=vch[s],
                rhs=pv_rhs,
                start=first,
                stop=last,
                skip_group_check=True,
            )
            if last:
                # free both PSUM banks via ScalarE copies (short queue) so the
                # PE's WAR on them never waits behind the DVE backlog; the
                # reciprocal + rescale then run on SBUF off the critical path
                den_sb = phd.tile([128, TB], F32, name="den_sb", tag="den_sb")
                nc.scalar.copy(den_sb, c["den"])
                oT_f = phd.tile([128, TB], F32, name="oT_f", tag="oT_f")
                nc.scalar.copy(oT_f, c["oT"])
                rcp = phd.tile([128, TB], F32, name="rcp", tag="rcp")
                # ~51-ULP approx is ample for the softmax denominator
                nc.vector.reciprocal_approx_fast(rcp, den_sb)
                oT_sb = outp.tile([128, TB], BF16, name="oT", tag="oT")
                nc.vector.tensor_mul(oT_sb, oT_f, rcp)
                oT_live.setdefault(tb, []).append(oT_sb)
                del ctx[(tb, h)]
                # o_proj for t-block tb is emitted two heads LATER (during
                # (tb+1, h1)'s attention) so its lhsT never waits on the
                # rescale chain of tb's last head
                if h == 1 and tb > 0:
                    emit_oproj(tb - 1)
                if tb == NTB - 1 and h == G - 1:
                    emit_oproj(tb)

        from collections import deque

        pend = deque()
        for it in items:
            emit_score(it)
            pend.append(it)
            if len(pend) > LA:
                emit_acc(pend.popleft())
        while pend:
            emit_acc(pend.popleft())

        outp.release()
        p3sb.release()
        phd.release()
        p2sb.release()
        rpool.release()
        ps.release()
        xp.release()
        qkv.release()
        wop.release()
        const.release()

    nc.compile()
    return nc


def _prep_inputs(x, cos, sin, Wq, Wk, Wv, Wo, mask_tiles, n_masks):
    cos = np.asarray(cos, dtype=np.float32).reshape(T, HEAD_DIM // 2)
    sin = np.asarray(sin, dtype=np.float32).reshape(T, HEAD_DIM // 2)
    ctab = np.ascontiguousarray(np.repeat(cos, 2, axis=1).T).astype(BF)  # [128, T]
    s2 = np.repeat(sin, 2, axis=1)
    s2[:, 0::2] *= -1.0
    stab = np.ascontiguousarray(s2.T).astype(BF)
    trineg = (-512.0 * (np.arange(ST)[:, None] > np.arange(ST)[None, :])).astype(BF)

    xTb = [
        np.ascontiguousarray(np.asarray(x[b], dtype=np.float32).T).astype(BF)
        for b in range(B)
    ]
    in_maps = []
    for core in range(8):
        b, g = divmod(core, NUM_KV_HEADS)
        wqkv = np.concatenate(
            [
                Wq[:, g * 512 : (g + 1) * 512],
                Wk[:, g * 128 : (g + 1) * 128],
                Wv[:, g * 128 : (g + 1) * 128],
            ],
            axis=1,
        )
        m = {
            "xT": xTb[b],
            "wqkv": np.ascontiguousarray(wqkv).astype(BF),
            "wo": np.ascontiguousarray(Wo[g * 512 : (g + 1) * 512, :]).astype(BF),
            "ctab": ctab,
            "stab": stab,
            "ones": np.ones((128, 2 * 128), dtype=ml_dtypes.float8_e4m3),
            "ident": np.eye(128, dtype=BF),
            "tri": trineg,
        }
        if n_masks:
            m["masks"] = mask_tiles.reshape(n_masks * ST, TB).astype(BF)
        in_maps.append(m)
    return in_maps


def kernel(x, cos, sin, mask, Wq, Wk, Wv, Wo, _trace=False, _result_box=None):
    from concourse.bass_utils import run_bass_kernel_spmd

    mask2d = np.asarray(mask).reshape(T, T).astype(bool)
    plan, mask_tiles = _classify_mask(mask2d)
    n_masks = int(mask_tiles.shape[0])

    key = (plan, n_masks)
    nc = _nc_cache.get(key)
    if nc is None:
        nc = _build(plan, n_masks)
        _nc_cache[key] = nc

    in_maps = _prep_inputs(x, cos, sin, Wq, Wk, Wv, Wo, mask_tiles, n_masks)
    res = run_bass_kernel_spmd(nc, in_maps, core_ids=list(range(8)), trace=_trace)
    if _result_box is not None:
        _result_box.append(res)

    out = np.zeros((B, T, C), dtype=np.float32)
    for core in range(8):
        b = core // NUM_KV_HEADS
        out[b] += res.results[core]["out"].astype(np.float32)
    return out


# revision 48
# speedup vs baseline: 2.1829x; 1.0001x over previous
"""Grouped-Query Attention (B=2, T=2048, C=2048, 16 Q heads / 4 KV heads,
D=128) on 8 Trainium2 NeuronCores.

Sharding: core (b, g) for b in {0,1}, g in {0..3} handles batch b and KV head
g (= query heads 4g..4g+3). Each core computes its 4 heads' attention plus the
partial output projection against its 512-row slice of Wo; the host sums the
4 partials per batch (the "all-reduce" of the o_proj, done in numpy).

All matmul operands are bf16 (host-cast); PSUM accumulation stays fp32, so
the only precision loss is input rounding (~4e-3 rel err vs the 2e-2 gate).

Layout/scheduling notes (from trace analysis):
- One PSUM pool with 8 [128,512]-f32 bank tags reused across stages (no
  mid-kernel pool releases -> no cross-stage drain bubbles; the PE pstate
  ramp resets on idle gaps, so a dense PE queue is worth ~1.5x clock).
- Startup DMAs interleaved per contraction chunk (wq/wk/wv/x) so the first
  projection matmul unblocks after ~4 transfers instead of all weights.
- Softmax denominator is computed REPLICATED across all 128 partitions
  (lhsT = all-ones [128,128]) so the reciprocal runs as a full-width DVE op
  (~0.65us) instead of a 1-partition op (3.3us) + GpSimd partition
  broadcast; the per-head tail stall on the PE disappears.
- Stage-2 software pipeline: score matmuls run 2 s-tiles ahead of the
  dependent den/PV matmuls so the PE never waits on ScalarE's exp.
- Diagonal (causal-boundary) s-tiles only compute the t-window right of the
  diagonal plus one shared [128,128] triangular 0/1 multiply.
"""
import sys

sys.path.insert(0, "/opt/trn_rl_repo")

import numpy as np
import ml_dtypes

B, T, C = 2, 2048, 2048
NUM_HEADS, NUM_KV_HEADS, HEAD_DIM = 16, 4, 128
G = NUM_HEADS // NUM_KV_HEADS  # 4 query heads per core
SCALE = float(HEAD_DIM) ** -0.5
TB = 512  # t-block (matmul moving free dim)
NTB = T // TB  # 4
ST = 128  # s-tile
NST = T // ST  # 16
NCT = C // 128  # 16 contraction tiles
LA = 3  # stage-2 score-matmul lookahead (s-tiles in flight past exp)

SWAP_MASK = [i ^ 1 for i in range(32)]
BF = ml_dtypes.bfloat16

_nc_cache: dict = {}

# plan entry kinds
FULL, DIAG, GEN = 0, 1, 2


def _classify_mask(mask2d: np.ndarray):
    """mask2d[t, s] bool. Returns (plan, mask_tiles).

    plan[tb] = tuple of (s_tile_idx, w0, kind, mask_id). w0 is the t-window
    start within the t-block (columns < w0 are entirely masked for this
    s-tile). kind: FULL (no mask work), DIAG (shared lower-triangular 0/1
    multiply on the first 128 window columns), GEN (per-tile 0/1 multiply
    over the whole window; mask_id indexes mask_tiles)."""
    tri = (np.arange(ST)[:, None] <= np.arange(ST)[None, :])
    plan = []
    uniq: dict = {}
    tiles = []
    for tb in range(NTB):
        sub_t = mask2d[tb * TB : (tb + 1) * TB]  # [TB(t), T(s)]
        entries = []
        for s in range(NST):
            sub = sub_t[:, s * ST : (s + 1) * ST]  # [TB(t), ST(s)]
            if sub.all():
                entries.append((s, 0, FULL, None))
                continue
            if not sub.any():
                continue
            m = sub.T  # [s, t]
            w0 = 0
            while w0 + ST <= TB and not m[:, w0 : w0 + ST].any():
                w0 += ST
            win = m[:, w0:]
            if (
                win.shape[1] >= ST
                and (win[:, :ST] == tri).all()
                and win[:, ST:].all()
            ):
                entries.append((s, w0, DIAG, None))
                continue
            tile_m = np.zeros((ST, TB), dtype=np.float32)
            tile_m[:, : TB - w0] = win.astype(np.float32)
            key = (w0, tile_m.tobytes())
            mid = uniq.get(key)
            if mid is None:
                mid = len(tiles)
                uniq[key] = mid
                tiles.append(tile_m)
            entries.append((s, w0, GEN, mid))
        plan.append(tuple(entries))
    mask_tiles = (
        np.stack(tiles) if tiles else np.zeros((0, ST, TB), dtype=np.float32)
    )
    return tuple(plan), mask_tiles


def _build(plan, n_masks):
    import concourse.bacc as bacc
    import concourse.mybir as mybir
    import concourse.tile as tile

    F32 = mybir.dt.float32
    BF16 = mybir.dt.bfloat16
    Exp = mybir.ActivationFunctionType.Exp

    nc = bacc.Bacc()

    xT_d = nc.declare_dram_parameter("xT", [C, T], BF16, isOutput=False)
    # wqkv = [Wq | Wk | Wv] columns, one DMA per 128-row chunk
    wqkv_d = nc.declare_dram_parameter(
        "wqkv", [C, (G + 2) * HEAD_DIM], BF16, isOutput=False
    )
    wo_d = nc.declare_dram_parameter("wo", [G * HEAD_DIM, C], BF16, isOutput=False)
    on_d = nc.declare_dram_parameter("ones", [128, 128], BF16, isOutput=False)
    id_d = nc.declare_dram_parameter("ident", [128, 128], BF16, isOutput=False)
    tr_d = nc.declare_dram_parameter("tri", [ST, ST], BF16, isOutput=False)
    ct_d = nc.declare_dram_parameter("ctab", [HEAD_DIM, T], BF16, isOutput=False)
    st_d = nc.declare_dram_parameter("stab", [HEAD_DIM, T], BF16, isOutput=False)
    if n_masks:
        mk_d = nc.declare_dram_parameter(
            "masks", [n_masks * ST, TB], BF16, isOutput=False
        )
    out_d = nc.declare_dram_parameter("out", [T, C], BF16, isOutput=True)

    with tile.TileContext(nc) as tc:
        const = tc.alloc_tile_pool(name="const", bufs=1)
        wop = tc.alloc_tile_pool(name="wop", bufs=1)
        qkv = tc.alloc_tile_pool(name="qkv", bufs=1)
        xp = tc.alloc_tile_pool(name="xp", bufs=1)

        # --- interleaved startup DMAs: per-chunk weights + x so the first
        # projection matmuls unblock after a handful of transfers ---
        wqkv_sb = [
            wop.tile([128, (G + 2) * HEAD_DIM], BF16, name=f"wqkv{i}")
            for i in range(NCT)
        ]
        # x split into tb0-slice + rest tiles (separate tiles, deps are
        # tile-granular) so tb0's projection pass only waits on 4.7MB
        # (weights + tb0 x slices), not the full 11MB
        xt0 = [xp.tile([128, TB], BF16, name=f"xt0_{i}") for i in range(NCT)]
        xtr = [xp.tile([128, T - TB], BF16, name=f"xtr{i}") for i in range(NCT)]
        for i in range(NCT):
            sl = slice(i * 128, (i + 1) * 128)
            nc.sync.dma_start(out=wqkv_sb[i], in_=wqkv_d.ap()[sl, :])
            nc.sync.dma_start(out=xt0[i], in_=xT_d.ap()[sl, :TB])
        for i in range(NCT):
            sl = slice(i * 128, (i + 1) * 128)
            nc.sync.dma_start(out=xtr[i], in_=xT_d.ap()[sl, TB:])

        ctab = const.tile([HEAD_DIM, T], BF16, name="ctab")
        stab = const.tile([HEAD_DIM, T], BF16, name="stab")
        nc.sync.dma_start(out=ctab, in_=ct_d.ap())
        nc.sync.dma_start(out=stab, in_=st_d.ap())
        ones_sb = const.tile([128, 128], BF16, name="ones_sb")
        ident = const.tile([128, 128], BF16, name="ident")
        trineg = const.tile([ST, ST], BF16, name="trineg")
        nc.sync.dma_start(out=ones_sb, in_=on_d.ap())
        nc.sync.dma_start(out=ident, in_=id_d.ap())
        nc.sync.dma_start(out=trineg, in_=tr_d.ap())
        if n_masks:
            msk_sb = const.tile([ST, n_masks * TB], BF16, name="msk_sb")
            for i in range(n_masks):
                nc.sync.dma_start(
                    out=msk_sb[:, i * TB : (i + 1) * TB],
                    in_=mk_d.ap()[i * ST : (i + 1) * ST, :],
                )
        wo_sb = [wop.tile([128, C], BF16, name=f"wo{h}") for h in range(G)]
        for h in range(G):
            nc.sync.dma_start(out=wo_sb[h], in_=wo_d.ap()[h * 128 : (h + 1) * 128, :])

        # per-t-block tiles (not one [128, T] tile) so stage-2 readers only
        # depend on the t-blocks they actually use — tile-granular dependency
        # tracking would otherwise serialize stage 2 behind ALL RoPE work
        qT = [
            [qkv.tile([128, TB], BF16, name=f"qT{h}_{tb}") for tb in range(NTB)]
            for h in range(G)
        ]
        kT = [qkv.tile([128, TB], BF16, name=f"kT{tb}") for tb in range(NTB)]
        vT = [qkv.tile([128, TB], BF16, name=f"vT{tb}") for tb in range(NTB)]
        vch = [qkv.tile([128, 128], BF16, name=f"v{s}") for s in range(NST)]

        # single PSUM pool: 8 x [128, 512] f32 bank tags, reused across stages
        ps = tc.alloc_tile_pool(name="ps", bufs=1, space="PSUM")

        def bank(tag):
            return ps.tile([128, TB], F32, name=tag, tag=tag)

        rpool = tc.alloc_tile_pool(name="rpool", bufs=3)

        # ---- stage 1: projections + RoPE + v transpose, pipelined per tb ----
        def emit_v_post(tb, v_ps):
            nc.vector.tensor_copy(vT[tb], v_ps)
            for r in range(4):
                s = 4 * tb + r
                vtp = bank(f"bk{6 + (r % 2)}").bitcast(BF16)[:, :128]
                nc.tensor.transpose(vtp, vT[tb][:, r * 128 : (r + 1) * 128], ident)
                nc.vector.tensor_copy(vch[s], vtp)

        def emit_rope(tb, dst, src_ps):
            tsl = slice(tb * TB, (tb + 1) * TB)
            nc.vector.tensor_copy(dst, src_ps)
            swp = rpool.tile([128, TB], BF16, name="swp", tag="swp")
            tmp = rpool.tile([128, TB], BF16, name="tmp", tag="tmp")
            nc.vector.stream_shuffle(swp, dst, SWAP_MASK)
            nc.vector.tensor_mul(tmp, dst, ctab[:, tsl])
            nc.vector.tensor_mul(swp, swp, stab[:, tsl])
            nc.vector.tensor_add(dst, tmp, swp)

        for tb in range(NTB):
            q_ps = [bank(f"bk{h}") for h in range(G)]
            k_ps = bank("bk4")
            v_ps = bank("bk5")

            def rhs_for(ci):
                return (
                    xt0[ci] if tb == 0 else xtr[ci][:, (tb - 1) * TB : tb * TB]
                )

            if tb == 0:
                # ci-major: tb0 is paced by the input DMA, so touch each
                # freshly-arrived chunk with all 6 matmuls at once
                for ci in range(NCT):
                    first, last = ci == 0, ci == NCT - 1
                    rhs = rhs_for(ci)
                    w = wqkv_sb[ci]
                    nc.tensor.matmul(
                        v_ps, lhsT=w[:, 640:768], rhs=rhs, start=first, stop=last
                    )
                    nc.tensor.matmul(
                        k_ps, lhsT=w[:, 512:640], rhs=rhs, start=first, stop=last
                    )
                    for h in range(G):
                        nc.tensor.matmul(
                            q_ps[h],
                            lhsT=w[:, h * 128 : (h + 1) * 128],
                            rhs=rhs,
                            start=first,
                            stop=last,
                        )
                emit_v_post(tb, v_ps)
                emit_rope(tb, kT[tb], k_ps)
                for h in range(G):
                    emit_rope(tb, qT[h][tb], q_ps[h])
            else:
                # per-tensor passes with the DVE consumer emitted right after
                # each pass: RoPE/transposes overlap THIS t-block's remaining
                # projections instead of piling up at the t-block boundary
                passes = [
                    (v_ps, slice(640, 768), lambda: emit_v_post(tb, v_ps)),
                    (k_ps, slice(512, 640), lambda: emit_rope(tb, kT[tb], k_ps)),
                ] + [
                    (
                        q_ps[h],
                        slice(h * 128, (h + 1) * 128),
                        (lambda h=h: emit_rope(tb, qT[h][tb], q_ps[h])),
                    )
                    for h in range(G)
                ]
                for ps_bank, wsl, post in passes:
                    for ci in range(NCT):
                        nc.tensor.matmul(
                            ps_bank,
                            lhsT=wqkv_sb[ci][:, wsl],
                            rhs=rhs_for(ci),
                            start=ci == 0,
                            stop=ci == NCT - 1,
                        )
                    post()

        # ---- stages 2+3: one global software pipeline over (tb, head, s-tile)
        # so head/t-block boundaries never drain the PE. Scores run LA s-tiles
        # ahead of the dependent den/PV matmuls (exp latency hidden). ----
        p2sb = tc.alloc_tile_pool(name="p2sb", bufs=6)  # ep tiles (LA+3 live)
        phd = tc.alloc_tile_pool(name="phd", bufs=2)  # per-head den/oT/rcp
        p3sb = tc.alloc_tile_pool(name="p3sb", bufs=3)
        outp = tc.alloc_tile_pool(name="outp", bufs=10)
        state = {"score": 0, "head": 0, "ncopy": 0}
        oT_live: dict = {}

        def emit_oproj(tb):
            oT_sbs = oT_live.pop(tb)
            for tch in range(TB // 128):
                # stage the full [128, C] row block in SBUF and ship it as ONE
                # DMA (4KB contiguous per partition -> fat packets; the tail
                # after the last matmul drains ~4x faster)
                osb = p3sb.tile([128, C], BF16, name="osb", tag="osb")
                for cb in range(C // 512):
                    ops = bank(f"bk{2 + (cb % 2)}")
                    for h in range(G):
                        nc.tensor.matmul(
                            ops,
                            lhsT=oT_sbs[h][:, tch * 128 : (tch + 1) * 128],
                            rhs=wo_sb[h][:, cb * 512 : (cb + 1) * 512],
                            start=h == 0,
                            stop=h == G - 1,
                        )
                    dst = osb[:, cb * 512 : (cb + 1) * 512]
                    # alternate the PSUM->SBUF copies between ScalarE and DVE
                    # so neither queue builds a backlog
                    if state["ncopy"] % 2 == 0:
                        nc.scalar.copy(dst, ops)
                    else:
                        nc.vector.tensor_copy(dst, ops)
                    state["ncopy"] += 1
                t0 = tb * TB + tch * 128
                # 4 partition-range DMAs: parallel queues AND 4KB packets
                for q in range(4):
                    nc.sync.dma_start(
                        out=out_d.ap()[t0 + q * 32 : t0 + (q + 1) * 32, :],
                        in_=osb[q * 32 : (q + 1) * 32, :],
                    )

        items = []  # (tb, h, idx)
        for tb in range(NTB):
            for h in range(G):
                for idx in range(len(plan[tb])):
                    items.append((tb, h, idx))

        ctx: dict = {}  # (tb,h) -> dict with oT_ps, den, eps

        def emit_score(it):
            tb, h, idx = it
            entries = plan[tb]
            s, w0, kind, mid = entries[idx]
            # bank roles chosen so stage-2 tiles reuse the PSUM banks that
            # stage-1's trailing (tb=3) DVE stream releases earliest:
            # v (bk5) and vtp (bk6/7) first -> stp; k (bk4) -> den;
            # q0/q1 (bk0/1) -> oT; q2/q3 (bk2/3) -> o_proj accumulators
            if idx == 0:
                ctx[(tb, h)] = {
                    "oT": bank(f"bk{0 + (state['head'] % 2)}"),
                    "den": bank("bk4"),
                    "eps": {},
                }
                state["head"] += 1
            stp = bank(f"bk{5 + (state['score'] % 3)}")
            state["score"] += 1
            diag = kind == DIAG
            nc.tensor.matmul(
                stp[:, w0:],
                lhsT=kT[s // 4][:, (s % 4) * 128 : (s % 4 + 1) * 128],
                rhs=qT[h][tb][:, w0:],
                start=True,
                stop=not diag,
                skip_group_check=diag,
            )
            if diag:
                # additive -512*(s>t) triangular mask folded into the score
                # accumulation on the PE (keeps DVE off the critical path);
                # exp then underflows to ~e-18 which is negligible in den/PV
                nc.tensor.matmul(
                    stp[:, w0 : w0 + ST],
                    lhsT=ident,
                    rhs=trineg,
                    start=False,
                    stop=True,
                    skip_group_check=True,
                )
            ep = p2sb.tile([ST, TB], BF16, name="ep", tag="ep")
            nc.scalar.activation(ep[:, w0:], stp[:, w0:], Exp, scale=SCALE)
            if kind == GEN:
                nc.vector.tensor_mul(
                    ep[:, w0:],
                    ep[:, w0:],
                    msk_sb[:, mid * TB : mid * TB + TB - w0],
                )
            ctx[(tb, h)]["eps"][idx] = ep

        def emit_acc(it):
            tb, h, idx = it
            entries = plan[tb]
            s, w0, kind, mid = entries[idx]
            c = ctx[(tb, h)]
            ep = c["eps"].pop(idx)
            first, last = idx == 0, idx == len(entries) - 1
            nc.tensor.matmul(
                c["den"][:, w0:],
                lhsT=ones_sb,
                rhs=ep[:, w0:],
                start=first,
                stop=last,
                skip_group_check=True,
            )
            nc.tensor.matmul(
                c["oT"][:, w0:],
                lhsT=vch[s],
                rhs=ep[:, w0:],
                start=first,
                stop=last,
                skip_group_check=True,
            )
            if last:
                # free both PSUM banks via ScalarE copies (short queue) so the
                # PE's WAR on them never waits behind the DVE backlog; the
                # reciprocal + rescale then run on SBUF off the critical path
                den_sb = phd.tile([128, TB], F32, name="den_sb", tag="den_sb")
                nc.scalar.copy(den_sb, c["den"])
                oT_f = phd.tile([128, TB], F32, name="oT_f", tag="oT_f")
                nc.scalar.copy(oT_f, c["oT"])
                rcp = phd.tile([128, TB], F32, name="rcp", tag="rcp")
                # ~51-ULP approx is ample for the softmax denominator
                nc.vector.reciprocal_approx_fast(rcp, den_sb)
                oT_sb = outp.tile([128, TB], BF16, name="oT", tag="oT")
                nc.vector.tensor_mul(oT_sb, oT_f, rcp)
                oT_live.setdefault(tb, []).append(oT_sb)
                del ctx[(tb, h)]
                # o_proj for t-block tb is emitted two heads LATER (during
                # (tb+1, h1)'s attention) so its lhsT never waits on the
                # rescale chain of tb's last head
                if h == 1 and tb > 0:
                    emit_oproj(tb - 1)
                if tb == NTB - 1 and h == G - 1:
                    emit_oproj(tb)

        from collections import deque

        pend = deque()
        for it in items:
            emit_score(it)
            pend.append(it)
            if len(pend) > LA:
                emit_acc(pend.popleft())
        while pend:
            emit_acc(pend.popleft())

        outp.release()
        p3sb.release()
        phd.release()
        p2sb.release()
        rpool.release()
        ps.release()
        xp.release()
        qkv.release()
        wop.release()
        const.release()

    nc.compile()
    return nc


def _prep_inputs(x, cos, sin, Wq, Wk, Wv, Wo, mask_tiles, n_masks):
    cos = np.asarray(cos, dtype=np.float32).reshape(T, HEAD_DIM // 2)
    sin = np.asarray(sin, dtype=np.float32).reshape(T, HEAD_DIM // 2)
    ctab = np.ascontiguousarray(np.repeat(cos, 2, axis=1).T).astype(BF)  # [128, T]
    s2 = np.repeat(sin, 2, axis=1)
    s2[:, 0::2] *= -1.0
    stab = np.ascontiguousarray(s2.T).astype(BF)
    trineg = (-512.0 * (np.arange(ST)[:, None] > np.arange(ST)[None, :])).astype(BF)

    xTb = [
        np.ascontiguousarray(np.asarray(x[b], dtype=np.float32).T).astype(BF)
        for b in range(B)
    ]
    in_maps = []
    for core in range(8):
        b, g = divmod(core, NUM_KV_HEADS)
        wqkv = np.concatenate(
            [
                Wq[:, g * 512 : (g + 1) * 512],
                Wk[:, g * 128 : (g + 1) * 128],
                Wv[:, g * 128 : (g + 1) * 128],
            ],
            axis=1,
        )
        m = {
            "xT": xTb[b],
            "wqkv": np.ascontiguousarray(wqkv).astype(BF),
            "wo": np.ascontiguousarray(Wo[g * 512 : (g + 1) * 512, :]).astype(BF),
            "ctab": ctab,
            "stab": stab,
            "ones": np.ones((128, 128), dtype=BF),
            "ident": np.eye(128, dtype=BF),
            "tri": trineg,
        }
        if n_masks:
            m["masks"] = mask_tiles.reshape(n_masks * ST, TB).astype(BF)
        in_maps.append(m)
    return in_maps


def kernel(x, cos, sin, mask, Wq, Wk, Wv, Wo, _trace=False, _result_box=None):
    from concourse.bass_utils import run_bass_kernel_spmd

    mask2d = np.asarray(mask).reshape(T, T).astype(bool)
    plan, mask_tiles = _classify_mask(mask2d)
    n_masks = int(mask_tiles.shape[0])

    key = (plan, n_masks)
    nc = _nc_cache.get(key)
    if nc is None:
        nc = _build(plan, n_masks)
        _nc_cache[key] = nc

    in_maps = _prep_inputs(x, cos, sin, Wq, Wk, Wv, Wo, mask_tiles, n_masks)
    res = run_bass_kernel_spmd(nc, in_maps, core_ids=list(range(8)), trace=_trace)
    if _result_box is not None:
        _result_box.append(res)

    out = np.zeros((B, T, C), dtype=np.float32)
    for core in range(8):
        b = core // NUM_KV_HEADS
        out[b] += res.results[core]["out"].astype(np.float32)
    return out
